# revision 10
# baseline (speedup 1.0000x reference)
"""BinaryBasicBlock TRN2 kernel: 8-core batch-parallel, raw Bass.

Reference computation (per core: 8 images, C=64, 56x56):
  y1   = conv3x3(x, sign(w1))            # exact: x = fp16(x) + fp16(residual)
  bin1 = sign((y1 - mu1) * rsqrt(var1+eps) * g1 + b1)   # global batch stats
  y2   = conv3x3(bin1, sign(w2))         # exact (+-1 x +-1 in fp8)
  out  = sign((y2 - mu2) * rsqrt(var2+eps) * g2 + b2 + x)

Batch stats are exact: per-core (sum, sumsq) partials are AllReduced across
the 8 cores mid-kernel (both partition halves stored side by side as [64,4]
so no on-chip cross-partition fold is needed).

Layout: channels on partitions, 2 images per 128 partitions (top/bottom
halves), 4 "slots" of [128, 58, 58] padded images per core.

conv1 runs as 9-tap f16 matmul accumulation with all four 64x64 PE quadrants
streaming four different images concurrently (hi + lo pass for fp32
exactness). conv2 runs on fp8 inputs (bin1 is +-1, exact in e4m3) as
full-128 block-diagonal DoubleRow matmuls: weights hold both partition
halves' 64x64 blocks on the diagonal and each matmul processes a pair of
taps, so 5 matmuls replace 9 per (slot, subchunk).

The PE p-state ramp is kept hot across the conv1->conv2 stats barrier by a
stream of filler matmuls into a scratch PSUM region (the tensor engine
down-clocks after idling, which would slow conv2's first ~3us).

conv2 PSUM evacuation is split: ACT evacuates slot 2q (with sum
accumulation), DVE evacuates slot 2q+1 (tensor_scalar with accum), so the
fp8 conv2 is not ACT-bound. y2 is exact in f16 (integer-valued, |y2|<=576).

Final stage: t = xhi+xlo (f32, precomputed during conv1), then per chunk
one DVE/Pool scalar_tensor_tensor (w = y2*a2 + t) and one ACT Sign with
per-channel bias, written as fp8 (+-1 exact) and stored per slot.

Toolchain constraints honored: raw Bass only, max one semaphore wait per
instruction, single PSUM reader engine per bank, drain-backed semaphore
increments on every cross-engine RAW edge, explicit DVE drains between
dependent vector ops. DoubleRow matmuls keep dst partition base 0 (ISA
constraint s3d3_mm_valid_dst_partition).
"""
import numpy as np
import ml_dtypes
import concourse.bass as bass
import concourse.mybir as mybir
from concourse import bass_utils
from concourse.ap import AP as APcls
from contextlib import ExitStack

F32 = mybir.dt.float32
BF16 = mybir.dt.bfloat16
F16 = mybir.dt.float16
F8 = mybir.dt.float8e4
AF = mybir.ActivationFunctionType
ALU = mybir.AluOpType
DRM = mybir.MatmulPerfMode.DoubleRow

N_CORES = 8
N, C, H, W = 64, 64, 56, 56
IMGS = N // N_CORES          # 8 images per core
SLOTS = IMGS // 2            # 4 slots (2 images per slot)
QG = SLOTS // 2              # 2 quadgroups (4 images each)
HP = H + 2                   # 58 padded
CHROWS = 8                   # output rows per 448-subchunk
CHUNK = CHROWS * W           # 448
NCH = H // CHROWS            # 7 subchunks per image
SUPERS = [(0, 2), (2, 4), (4, 6), (6, 7)]   # subchunk ranges per super-iter
NSUP = len(SUPERS)           # 4 super-iters per quadgroup
ITERS = QG * NSUP            # 8 super-iters per conv
PERIMG = H * W               # 3136
YCOLS = SLOTS * PERIMG       # 12544
SLOTPIX = HP * HP            # 3364
N_TOT = float(N * H * W)     # global batch-stat count
EPS = 1e-5
NF = SLOTS * 2               # 8 final-stage chunks (half-slots of 1568)
PAIRS = [(0, 1), (2, 3), (4, 5), (6, 7), (8, 9)]  # conv2 tap pairs (9=zero)
POOL_J = ()   # final iterations handled by GPSIMD (stt not Pool-legal)

W0_DUMMIES = 30              # PE warmup fillers (initial load latency)
W1_DUMMIES = 92              # PE fillers across the stats1 barrier

DEBUG = False
CC_STUB = False   # replace AllReduce with a local DMA (for TimelineSim)

# row chunks per slot for the staged input loads
ROWCH = [(0, 18), (18, 34), (34, 50), (50, 58)]


def build_bass():
    nc = bass.Bass(trn_type="TRN2", target_bir_lowering=False, debug=False,
                   num_devices=N_CORES)

    d_xhi = nc.dram_tensor("xhi", [128, SLOTS, HP, HP], F16, kind="ExternalInput")
    d_xlo = nc.dram_tensor("xlo", [128, SLOTS, HP, HP], F16, kind="ExternalInput")
    d_wf16 = nc.dram_tensor("wf16", [128, 576], F16, kind="ExternalInput")
    d_w8 = nc.dram_tensor("w8", [128, 1280], F8, kind="ExternalInput")
    d_consts = nc.dram_tensor("consts", [128, 8], F32, kind="ExternalInput")
    d_tq = nc.dram_tensor("tq", [128, YCOLS], F32, kind="ExternalInput")
    d_out = nc.dram_tensor("outp", [128, YCOLS], F8, kind="ExternalOutput")
    db1_in = nc.dram_tensor("db1_in", [64, 4], F32)
    db1_out = nc.dram_tensor("db1_out", [64, 4], F32, addr_space="Shared")
    db2_in = nc.dram_tensor("db2_in", [64, 4], F32)
    db2_out = nc.dram_tensor("db2_out", [64, 4], F32, addr_space="Shared")

    es = ExitStack()
    def sb(name, shape, dt):
        return es.enter_context(nc.sbuf_tensor(name, shape, dt))
    def ps(name, shape, dt):
        return es.enter_context(nc.psum_tensor(name, shape, dt))
    def sem(name):
        return es.enter_context(nc.semaphore(name))

    xhi = sb("xhi_t", [128, SLOTS, HP, HP], F16)
    xlo = sb("xlo_t", [128, SLOTS, HP, HP], F16)
    wf16 = sb("wf16_t", [128, 576], F16)
    w8 = sb("w8_t", [128, 1280], F8)
    consts = sb("consts_t", [128, 8], F32)
    bin1 = sb("bin1_t", [128, SLOTS, HP, HP], F8)
    tq = sb("tq_t", [128, YCOLS], F32)
    y1 = sb("y1_t", [128, YCOLS], F32)
    # y2 (f16) and the fp8 output live in y1's bytes (dead regions by then):
    #   y2v   = f16 cols 0..12543     (y1 f32 cols 0..6271   = slots 0,1)
    #   outv  = f8 cols 25088..37631  (y1 f32 cols 6272..9407 = slots 2,3lo)
    y2v = y1[:].bitcast(F16)
    outv = y1[:].bitcast(F8)
    OUTOFF = 2 * YCOLS
    NPART = 2 * ITERS            # partial columns per conv
    ps1 = sb("ps1", [128, NPART], F32)
    pq1 = sb("pq1", [128, NPART], F32)
    ps2 = sb("ps2", [128, NPART], F32)
    pq2 = sb("pq2", [128, NPART], F32)
    stats1 = sb("stats1", [128, 2], F32)
    stats2 = sb("stats2", [128, 2], F32)
    glob1 = sb("glob1", [128, 8], F32)
    glob2 = sb("glob2", [128, 8], F32)
    scr = sb("scr", [128, 2 * CHUNK], F32)
    scr16 = scr[:].bitcast(F16)
    wbuf = [sb(f"wb{i}", [128, PERIMG // 2], F32) for i in range(2)]
    dscr = sb("dscr", [128, 512], F16)
    pbX = [ps(f"pbX{i}", [128, 1024], F32) for i in range(2)]
    pbY = [ps(f"pbY{i}", [128, 1024], F32) for i in range(2)]

    dsem = sem("dsem")
    s_pe1 = sem("s_pe1"); s_ev1 = sem("s_ev1")
    s_pe2 = sem("s_pe2"); s_ev2 = sem("s_ev2"); s_dv2 = sem("s_dv2")
    s_sg1 = sem("s_sg1"); s_ms = sem("s_ms")
    s_st1 = sem("s_st1"); s_st2 = sem("s_st2"); s_acst = sem("s_acst")
    s_cc = sem("s_cc")
    s_fvd = sem("s_fvd"); s_fvp = sem("s_fvp"); s_fs = sem("s_fs")

    CCV = 16 if CC_STUB else 1

    def ycol(slot, c):
        return slot * PERIMG + c * CHUNK

    HCOLS = PERIMG // 2          # 1568
    FINALS = [(s, h) for s in range(SLOTS) for h in (0, 1)]

    # ---- input load schedule --------------------------------------------
    # list of (sbuf_dst_ap_fn, dram_src_ap_fn); dsem marks derived from index
    loads = []
    def add_load(dst, src):
        loads.append((dst, src))
        return len(loads)  # 1-based count

    n_wf = add_load(wf16[:], d_wf16[:])
    for s in (0, 1):
        add_load(xhi[:, s, 0:18], d_xhi[:, s, 0:18])
    for s in (0, 1):
        add_load(xlo[:, s, 0:18], d_xlo[:, s, 0:18])
    D_S0 = len(loads) * 16
    add_load(w8[:], d_w8[:])
    add_load(consts[:], d_consts[:])
    for (r0, r1) in ROWCH[1:3]:
        for s in (0, 1):
            add_load(xhi[:, s, r0:r1], d_xhi[:, s, r0:r1])
        for s in (0, 1):
            add_load(xlo[:, s, r0:r1], d_xlo[:, s, r0:r1])
        if r1 == 34:
            D_S1 = len(loads) * 16
        else:
            D_S2 = len(loads) * 16
    D_T = {}
    for s in (0, 1):
        add_load(xhi[:, s, 50:58], d_xhi[:, s, 50:58])
        n = add_load(xlo[:, s, 50:58], d_xlo[:, s, 50:58])
        D_T[s] = n * 16
    D_S3 = D_T[1]
    for s in (2, 3):
        add_load(xhi[:, s], d_xhi[:, s])
        n = add_load(xlo[:, s], d_xlo[:, s])
        D_T[s] = n * 16
    D_QG1 = len(loads) * 16
    for s in range(SLOTS):
        add_load(tq[:, s * PERIMG : (s + 1) * PERIMG],
                 d_tq[:, s * PERIMG : (s + 1) * PERIMG])
    D_TQ = len(loads) * 16
    NLOADS = len(loads)
    D_B1ST = (NLOADS + 2) * 16
    D_G1 = (NLOADS + 4) * 16
    D_B2ST = (NLOADS + 6) * 16
    D_G2 = (NLOADS + 8) * 16

    CONV1_GATES = {(0, 0): D_S0, (0, 1): D_S1, (0, 2): D_S2, (0, 3): D_S3,
                   (1, 0): D_QG1}

    with nc.Block() as block:

        @block.sync
        def _(sync):
            for dst, src in loads:
                sync.dma_start(dst, src).then_inc(dsem, 16)
            # stats chains: store half 2 / load half 2 ride on ACT and Pool
            sync.wait_ge(s_st1, 1)
            sync.dma_start(db1_in[:, 0:2], stats1[0:64, 0:2]).then_inc(dsem, 16)
            sync.wait_ge(s_cc, CCV)
            sync.dma_start(glob1[0:64, 0:4], db1_out[:]).then_inc(dsem, 16)
            sync.dma_start(glob1[64:128, 0:4], db1_out[:]).then_inc(dsem, 16)
            sync.wait_ge(s_st2, 1)
            sync.dma_start(db2_in[:, 0:2], stats2[0:64, 0:2]).then_inc(dsem, 16)
            sync.wait_ge(s_cc, 2 * CCV)
            sync.dma_start(glob2[0:64, 0:4], db2_out[:]).then_inc(dsem, 16)
            sync.dma_start(glob2[64:128, 0:4], db2_out[:]).then_inc(dsem, 16)
            # output stores, one per final chunk
            for k in range(NF):
                sl, h = FINALS[k]
                off = sl * PERIMG + h * HCOLS
                sync.wait_ge(s_fs, k + 1)
                sync.dma_start(
                    d_out[:, off : off + HCOLS],
                    outv[:, OUTOFF + off : OUTOFF + off + HCOLS],
                ).then_inc(dsem, 16)

        @block.tensor
        def _(tensor):
            def dummy(n):
                # keep the PE p-state hot: harmless f16 matmuls into a
                # region of pbX[0] that is dead at every dummy site
                for i in range(n):
                    nc.tensor.ldweights(dscr[:, 0:64], tile_position=(0, 0))
                    nc.tensor.matmul(pbX[0][0:64, 0:448], dscr[:, 0:64],
                                     dscr[:, 64:512], start=True, stop=True,
                                     tile_position=(0, 0),
                                     skip_group_check=True)

            tensor.wait_ge(s_ms, 1)
            dummy(W0_DUMMIES)

            # conv1: f16 hi/lo, 4 quadrants (4 images concurrent on HW)
            it = 0
            for q in range(QG):
                for si, (c0, c1) in enumerate(SUPERS):
                    gate = CONV1_GATES.get((q, si))
                    if gate is not None:
                        tensor.wait_ge(dsem, gate)
                    nsub = c1 - c0
                    if it >= 2:
                        tensor.wait_ge(s_ev1, it - 1)
                    pX = pbX[it % 2]
                    pY = pbY[it % 2]
                    quads = [
                        ((0, 0), slice(0, 64), 2 * q, pX, slice(0, 64)),
                        ((64, 0), slice(64, 128), 2 * q, pY, slice(0, 64)),
                        ((0, 64), slice(0, 64), 2 * q + 1, pX, slice(64, 128)),
                        ((64, 64), slice(64, 128), 2 * q + 1, pY,
                         slice(64, 128)),
                    ]
                    for tap in range(9):
                        kh, kw = tap // 3, tap % 3
                        wcol = tap * 64
                        for tp, rows, _, _, _ in quads:
                            nc.tensor.ldweights(wf16[rows, wcol : wcol + 64],
                                                tile_position=tp)
                        for rhs_t in (xhi, xlo):
                            ip = 0 if rhs_t is xhi else 1
                            for tp, rows, dslot, pdst, phalf in quads:
                                for s in range(nsub):
                                    c = c0 + s
                                    first = ip == 0 and tap == 0
                                    last = ip == 1 and tap == 8
                                    rap = rhs_t[rows, dslot,
                                                c * CHROWS + kh :
                                                c * CHROWS + kh + CHROWS,
                                                kw : kw + W]
                                    nc.tensor.matmul(
                                        pdst[phalf, s * 512 : s * 512 + CHUNK],
                                        wf16[rows, wcol : wcol + 64], rap,
                                        start=first, stop=last,
                                        tile_position=tp,
                                        skip_group_check=True)
                    tensor.drain().then_inc(s_pe1, 1)
                    it += 1

            # fill the stats1 -> bin1 barrier (evac of it=14 must be done
            # before reusing pbX[0]; evac15 targets pbX[1]/pbY[1])
            tensor.wait_ge(s_ev1, ITERS - 1)
            dummy(W1_DUMMIES)

            # conv2: fp8 block-diagonal DoubleRow, 5 tap-pairs
            it = 0
            for q in range(QG):
                tensor.wait_ge(s_sg1, 2 if q == 0 else 4)
                for si, (c0, c1) in enumerate(SUPERS):
                    nsub = c1 - c0
                    if it >= 2:
                        tensor.wait_ge(s_ev2, it - 1)
                        tensor.wait_ge(s_dv2, it - 1)
                    pX = pbX[it % 2]
                    pY = pbY[it % 2]
                    for ip, (ta, tb) in enumerate(PAIRS):
                        kha, kwa = ta // 3, ta % 3
                        if tb == 9:
                            delta = -58  # zero weights; any in-bounds window
                        else:
                            delta = (tb // 3 - kha) * HP + (tb % 3 - kwa)
                        wap = APcls(tensor=w8[:].tensor, offset=ta * 128,
                                    ap=[[1280, 128], [128, 2], [1, 128]])
                        nc.tensor.ldweights(wap, perf_mode=DRM)
                        for sj in range(2):
                            slot = 2 * q + sj
                            pdst = pX if sj == 0 else pY
                            for s in range(nsub):
                                c = c0 + s
                                offa = (slot * SLOTPIX
                                        + (c * CHROWS + kha) * HP + kwa)
                                rap = APcls(
                                    tensor=bin1[:].tensor, offset=offa,
                                    ap=[[SLOTS * SLOTPIX, 128], [delta, 2],
                                        [HP, CHROWS], [1, W]])
                                nc.tensor.matmul(
                                    pdst[:, s * 512 : s * 512 + CHUNK],
                                    wap, rap, start=(ip == 0), stop=(ip == 4),
                                    perf_mode=DRM, skip_group_check=True)
                    tensor.drain().then_inc(s_pe2, 1)
                    it += 1

        @block.scalar
        def _(scalar):
            # conv1 evacs: PSUM -> y1 (f32) with sum accumulation
            it = 0
            for q in range(QG):
                for (c0, c1) in SUPERS:
                    nsub = c1 - c0
                    scalar.wait_ge(s_pe1, it + 1)
                    pX = pbX[it % 2]
                    pY = pbY[it % 2]
                    for half, slot, pt in ((0, 2 * q, pX), (1, 2 * q + 1, pY)):
                        src = pt[:, 0 : nsub * 512].rearrange(
                            "p (s k) -> p s k", s=nsub)[:, :, 0:CHUNK]
                        nc.scalar.activation(
                            y1[:, ycol(slot, c0) :
                               ycol(slot, c0) + nsub * CHUNK],
                            src, AF.Copy,
                            accum_out=ps1[:, 2 * it + half :
                                          2 * it + half + 1])
                    scalar.drain().then_inc(s_ev1, 1)
                    it += 1
            # stats1: store the bottom half's partials, then sqrt(var+eps)
            scalar.wait_ge(s_st1, 1)
            nc.scalar.dma_start(db1_in[:, 2:4],
                                stats1[64:128, 0:2]).then_inc(dsem, 16)
            scalar.wait_ge(s_st1, 2)
            nc.scalar.activation(glob1[:, 2:3], glob1[:, 3:4], AF.Sqrt,
                                 bias=consts[:, 4:5])
            scalar.drain().then_inc(s_acst, 1)
            # bin1 = Sign(y1 * a1 + b1) into padded fp8 slots
            scalar.wait_ge(s_ms, 5)
            scalar.wait_ge(s_st1, 3)
            def sign1(s):
                nc.scalar.activation(
                    bin1[:, s, 1 : 1 + H, 1 : 1 + W],
                    y1[:, s * PERIMG : (s + 1) * PERIMG],
                    AF.Sign, bias=glob1[:, 7:8], scale=glob1[:, 6:7])
                scalar.drain().then_inc(s_sg1, 1)
            sign1(0)
            sign1(1)

            # conv2 evacs of pX (slot 2q) with accum; slots 2,3 signs woven in
            def evac2(itv, c0, nsub, q):
                scalar.wait_ge(s_pe2, itv + 1)
                pX = pbX[itv % 2]
                src = pX[:, 0 : nsub * 512].rearrange(
                    "p (s k) -> p s k", s=nsub)[:, :, 0:CHUNK]
                nc.scalar.activation(
                    y2v[:, ycol(2 * q, c0) : ycol(2 * q, c0) + nsub * CHUNK],
                    src, AF.Copy,
                    accum_out=ps2[:, 2 * itv : 2 * itv + 1])
                scalar.drain().then_inc(s_ev2, 1)

            it = 0
            for q in range(QG):
                for si, (c0, c1) in enumerate(SUPERS):
                    evac2(it, c0, c1 - c0, q)
                    if it == 0:
                        sign1(2)
                    elif it == 1:
                        sign1(3)
                    it += 1
            # stats2: bottom-half store, then sqrt
            scalar.wait_ge(s_st2, 1)
            nc.scalar.dma_start(db2_in[:, 2:4],
                                stats2[64:128, 0:2]).then_inc(dsem, 16)
            scalar.wait_ge(s_st2, 2)
            nc.scalar.activation(glob2[:, 2:3], glob2[:, 3:4], AF.Sqrt,
                                 bias=consts[:, 4:5])
            scalar.drain().then_inc(s_acst, 2)
            # final: out = Sign(w + b2'), w produced by DVE/Pool
            for j in range(NF):
                sl, h = FINALS[j]
                off = sl * PERIMG + h * HCOLS
                scalar.wait_ge(s_fvd, j + 1)
                nc.scalar.activation(
                    outv[:, OUTOFF + off : OUTOFF + off + HCOLS],
                    wbuf[j % 2][:, 0:HCOLS], AF.Sign,
                    bias=glob2[:, 7:8])
                scalar.drain().then_inc(s_fs, 1)

        @block.vector
        def _(vector):
            # conv1 sumsq partials
            it = 0
            for q in range(QG):
                for (c0, c1) in SUPERS:
                    nsub = c1 - c0
                    vector.wait_ge(s_ev1, it + 1)
                    for half, slot in ((0, 2 * q), (1, 2 * q + 1)):
                        yc = y1[:, ycol(slot, c0) :
                                ycol(slot, c0) + nsub * CHUNK]
                        nc.vector.scalar_tensor_tensor(
                            out=scr[:, 0 : nsub * CHUNK], in0=yc,
                            scalar=1.0, in1=yc,
                            op0=ALU.mult, op1=ALU.mult,
                            accum_out=pq1[:, 2 * it + half :
                                          2 * it + half + 1])
                    it += 1

            def stats(pstats_s, pstats_q, st, dsem_in, acst_v, statst, g,
                      which):
                nc.vector.drain()
                nc.vector.reduce_sum(statst[:, 0:1], pstats_s[:],
                                     axis=mybir.AxisListType.X)
                nc.vector.reduce_sum(statst[:, 1:2], pstats_q[:],
                                     axis=mybir.AxisListType.X)
                nc.vector.drain().then_inc(st, 1)
                vector.wait_ge(dsem, dsem_in)
                # halves side by side: fold on-partition, then bn math
                nc.vector.tensor_tensor(out=g[:, 4:6], in0=g[:, 0:2],
                                        in1=g[:, 2:4], op=ALU.add)
                nc.vector.drain()
                nc.vector.tensor_scalar_mul(g[:, 0:1], g[:, 4:5], 1.0 / N_TOT)
                nc.vector.tensor_scalar_mul(g[:, 1:2], g[:, 5:6], 1.0 / N_TOT)
                nc.vector.drain()
                nc.vector.tensor_tensor(out=g[:, 2:3], in0=g[:, 0:1],
                                        in1=g[:, 0:1], op=ALU.mult)
                nc.vector.drain()
                nc.vector.tensor_tensor(out=g[:, 3:4], in0=g[:, 1:2],
                                        in1=g[:, 2:3], op=ALU.subtract)
                nc.vector.drain().then_inc(st, 1)
                # ACT: g[:,2:3] = sqrt(g[:,3:4] + eps)
                vector.wait_ge(s_acst, acst_v)
                gcol, bcol = 2 * which, 2 * which + 1
                nc.vector.reciprocal(g[:, 3:4], g[:, 2:3])
                nc.vector.drain()
                nc.vector.tensor_tensor(out=g[:, 6:7], in0=g[:, 3:4],
                                        in1=consts[:, gcol : gcol + 1],
                                        op=ALU.mult)
                nc.vector.drain()
                nc.vector.tensor_tensor(out=g[:, 4:5], in0=g[:, 0:1],
                                        in1=g[:, 6:7], op=ALU.mult)
                nc.vector.drain()
                nc.vector.tensor_tensor(out=g[:, 7:8],
                                        in0=consts[:, bcol : bcol + 1],
                                        in1=g[:, 4:5], op=ALU.subtract)
                nc.vector.drain().then_inc(st, 1)

            stats(ps1, pq1, s_st1, D_G1, 1, stats1, glob1, 0)

            # conv2: DVE evacs pY (slot 2q+1) + both sumsq partials
            it = 0
            for q in range(QG):
                for (c0, c1) in SUPERS:
                    nsub = c1 - c0
                    vector.wait_ge(s_pe2, it + 1)
                    pY = pbY[it % 2]
                    src = pY[:, 0 : nsub * 512].rearrange(
                        "p (s k) -> p s k", s=nsub)[:, :, 0:CHUNK]
                    nc.vector.tensor_scalar(
                        y2v[:, ycol(2 * q + 1, c0) :
                            ycol(2 * q + 1, c0) + nsub * CHUNK],
                        src, 1.0, 0.0, ALU.mult, ALU.add,
                        accum_out=ps2[:, 2 * it + 1 : 2 * it + 2])
                    nc.vector.drain()
                    vector.wait_ge(s_ev2, it + 1)
                    for half, slot in ((0, 2 * q), (1, 2 * q + 1)):
                        yc = y2v[:, ycol(slot, c0) :
                                 ycol(slot, c0) + nsub * CHUNK]
                        nc.vector.scalar_tensor_tensor(
                            out=scr16[:, 0 : nsub * CHUNK], in0=yc,
                            scalar=1.0, in1=yc,
                            op0=ALU.mult, op1=ALU.mult,
                            accum_out=pq2[:, 2 * it + half :
                                          2 * it + half + 1])
                    nc.vector.drain().then_inc(s_dv2, 1)
                    it += 1

            stats(ps2, pq2, s_st2, D_G2, 2, stats2, glob2, 1)

            # final w = y2 * a2 + tq
            vector.wait_ge(dsem, D_TQ)
            for j in range(NF):
                sl, h = FINALS[j]
                off = sl * PERIMG + h * HCOLS
                if j >= 2:
                    vector.wait_ge(s_fs, j - 1)
                nc.vector.scalar_tensor_tensor(
                    out=wbuf[j % 2][:, 0:HCOLS],
                    in0=y2v[:, off : off + HCOLS],
                    scalar=glob2[:, 6:7],
                    in1=tq[:, off : off + HCOLS],
                    op0=ALU.mult, op1=ALU.add)
                nc.vector.drain().then_inc(s_fvd, 1)

        @block.gpsimd
        def _(gpsimd):
            nc.gpsimd.memset(dscr[:], 0).then_inc(s_ms, 1)
            for s in range(SLOTS):
                nc.gpsimd.memset(bin1[:, s], 0).then_inc(s_ms, 1)
            gpsimd.wait_ge(dsem, D_B1ST)
            if CC_STUB:
                nc.gpsimd.dma_start(db1_out[:], db1_in[:]).then_inc(s_cc, 16)
            else:
                nc.gpsimd.collective_compute(
                    "AllReduce", ALU.add, replica_groups=[list(range(N_CORES))],
                    ins=[db1_in[:]], outs=[db1_out[:]]).then_inc(s_cc, 1)
            gpsimd.wait_ge(dsem, D_B2ST)
            if CC_STUB:
                nc.gpsimd.dma_start(db2_out[:], db2_in[:]).then_inc(s_cc, 16)
            else:
                nc.gpsimd.collective_compute(
                    "AllReduce", ALU.add, replica_groups=[list(range(N_CORES))],
                    ins=[db2_in[:]], outs=[db2_out[:]]).then_inc(s_cc, 1)

    return nc


_CACHE = {}


def _get_nc():
    if "nc" not in _CACHE:
        _CACHE["nc"] = build_bass()
    return _CACHE["nc"]


def kernel(x, w1, gamma1, beta1, w2, gamma2, beta2):
    x = np.asarray(x, np.float32)
    w1 = np.asarray(w1, np.float32)
    w2 = np.asarray(w2, np.float32)
    gamma1 = np.asarray(gamma1, np.float32)
    beta1 = np.asarray(beta1, np.float32)
    gamma2 = np.asarray(gamma2, np.float32)
    beta2 = np.asarray(beta2, np.float32)

    f8np = mybir.dt.np(F8)

    # conv1 weights: [tap, cin, cout] -> [cin, tap*cout], rows duplicated
    wb1 = np.where(w1 >= 0, 1.0, -1.0).astype(np.float32)
    wt1 = wb1.transpose(1, 2, 3, 0).reshape(64, 9, 64).reshape(64, 576)
    wf16_np = np.concatenate([wt1, wt1], axis=0).astype(np.float16)

    # conv2 weights: fp8 block-diagonal, 10 taps (tap 9 zero)
    wb2 = np.where(w2 >= 0, 1.0, -1.0).astype(np.float32)
    wt2 = wb2.transpose(1, 2, 3, 0).reshape(64, 9, 64)  # [cin, tap, cout]
    w8_np = np.zeros((128, 1280), np.float32)
    for t in range(9):
        w8_np[0:64, t * 128 : t * 128 + 64] = wt2[:, t, :]
        w8_np[64:128, t * 128 + 64 : t * 128 + 128] = wt2[:, t, :]
    w8_np = w8_np.astype(f8np)

    consts_np = np.zeros((128, 8), np.float32)
    for col, v in enumerate([gamma1, beta1, gamma2, beta2]):
        consts_np[0:64, col] = v
        consts_np[64:128, col] = v
    consts_np[:, 4] = EPS

    in_maps = []
    for k in range(N_CORES):
        xc = x[IMGS * k : IMGS * (k + 1)]            # [8, 64, 56, 56]
        xp = np.zeros((IMGS, C, HP, HP), np.float32)
        xp[:, :, 1 : 1 + H, 1 : 1 + W] = xc
        arr = xp.reshape(SLOTS, 2, C, HP, HP).transpose(1, 2, 0, 3, 4)
        arr = np.ascontiguousarray(arr).reshape(128, SLOTS, HP, HP)
        ahi = arr.astype(np.float16)
        alo = (arr - ahi.astype(np.float32)).astype(np.float16)
        # conv1's quadrant pattern permutes (slot, half): y slot s half h holds
        # image Q(s,h) = 4*(s//2) + s%2 + 2*h. The final residual add needs x
        # in that same arrangement.
        tq_np = np.empty((128, SLOTS, PERIMG), np.float32)
        for s in range(SLOTS):
            for h in (0, 1):
                img = 4 * (s // 2) + (s % 2) + 2 * h
                tq_np[h * 64 : h * 64 + 64, s] = xc[img].reshape(C, PERIMG)
        in_maps.append({
            "xhi": ahi, "xlo": alo, "wf16": wf16_np, "w8": w8_np,
            "consts": consts_np, "tq": tq_np.reshape(128, YCOLS),
        })

    nc = _get_nc()
    res = bass_utils.run_bass_kernel_spmd(nc, in_maps, core_ids=list(range(N_CORES)))

    out = np.empty((N, C, H, W), np.float32)
    for k in range(N_CORES):
        o = np.asarray(res.results[k]["outp"]).astype(np.float32)  # [128, 12544]
        o = o.reshape(2, C, SLOTS, H, W)  # [half, ch, slot, H, W]
        for s in range(SLOTS):
            for h in (0, 1):
                img = 4 * (s // 2) + (s % 2) + 2 * h
                out[IMGS * k + img] = o[h, :, s]
    return out


if __name__ == "__main__":
    rng = np.random.default_rng(0)
    xs = rng.standard_normal((N, C, H, W)).astype(np.float32)
    w1s = (rng.standard_normal((C, C, 3, 3)) * 0.1).astype(np.float32)
    w2s = (rng.standard_normal((C, C, 3, 3)) * 0.1).astype(np.float32)
    ones = np.ones(C, np.float32)
    zeros = np.zeros(C, np.float32)
    r = kernel(x=xs, w1=w1s, gamma1=ones, beta1=zeros, w2=w2s, gamma2=ones,
               beta2=zeros)
    print("ran, out uniq:", np.unique(r))


# revision 12
# speedup vs baseline: 1.1947x; 1.1947x over previous
"""BinaryBasicBlock TRN2 kernel: 8-core batch-parallel, raw Bass.

Reference computation (per core: 8 images, C=64, 56x56):
  y1   = conv3x3(x, sign(w1))            # exact: x = fp16(x) + fp16(residual)
  bin1 = sign((y1 - mu1) * rsqrt(var1+eps) * g1 + b1)   # global batch stats
  y2   = conv3x3(bin1, sign(w2))         # exact (+-1 x +-1 in fp8)
  out  = sign((y2 - mu2) * rsqrt(var2+eps) * g2 + b2 + x)

Batch stats are exact: per-core (sum, sumsq) partials are AllReduced across
the 8 cores mid-kernel (both partition halves stored side by side as [64,4]
so no on-chip cross-partition fold is needed).

Layout: channels on partitions, 2 images per 128 partitions (top/bottom
halves), 4 "slots" of [128, 58, 58] padded images per core.

conv1 runs as 9-tap f16 matmul accumulation with all four 64x64 PE quadrants
streaming four different images concurrently (hi + lo pass for fp32
exactness). conv2 runs on fp8 inputs (bin1 is +-1, exact in e4m3) as
full-128 block-diagonal DoubleRow matmuls: weights hold both partition
halves' 64x64 blocks on the diagonal and each matmul processes a pair of
taps, so 5 matmuls replace 9 per (slot, subchunk).

The PE p-state ramp is kept hot across the conv1->conv2 stats barrier by a
stream of filler matmuls into a scratch PSUM region (the tensor engine
down-clocks after idling, which would slow conv2's first ~3us).

conv2 PSUM evacuation is split: ACT evacuates slot 2q (with sum
accumulation), DVE evacuates slot 2q+1 (tensor_scalar with accum), so the
fp8 conv2 is not ACT-bound. y2 is exact in f16 (integer-valued, |y2|<=576).

Final stage: t = xhi+xlo (f32, precomputed during conv1), then per chunk
one DVE/Pool scalar_tensor_tensor (w = y2*a2 + t) and one ACT Sign with
per-channel bias, written as fp8 (+-1 exact) and stored per slot.

Toolchain constraints honored: raw Bass only, max one semaphore wait per
instruction, single PSUM reader engine per bank, drain-backed semaphore
increments on every cross-engine RAW edge, explicit DVE drains between
dependent vector ops. DoubleRow matmuls keep dst partition base 0 (ISA
constraint s3d3_mm_valid_dst_partition).
"""
import numpy as np
import ml_dtypes
import concourse.bass as bass
import concourse.mybir as mybir
from concourse import bass_utils
from concourse.ap import AP as APcls
from contextlib import ExitStack

F32 = mybir.dt.float32
BF16 = mybir.dt.bfloat16
F16 = mybir.dt.float16
F8 = mybir.dt.float8e4
F8E5 = mybir.dt.float8e5
AF = mybir.ActivationFunctionType
ALU = mybir.AluOpType
DRM = mybir.MatmulPerfMode.DoubleRow

N_CORES = 8
N, C, H, W = 64, 64, 56, 56
IMGS = N // N_CORES          # 8 images per core
SLOTS = IMGS // 2            # 4 slots (2 images per slot)
QG = SLOTS // 2              # 2 quadgroups (4 images each)
HP = H + 2                   # 58 padded
CHROWS = 8                   # output rows per 448-subchunk
CHUNK = CHROWS * W           # 448
NCH = H // CHROWS            # 7 subchunks per image
SUPERS = [(0, 2), (2, 4), (4, 6), (6, 7)]   # subchunk ranges per super-iter
NSUP = len(SUPERS)           # 4 super-iters per quadgroup
ITERS = QG * NSUP            # 8 super-iters per conv
PERIMG = H * W               # 3136
YCOLS = SLOTS * PERIMG       # 12544
SLOTPIX = HP * HP            # 3364
N_TOT = float(N * H * W)     # global batch-stat count
EPS = 1e-5
NF = SLOTS * 2               # 8 final-stage chunks (half-slots of 1568)
PAIRS = [(0, 1), (2, 3), (4, 5), (6, 7), (8, 9)]  # conv2 tap pairs (9=zero)
POOL_J = ()   # final iterations handled by GPSIMD (stt not Pool-legal)

W0_DUMMIES = 30              # PE warmup fillers (initial load latency)
W1_DUMMIES = 92              # PE fillers across the stats1 barrier

DEBUG = False
CC_STUB = False   # replace AllReduce with a local DMA (for TimelineSim)

# row chunks per slot for the staged input loads
ROWCH = [(0, 18), (18, 34), (34, 50), (50, 58)]


def build_bass():
    nc = bass.Bass(trn_type="TRN2", target_bir_lowering=False, debug=False,
                   num_devices=N_CORES)

    d_xhi = nc.dram_tensor("xhi", [128, SLOTS, HP, HP], F16, kind="ExternalInput")
    d_xm8 = nc.dram_tensor("xm8", [128, SLOTS, HP, HP], F8, kind="ExternalInput")
    d_xl8 = nc.dram_tensor("xl8", [128, SLOTS, HP, HP], F8, kind="ExternalInput")
    d_wf16 = nc.dram_tensor("wf16", [128, 576], F16, kind="ExternalInput")
    d_w8 = nc.dram_tensor("w8", [128, 1280], F8, kind="ExternalInput")
    d_w1m = nc.dram_tensor("w1m", [128, 1280], F8E5, kind="ExternalInput")
    d_consts = nc.dram_tensor("consts", [128, 8], F32, kind="ExternalInput")
    d_tq = nc.dram_tensor("tq", [128, YCOLS], F32, kind="ExternalInput")
    d_out = nc.dram_tensor("outp", [128, YCOLS], F8, kind="ExternalOutput")
    db1_in = nc.dram_tensor("db1_in", [64, 4], F32)
    db1_out = nc.dram_tensor("db1_out", [64, 4], F32, addr_space="Shared")
    db2_in = nc.dram_tensor("db2_in", [64, 4], F32)
    db2_out = nc.dram_tensor("db2_out", [64, 4], F32, addr_space="Shared")

    es = ExitStack()
    def sb(name, shape, dt):
        return es.enter_context(nc.sbuf_tensor(name, shape, dt))
    def ps(name, shape, dt):
        return es.enter_context(nc.psum_tensor(name, shape, dt))
    def sem(name):
        return es.enter_context(nc.semaphore(name))

    xhi = sb("xhi_t", [128, SLOTS, HP, HP], F16)
    xm8 = sb("xm8_t", [128, SLOTS, HP, HP], F8)
    xl8 = sb("xl8_t", [128, SLOTS, HP, HP], F8)
    wf16 = sb("wf16_t", [128, 576], F16)
    w8 = sb("w8_t", [128, 1280], F8)
    w1m = sb("w1m_t", [128, 1280], F8E5)
    consts = sb("consts_t", [128, 8], F32)
    bin1 = sb("bin1_t", [128, SLOTS, HP, HP], F8)
    tq = sb("tq_t", [128, YCOLS], F32)
    y1 = sb("y1_t", [128, YCOLS], F32)
    # y2 (f16) and the fp8 output live in y1's bytes (dead regions by then):
    #   y2v   = f16 cols 0..12543     (y1 f32 cols 0..6271   = slots 0,1)
    #   outv  = f8 cols 25088..37631  (y1 f32 cols 6272..9407 = slots 2,3lo)
    y2v = y1[:].bitcast(F16)
    outv = y1[:].bitcast(F8)
    OUTOFF = 2 * YCOLS
    NPART = 2 * ITERS            # partial columns per conv
    ps1 = sb("ps1", [128, NPART], F32)
    pq1 = sb("pq1", [128, NPART], F32)
    ps2 = sb("ps2", [128, NPART], F32)
    pq2 = sb("pq2", [128, NPART], F32)
    stats1 = sb("stats1", [128, 2], F32)
    stats2 = sb("stats2", [128, 2], F32)
    glob1 = sb("glob1", [128, 8], F32)
    glob2 = sb("glob2", [128, 8], F32)
    scr = sb("scr", [128, 2 * CHUNK], F32)
    scr16 = scr[:].bitcast(F16)
    wbuf = [sb(f"wb{i}", [128, PERIMG // 2], F32) for i in range(2)]
    dscr = sb("dscr", [128, 512], F16)
    pbX = [ps(f"pbX{i}", [128, 1024], F32) for i in range(2)]
    pbY = [ps(f"pbY{i}", [128, 1024], F32) for i in range(2)]

    dsem = sem("dsem")
    s_pe1 = sem("s_pe1"); s_ev1 = sem("s_ev1")
    s_pe2 = sem("s_pe2"); s_ev2 = sem("s_ev2"); s_dv2 = sem("s_dv2")
    s_sg1 = sem("s_sg1"); s_ms = sem("s_ms")
    s_st1 = sem("s_st1"); s_st2 = sem("s_st2"); s_acst = sem("s_acst")
    s_cc = sem("s_cc")
    s_fvd = sem("s_fvd"); s_fvp = sem("s_fvp"); s_fs = sem("s_fs")

    CCV = 16 if CC_STUB else 1

    def ycol(slot, c):
        return slot * PERIMG + c * CHUNK

    HCOLS = PERIMG // 2          # 1568
    FINALS = [(s, h) for s in range(SLOTS) for h in (0, 1)]

    # ---- input load schedule --------------------------------------------
    # list of (sbuf_dst_ap_fn, dram_src_ap_fn); dsem marks derived from index
    loads = []
    def add_load(dst, src):
        loads.append((dst, src))
        return len(loads)  # 1-based count

    add_load(wf16[:], d_wf16[:])
    add_load(w1m[:], d_w1m[:])
    for t, dt_ in ((xhi, d_xhi), (xm8, d_xm8), (xl8, d_xl8)):
        for s in (0, 1):
            add_load(t[:, s, 0:18], dt_[:, s, 0:18])
    D_S0 = len(loads) * 16
    add_load(w8[:], d_w8[:])
    add_load(consts[:], d_consts[:])
    for (r0, r1) in ROWCH[1:]:
        for t, dt_ in ((xhi, d_xhi), (xm8, d_xm8), (xl8, d_xl8)):
            for s in (0, 1):
                add_load(t[:, s, r0:r1], dt_[:, s, r0:r1])
        if r1 == 34:
            D_S1 = len(loads) * 16
        elif r1 == 50:
            D_S2 = len(loads) * 16
        else:
            D_S3 = len(loads) * 16
    for s in (2, 3):
        for t, dt_ in ((xhi, d_xhi), (xm8, d_xm8), (xl8, d_xl8)):
            add_load(t[:, s], dt_[:, s])
    D_QG1 = len(loads) * 16
    for s in range(SLOTS):
        add_load(tq[:, s * PERIMG : (s + 1) * PERIMG],
                 d_tq[:, s * PERIMG : (s + 1) * PERIMG])
    D_TQ = len(loads) * 16
    NLOADS = len(loads)
    D_B1ST = (NLOADS + 2) * 16
    D_G1 = (NLOADS + 4) * 16
    D_B2ST = (NLOADS + 6) * 16
    D_G2 = (NLOADS + 8) * 16

    CONV1_GATES = {(0, 0): D_S0, (0, 1): D_S1, (0, 2): D_S2, (0, 3): D_S3,
                   (1, 0): D_QG1}

    with nc.Block() as block:

        @block.sync
        def _(sync):
            for dst, src in loads:
                sync.dma_start(dst, src).then_inc(dsem, 16)
            # stats chains: store half 2 / load half 2 ride on ACT and Pool
            sync.wait_ge(s_st1, 1)
            sync.dma_start(db1_in[:, 0:2], stats1[0:64, 0:2]).then_inc(dsem, 16)
            sync.wait_ge(s_cc, CCV)
            sync.dma_start(glob1[0:64, 0:4], db1_out[:]).then_inc(dsem, 16)
            sync.dma_start(glob1[64:128, 0:4], db1_out[:]).then_inc(dsem, 16)
            sync.wait_ge(s_st2, 1)
            sync.dma_start(db2_in[:, 0:2], stats2[0:64, 0:2]).then_inc(dsem, 16)
            sync.wait_ge(s_cc, 2 * CCV)
            sync.dma_start(glob2[0:64, 0:4], db2_out[:]).then_inc(dsem, 16)
            sync.dma_start(glob2[64:128, 0:4], db2_out[:]).then_inc(dsem, 16)
            # output stores, one per final chunk
            for k in range(NF):
                sl, h = FINALS[k]
                off = sl * PERIMG + h * HCOLS
                sync.wait_ge(s_fs, k + 1)
                sync.dma_start(
                    d_out[:, off : off + HCOLS],
                    outv[:, OUTOFF + off : OUTOFF + off + HCOLS],
                ).then_inc(dsem, 16)

        @block.tensor
        def _(tensor):
            def dummy(n):
                # keep the PE p-state hot: harmless f16 matmuls into a
                # region of pbX[0] that is dead at every dummy site
                for i in range(n):
                    nc.tensor.ldweights(dscr[:, 0:64], tile_position=(0, 0))
                    nc.tensor.matmul(pbX[0][0:64, 0:448], dscr[:, 0:64],
                                     dscr[:, 64:512], start=True, stop=True,
                                     tile_position=(0, 0),
                                     skip_group_check=True)

            tensor.wait_ge(s_ms, 1)
            dummy(W0_DUMMIES)

            # conv1: f16 hi/lo, 4 quadrants (4 images concurrent on HW)
            it = 0
            for q in range(QG):
                for si, (c0, c1) in enumerate(SUPERS):
                    gate = CONV1_GATES.get((q, si))
                    if gate is not None:
                        tensor.wait_ge(dsem, gate)
                    nsub = c1 - c0
                    if it >= 2:
                        tensor.wait_ge(s_ev1, it - 1)
                    pX = pbX[it % 2]
                    pY = pbY[it % 2]
                    quads = [
                        ((0, 0), slice(0, 64), 2 * q, pX, slice(0, 64)),
                        ((64, 0), slice(64, 128), 2 * q, pY, slice(0, 64)),
                        ((0, 64), slice(0, 64), 2 * q + 1, pX, slice(64, 128)),
                        ((64, 64), slice(64, 128), 2 * q + 1, pY,
                         slice(64, 128)),
                    ]
                    for tap in range(9):
                        kh, kw = tap // 3, tap % 3
                        wcol = tap * 64
                        for tp, rows, _, _, _ in quads:
                            nc.tensor.ldweights(wf16[rows, wcol : wcol + 64],
                                                tile_position=tp)
                        for tp, rows, dslot, pdst, phalf in quads:
                            for s in range(nsub):
                                c = c0 + s
                                rap = xhi[rows, dslot,
                                          c * CHROWS + kh :
                                          c * CHROWS + kh + CHROWS,
                                          kw : kw + W]
                                nc.tensor.matmul(
                                    pdst[phalf, s * 512 : s * 512 + CHUNK],
                                    wf16[rows, wcol : wcol + 64], rap,
                                    start=(tap == 0), stop=False,
                                    tile_position=tp,
                                    skip_group_check=True)
                    # fp8 mid/lo limbs: block-diagonal DoubleRow, weights
                    # are +-2^-12 so they accumulate into the same groups
                    for li, lt in enumerate((xm8, xl8)):
                        for ip, (ta, tb) in enumerate(PAIRS):
                            kha, kwa = ta // 3, ta % 3
                            if tb == 9:
                                delta = -HP
                            else:
                                delta = (tb // 3 - kha) * HP + (tb % 3 - kwa)
                            wap = APcls(tensor=w1m[:].tensor, offset=ta * 128,
                                        ap=[[1280, 128], [128, 2], [1, 128]])
                            nc.tensor.ldweights(wap, perf_mode=DRM)
                            for sj in range(2):
                                slot = 2 * q + sj
                                pdst = pX if sj == 0 else pY
                                for s in range(nsub):
                                    c = c0 + s
                                    offa = (slot * SLOTPIX
                                            + (c * CHROWS + kha) * HP + kwa)
                                    rap = APcls(
                                        tensor=lt[:].tensor, offset=offa,
                                        ap=[[SLOTS * SLOTPIX, 128], [delta, 2],
                                            [HP, CHROWS], [1, W]])
                                    nc.tensor.matmul(
                                        pdst[:, s * 512 : s * 512 + CHUNK],
                                        wap, rap, start=False,
                                        stop=(li == 1 and ip == 4),
                                        perf_mode=DRM, skip_group_check=True)
                    tensor.drain().then_inc(s_pe1, 1)
                    it += 1

            # fill the stats1 -> bin1 barrier (evac of it=14 must be done
            # before reusing pbX[0]; evac15 targets pbX[1]/pbY[1])
            tensor.wait_ge(s_ev1, ITERS - 1)
            dummy(W1_DUMMIES)

            # conv2: fp8 block-diagonal DoubleRow, 5 tap-pairs
            it = 0
            for q in range(QG):
                tensor.wait_ge(s_sg1, 2 if q == 0 else 4)
                for si, (c0, c1) in enumerate(SUPERS):
                    nsub = c1 - c0
                    if it >= 2:
                        tensor.wait_ge(s_ev2, it - 1)
                        tensor.wait_ge(s_dv2, it - 1)
                    pX = pbX[it % 2]
                    pY = pbY[it % 2]
                    for ip, (ta, tb) in enumerate(PAIRS):
                        kha, kwa = ta // 3, ta % 3
                        if tb == 9:
                            delta = -58  # zero weights; any in-bounds window
                        else:
                            delta = (tb // 3 - kha) * HP + (tb % 3 - kwa)
                        wap = APcls(tensor=w8[:].tensor, offset=ta * 128,
                                    ap=[[1280, 128], [128, 2], [1, 128]])
                        nc.tensor.ldweights(wap, perf_mode=DRM)
                        for sj in range(2):
                            slot = 2 * q + sj
                            pdst = pX if sj == 0 else pY
                            for s in range(nsub):
                                c = c0 + s
                                offa = (slot * SLOTPIX
                                        + (c * CHROWS + kha) * HP + kwa)
                                rap = APcls(
                                    tensor=bin1[:].tensor, offset=offa,
                                    ap=[[SLOTS * SLOTPIX, 128], [delta, 2],
                                        [HP, CHROWS], [1, W]])
                                nc.tensor.matmul(
                                    pdst[:, s * 512 : s * 512 + CHUNK],
                                    wap, rap, start=(ip == 0), stop=(ip == 4),
                                    perf_mode=DRM, skip_group_check=True)
                    tensor.drain().then_inc(s_pe2, 1)
                    it += 1

        @block.scalar
        def _(scalar):
            # conv1 evacs: PSUM -> y1 (f32) with sum accumulation
            it = 0
            for q in range(QG):
                for (c0, c1) in SUPERS:
                    nsub = c1 - c0
                    scalar.wait_ge(s_pe1, it + 1)
                    pX = pbX[it % 2]
                    pY = pbY[it % 2]
                    for half, slot, pt in ((0, 2 * q, pX), (1, 2 * q + 1, pY)):
                        src = pt[:, 0 : nsub * 512].rearrange(
                            "p (s k) -> p s k", s=nsub)[:, :, 0:CHUNK]
                        nc.scalar.activation(
                            y1[:, ycol(slot, c0) :
                               ycol(slot, c0) + nsub * CHUNK],
                            src, AF.Copy,
                            accum_out=ps1[:, 2 * it + half :
                                          2 * it + half + 1])
                    scalar.drain().then_inc(s_ev1, 1)
                    it += 1
            # stats1: store the bottom half's partials, then sqrt(var+eps)
            scalar.wait_ge(s_st1, 1)
            nc.scalar.dma_start(db1_in[:, 2:4],
                                stats1[64:128, 0:2]).then_inc(dsem, 16)
            scalar.wait_ge(s_st1, 2)
            nc.scalar.activation(glob1[:, 2:3], glob1[:, 3:4], AF.Sqrt,
                                 bias=consts[:, 4:5])
            scalar.drain().then_inc(s_acst, 1)
            # bin1 = Sign(y1 * a1 + b1) into padded fp8 slots
            scalar.wait_ge(s_ms, 5)
            scalar.wait_ge(s_st1, 3)
            def sign1(s):
                nc.scalar.activation(
                    bin1[:, s, 1 : 1 + H, 1 : 1 + W],
                    y1[:, s * PERIMG : (s + 1) * PERIMG],
                    AF.Sign, bias=glob1[:, 7:8], scale=glob1[:, 6:7])
                scalar.drain().then_inc(s_sg1, 1)
            sign1(0)
            sign1(1)

            # conv2 evacs of pX (slot 2q) with accum; slots 2,3 signs woven in
            def evac2(itv, c0, nsub, q):
                scalar.wait_ge(s_pe2, itv + 1)
                pX = pbX[itv % 2]
                src = pX[:, 0 : nsub * 512].rearrange(
                    "p (s k) -> p s k", s=nsub)[:, :, 0:CHUNK]
                nc.scalar.activation(
                    y2v[:, ycol(2 * q, c0) : ycol(2 * q, c0) + nsub * CHUNK],
                    src, AF.Copy,
                    accum_out=ps2[:, 2 * itv : 2 * itv + 1])
                scalar.drain().then_inc(s_ev2, 1)

            it = 0
            for q in range(QG):
                for si, (c0, c1) in enumerate(SUPERS):
                    evac2(it, c0, c1 - c0, q)
                    if it == 0:
                        sign1(2)
                    elif it == 1:
                        sign1(3)
                    it += 1
            # stats2: bottom-half store, then sqrt
            scalar.wait_ge(s_st2, 1)
            nc.scalar.dma_start(db2_in[:, 2:4],
                                stats2[64:128, 0:2]).then_inc(dsem, 16)
            scalar.wait_ge(s_st2, 2)
            nc.scalar.activation(glob2[:, 2:3], glob2[:, 3:4], AF.Sqrt,
                                 bias=consts[:, 4:5])
            scalar.drain().then_inc(s_acst, 2)
            # final: out = Sign(w + b2'), w produced by DVE/Pool
            for j in range(NF):
                sl, h = FINALS[j]
                off = sl * PERIMG + h * HCOLS
                scalar.wait_ge(s_fvd, j + 1)
                nc.scalar.activation(
                    outv[:, OUTOFF + off : OUTOFF + off + HCOLS],
                    wbuf[j % 2][:, 0:HCOLS], AF.Sign,
                    bias=glob2[:, 7:8])
                scalar.drain().then_inc(s_fs, 1)

        @block.vector
        def _(vector):
            # conv1 sumsq partials
            it = 0
            for q in range(QG):
                for (c0, c1) in SUPERS:
                    nsub = c1 - c0
                    vector.wait_ge(s_ev1, it + 1)
                    for half, slot in ((0, 2 * q), (1, 2 * q + 1)):
                        yc = y1[:, ycol(slot, c0) :
                                ycol(slot, c0) + nsub * CHUNK]
                        nc.vector.scalar_tensor_tensor(
                            out=scr[:, 0 : nsub * CHUNK], in0=yc,
                            scalar=1.0, in1=yc,
                            op0=ALU.mult, op1=ALU.mult,
                            accum_out=pq1[:, 2 * it + half :
                                          2 * it + half + 1])
                    it += 1

            def stats(pstats_s, pstats_q, st, dsem_in, acst_v, statst, g,
                      which):
                nc.vector.drain()
                nc.vector.reduce_sum(statst[:, 0:1], pstats_s[:],
                                     axis=mybir.AxisListType.X)
                nc.vector.reduce_sum(statst[:, 1:2], pstats_q[:],
                                     axis=mybir.AxisListType.X)
                nc.vector.drain().then_inc(st, 1)
                vector.wait_ge(dsem, dsem_in)
                # halves side by side: fold on-partition, then bn math
                nc.vector.tensor_tensor(out=g[:, 4:6], in0=g[:, 0:2],
                                        in1=g[:, 2:4], op=ALU.add)
                nc.vector.drain()
                nc.vector.tensor_scalar_mul(g[:, 0:1], g[:, 4:5], 1.0 / N_TOT)
                nc.vector.tensor_scalar_mul(g[:, 1:2], g[:, 5:6], 1.0 / N_TOT)
                nc.vector.drain()
                nc.vector.tensor_tensor(out=g[:, 2:3], in0=g[:, 0:1],
                                        in1=g[:, 0:1], op=ALU.mult)
                nc.vector.drain()
                nc.vector.tensor_tensor(out=g[:, 3:4], in0=g[:, 1:2],
                                        in1=g[:, 2:3], op=ALU.subtract)
                nc.vector.drain().then_inc(st, 1)
                # ACT: g[:,2:3] = sqrt(g[:,3:4] + eps)
                vector.wait_ge(s_acst, acst_v)
                gcol, bcol = 2 * which, 2 * which + 1
                nc.vector.reciprocal(g[:, 3:4], g[:, 2:3])
                nc.vector.drain()
                nc.vector.tensor_tensor(out=g[:, 6:7], in0=g[:, 3:4],
                                        in1=consts[:, gcol : gcol + 1],
                                        op=ALU.mult)
                nc.vector.drain()
                nc.vector.tensor_tensor(out=g[:, 4:5], in0=g[:, 0:1],
                                        in1=g[:, 6:7], op=ALU.mult)
                nc.vector.drain()
                nc.vector.tensor_tensor(out=g[:, 7:8],
                                        in0=consts[:, bcol : bcol + 1],
                                        in1=g[:, 4:5], op=ALU.subtract)
                nc.vector.drain().then_inc(st, 1)

            stats(ps1, pq1, s_st1, D_G1, 1, stats1, glob1, 0)

            # conv2: DVE evacs pY (slot 2q+1) + both sumsq partials
            it = 0
            for q in range(QG):
                for (c0, c1) in SUPERS:
                    nsub = c1 - c0
                    vector.wait_ge(s_pe2, it + 1)
                    pY = pbY[it % 2]
                    src = pY[:, 0 : nsub * 512].rearrange(
                        "p (s k) -> p s k", s=nsub)[:, :, 0:CHUNK]
                    nc.vector.tensor_scalar(
                        y2v[:, ycol(2 * q + 1, c0) :
                            ycol(2 * q + 1, c0) + nsub * CHUNK],
                        src, 1.0, 0.0, ALU.mult, ALU.add,
                        accum_out=ps2[:, 2 * it + 1 : 2 * it + 2])
                    nc.vector.drain()
                    vector.wait_ge(s_ev2, it + 1)
                    for half, slot in ((0, 2 * q), (1, 2 * q + 1)):
                        yc = y2v[:, ycol(slot, c0) :
                                 ycol(slot, c0) + nsub * CHUNK]
                        nc.vector.scalar_tensor_tensor(
                            out=scr16[:, 0 : nsub * CHUNK], in0=yc,
                            scalar=1.0, in1=yc,
                            op0=ALU.mult, op1=ALU.mult,
                            accum_out=pq2[:, 2 * it + half :
                                          2 * it + half + 1])
                    nc.vector.drain().then_inc(s_dv2, 1)
                    it += 1

            stats(ps2, pq2, s_st2, D_G2, 2, stats2, glob2, 1)

            # final w = y2 * a2 + tq
            vector.wait_ge(dsem, D_TQ)
            for j in range(NF):
                sl, h = FINALS[j]
                off = sl * PERIMG + h * HCOLS
                if j >= 2:
                    vector.wait_ge(s_fs, j - 1)
                nc.vector.scalar_tensor_tensor(
                    out=wbuf[j % 2][:, 0:HCOLS],
                    in0=y2v[:, off : off + HCOLS],
                    scalar=glob2[:, 6:7],
                    in1=tq[:, off : off + HCOLS],
                    op0=ALU.mult, op1=ALU.add)
                nc.vector.drain().then_inc(s_fvd, 1)

        @block.gpsimd
        def _(gpsimd):
            nc.gpsimd.memset(dscr[:], 0).then_inc(s_ms, 1)
            for s in range(SLOTS):
                nc.gpsimd.memset(bin1[:, s], 0).then_inc(s_ms, 1)
            gpsimd.wait_ge(dsem, D_B1ST)
            if CC_STUB:
                nc.gpsimd.dma_start(db1_out[:], db1_in[:]).then_inc(s_cc, 16)
            else:
                nc.gpsimd.collective_compute(
                    "AllReduce", ALU.add, replica_groups=[list(range(N_CORES))],
                    ins=[db1_in[:]], outs=[db1_out[:]]).then_inc(s_cc, 1)
            gpsimd.wait_ge(dsem, D_B2ST)
            if CC_STUB:
                nc.gpsimd.dma_start(db2_out[:], db2_in[:]).then_inc(s_cc, 16)
            else:
                nc.gpsimd.collective_compute(
                    "AllReduce", ALU.add, replica_groups=[list(range(N_CORES))],
                    ins=[db2_in[:]], outs=[db2_out[:]]).then_inc(s_cc, 1)

    return nc


_CACHE = {}


def _get_nc():
    if "nc" not in _CACHE:
        _CACHE["nc"] = build_bass()
    return _CACHE["nc"]


def kernel(x, w1, gamma1, beta1, w2, gamma2, beta2):
    x = np.asarray(x, np.float32)
    w1 = np.asarray(w1, np.float32)
    w2 = np.asarray(w2, np.float32)
    gamma1 = np.asarray(gamma1, np.float32)
    beta1 = np.asarray(beta1, np.float32)
    gamma2 = np.asarray(gamma2, np.float32)
    beta2 = np.asarray(beta2, np.float32)

    f8np = mybir.dt.np(F8)

    # conv1 weights: [tap, cin, cout] -> [cin, tap*cout], rows duplicated
    wb1 = np.where(w1 >= 0, 1.0, -1.0).astype(np.float32)
    wt1a = wb1.transpose(1, 2, 3, 0).reshape(64, 9, 64)
    wt1 = wt1a.reshape(64, 576)
    wf16_np = np.concatenate([wt1, wt1], axis=0).astype(np.float16)
    # fp8 limb weights: block-diagonal +-2^-12 (exact e4m3 subnormal)
    w1m_np = np.zeros((128, 1280), np.float32)
    for t in range(9):
        w1m_np[0:64, t * 128 : t * 128 + 64] = wt1a[:, t, :] * 2.0 ** -12
        w1m_np[64:128, t * 128 + 64 : t * 128 + 128] = wt1a[:, t, :] * 2.0 ** -12

    # conv2 weights: fp8 block-diagonal, 10 taps (tap 9 zero)
    wb2 = np.where(w2 >= 0, 1.0, -1.0).astype(np.float32)
    wt2 = wb2.transpose(1, 2, 3, 0).reshape(64, 9, 64)  # [cin, tap, cout]
    w8_np = np.zeros((128, 1280), np.float32)
    for t in range(9):
        w8_np[0:64, t * 128 : t * 128 + 64] = wt2[:, t, :]
        w8_np[64:128, t * 128 + 64 : t * 128 + 128] = wt2[:, t, :]
    w8_np = w8_np.astype(f8np)
    w1m_np = w1m_np.astype(mybir.dt.np(F8E5))

    consts_np = np.zeros((128, 8), np.float32)
    for col, v in enumerate([gamma1, beta1, gamma2, beta2]):
        consts_np[0:64, col] = v
        consts_np[64:128, col] = v
    consts_np[:, 4] = EPS

    in_maps = []
    for k in range(N_CORES):
        xc = x[IMGS * k : IMGS * (k + 1)]            # [8, 64, 56, 56]
        xp = np.zeros((IMGS, C, HP, HP), np.float32)
        xp[:, :, 1 : 1 + H, 1 : 1 + W] = xc
        arr = xp.reshape(SLOTS, 2, C, HP, HP).transpose(1, 2, 0, 3, 4)
        arr = np.ascontiguousarray(arr).reshape(128, SLOTS, HP, HP)
        ahi = arr.astype(np.float16)
        r1 = arr - ahi.astype(np.float32)
        m8 = (r1 * 4096.0).astype(f8np)
        r2 = r1 - m8.astype(np.float32) / 4096.0
        l8 = (r2 * 4096.0).astype(f8np)
        # Q-pack the fp8 limbs so the block-diagonal limb matmuls land in the
        # same (permuted) arrangement the hi quadrant pass produces
        m8q = np.empty_like(m8)
        l8q = np.empty_like(l8)
        for s in range(SLOTS):
            for h in (0, 1):
                img = 4 * (s // 2) + (s % 2) + 2 * h
                sp, hp_ = img // 2, img % 2
                m8q[h * 64 : h * 64 + 64, s] = m8[hp_ * 64 : hp_ * 64 + 64, sp]
                l8q[h * 64 : h * 64 + 64, s] = l8[hp_ * 64 : hp_ * 64 + 64, sp]
        # conv1's quadrant pattern permutes (slot, half): y slot s half h holds
        # image Q(s,h) = 4*(s//2) + s%2 + 2*h. The final residual add needs x
        # in that same arrangement.
        tq_np = np.empty((128, SLOTS, PERIMG), np.float32)
        for s in range(SLOTS):
            for h in (0, 1):
                img = 4 * (s // 2) + (s % 2) + 2 * h
                tq_np[h * 64 : h * 64 + 64, s] = xc[img].reshape(C, PERIMG)
        in_maps.append({
            "xhi": ahi, "xm8": m8q, "xl8": l8q, "wf16": wf16_np,
            "w8": w8_np, "w1m": w1m_np, "consts": consts_np,
            "tq": tq_np.reshape(128, YCOLS),
        })

    nc = _get_nc()
    res = bass_utils.run_bass_kernel_spmd(nc, in_maps, core_ids=list(range(N_CORES)))

    out = np.empty((N, C, H, W), np.float32)
    for k in range(N_CORES):
        o = np.asarray(res.results[k]["outp"]).astype(np.float32)  # [128, 12544]
        o = o.reshape(2, C, SLOTS, H, W)  # [half, ch, slot, H, W]
        for s in range(SLOTS):
            for h in (0, 1):
                img = 4 * (s // 2) + (s % 2) + 2 * h
                out[IMGS * k + img] = o[h, :, s]
    return out


if __name__ == "__main__":
    rng = np.random.default_rng(0)
    xs = rng.standard_normal((N, C, H, W)).astype(np.float32)
    w1s = (rng.standard_normal((C, C, 3, 3)) * 0.1).astype(np.float32)
    w2s = (rng.standard_normal((C, C, 3, 3)) * 0.1).astype(np.float32)
    ones = np.ones(C, np.float32)
    zeros = np.zeros(C, np.float32)
    r = kernel(x=xs, w1=w1s, gamma1=ones, beta1=zeros, w2=w2s, gamma2=ones,
               beta2=zeros)
    print("ran, out uniq:", np.unique(r))


# revision 13
# speedup vs baseline: 1.1956x; 1.0008x over previous
"""BinaryBasicBlock TRN2 kernel: 8-core batch-parallel, raw Bass.

Reference computation (per core: 8 images, C=64, 56x56):
  y1   = conv3x3(x, sign(w1))            # exact: x = fp16(x) + fp16(residual)
  bin1 = sign((y1 - mu1) * rsqrt(var1+eps) * g1 + b1)   # global batch stats
  y2   = conv3x3(bin1, sign(w2))         # exact (+-1 x +-1 in fp8)
  out  = sign((y2 - mu2) * rsqrt(var2+eps) * g2 + b2 + x)

Batch stats are exact: per-core (sum, sumsq) partials are AllReduced across
the 8 cores mid-kernel (both partition halves stored side by side as [64,4]
so no on-chip cross-partition fold is needed).

Layout: channels on partitions, 2 images per 128 partitions (top/bottom
halves), 4 "slots" of [128, 58, 58] padded images per core.

conv1 runs as 9-tap f16 matmul accumulation with all four 64x64 PE quadrants
streaming four different images concurrently (hi + lo pass for fp32
exactness). conv2 runs on fp8 inputs (bin1 is +-1, exact in e4m3) as
full-128 block-diagonal DoubleRow matmuls: weights hold both partition
halves' 64x64 blocks on the diagonal and each matmul processes a pair of
taps, so 5 matmuls replace 9 per (slot, subchunk).

The PE p-state ramp is kept hot across the conv1->conv2 stats barrier by a
stream of filler matmuls into a scratch PSUM region (the tensor engine
down-clocks after idling, which would slow conv2's first ~3us).

conv2 PSUM evacuation is split: ACT evacuates slot 2q (with sum
accumulation), DVE evacuates slot 2q+1 (tensor_scalar with accum), so the
fp8 conv2 is not ACT-bound. y2 is exact in f16 (integer-valued, |y2|<=576).

Final stage: t = xhi+xlo (f32, precomputed during conv1), then per chunk
one DVE/Pool scalar_tensor_tensor (w = y2*a2 + t) and one ACT Sign with
per-channel bias, written as fp8 (+-1 exact) and stored per slot.

Toolchain constraints honored: raw Bass only, max one semaphore wait per
instruction, single PSUM reader engine per bank, drain-backed semaphore
increments on every cross-engine RAW edge, explicit DVE drains between
dependent vector ops. DoubleRow matmuls keep dst partition base 0 (ISA
constraint s3d3_mm_valid_dst_partition).
"""
import numpy as np
import ml_dtypes
import concourse.bass as bass
import concourse.mybir as mybir
from concourse import bass_utils
from concourse.ap import AP as APcls
from contextlib import ExitStack

F32 = mybir.dt.float32
BF16 = mybir.dt.bfloat16
F16 = mybir.dt.float16
F8 = mybir.dt.float8e4
F8E5 = mybir.dt.float8e5
AF = mybir.ActivationFunctionType
ALU = mybir.AluOpType
DRM = mybir.MatmulPerfMode.DoubleRow

N_CORES = 8
N, C, H, W = 64, 64, 56, 56
IMGS = N // N_CORES          # 8 images per core
SLOTS = IMGS // 2            # 4 slots (2 images per slot)
QG = SLOTS // 2              # 2 quadgroups (4 images each)
HP = H + 2                   # 58 padded
CHROWS = 8                   # output rows per 448-subchunk
CHUNK = CHROWS * W           # 448
NCH = H // CHROWS            # 7 subchunks per image
SUPERS = [(0, 2), (2, 4), (4, 6), (6, 7)]   # subchunk ranges per super-iter
NSUP = len(SUPERS)           # 4 super-iters per quadgroup
ITERS = QG * NSUP            # 8 super-iters per conv
PERIMG = H * W               # 3136
YCOLS = SLOTS * PERIMG       # 12544
SLOTPIX = HP * HP            # 3364
N_TOT = float(N * H * W)     # global batch-stat count
EPS = 1e-5
NF = SLOTS * 2               # 8 final-stage chunks (half-slots of 1568)
PAIRS = [(0, 1), (2, 3), (4, 5), (6, 7), (8, 9)]  # conv2 tap pairs (9=zero)
POOL_J = ()   # final iterations handled by GPSIMD (stt not Pool-legal)

W0_DUMMIES = 30              # PE warmup fillers (initial load latency)
W1_DUMMIES = 80              # PE fillers across the stats1 barrier

DEBUG = False
CC_STUB = False   # replace AllReduce with a local DMA (for TimelineSim)

# row chunks per slot for the staged input loads
ROWCH = [(0, 18), (18, 34), (34, 50), (50, 58)]


def build_bass():
    nc = bass.Bass(trn_type="TRN2", target_bir_lowering=False, debug=False,
                   num_devices=N_CORES)

    d_xhi = nc.dram_tensor("xhi", [128, SLOTS, HP, HP], F16, kind="ExternalInput")
    d_xm8 = nc.dram_tensor("xm8", [128, SLOTS, HP, HP], F8, kind="ExternalInput")
    d_xl8 = nc.dram_tensor("xl8", [128, SLOTS, HP, HP], F8, kind="ExternalInput")
    d_wf16 = nc.dram_tensor("wf16", [128, 576], F16, kind="ExternalInput")
    d_w8 = nc.dram_tensor("w8", [128, 1280], F8, kind="ExternalInput")
    d_w1m = nc.dram_tensor("w1m", [128, 1280], F8E5, kind="ExternalInput")
    d_consts = nc.dram_tensor("consts", [128, 8], F32, kind="ExternalInput")
    d_tq = nc.dram_tensor("tq", [128, YCOLS], F32, kind="ExternalInput")
    d_out = nc.dram_tensor("outp", [128, YCOLS], F8, kind="ExternalOutput")
    db1_in = nc.dram_tensor("db1_in", [64, 4], F32)
    db1_out = nc.dram_tensor("db1_out", [64, 4], F32, addr_space="Shared")
    db2_in = nc.dram_tensor("db2_in", [64, 4], F32)
    db2_out = nc.dram_tensor("db2_out", [64, 4], F32, addr_space="Shared")

    es = ExitStack()
    def sb(name, shape, dt):
        return es.enter_context(nc.sbuf_tensor(name, shape, dt))
    def ps(name, shape, dt):
        return es.enter_context(nc.psum_tensor(name, shape, dt))
    def sem(name):
        return es.enter_context(nc.semaphore(name))

    xhi = sb("xhi_t", [128, SLOTS, HP, HP], F16)
    xm8 = sb("xm8_t", [128, SLOTS, HP, HP], F8)
    xl8 = sb("xl8_t", [128, SLOTS, HP, HP], F8)
    wf16 = sb("wf16_t", [128, 576], F16)
    w8 = sb("w8_t", [128, 1280], F8)
    w1m = sb("w1m_t", [128, 1280], F8E5)
    consts = sb("consts_t", [128, 8], F32)
    bin1 = sb("bin1_t", [128, SLOTS, HP, HP], F8)
    tq = sb("tq_t", [128, YCOLS], F32)
    y1 = sb("y1_t", [128, YCOLS], F32)
    # y2 (f16) and the fp8 output live in y1's bytes (dead regions by then):
    #   y2v   = f16 cols 0..12543     (y1 f32 cols 0..6271   = slots 0,1)
    #   outv  = f8 cols 25088..37631  (y1 f32 cols 6272..9407 = slots 2,3lo)
    y2v = y1[:].bitcast(F16)
    outv = y1[:].bitcast(F8)
    OUTOFF = 2 * YCOLS
    NPART = 2 * ITERS            # partial columns per conv
    ps1 = sb("ps1", [128, NPART], F32)
    pq1 = sb("pq1", [128, NPART], F32)
    ps2 = sb("ps2", [128, NPART], F32)
    pq2 = sb("pq2", [128, NPART], F32)
    stats1 = sb("stats1", [128, 2], F32)
    stats2 = sb("stats2", [128, 2], F32)
    glob1 = sb("glob1", [128, 8], F32)
    glob2 = sb("glob2", [128, 8], F32)
    scr = sb("scr", [128, 2 * CHUNK], F32)
    scr16 = scr[:].bitcast(F16)
    wbuf = [sb(f"wb{i}", [128, PERIMG // 2], F32) for i in range(2)]
    dscr = sb("dscr", [128, 512], F16)
    pbX = [ps(f"pbX{i}", [128, 1024], F32) for i in range(2)]
    pbY = [ps(f"pbY{i}", [128, 1024], F32) for i in range(2)]

    dsem = sem("dsem")
    s_pe1 = sem("s_pe1"); s_ev1 = sem("s_ev1")
    s_pe2 = sem("s_pe2"); s_ev2 = sem("s_ev2"); s_dv2 = sem("s_dv2")
    s_sg1 = sem("s_sg1"); s_ms = sem("s_ms")
    s_st1 = sem("s_st1"); s_st2 = sem("s_st2"); s_acst = sem("s_acst")
    s_cc = sem("s_cc")
    s_fvd = sem("s_fvd"); s_fvp = sem("s_fvp"); s_fs = sem("s_fs")

    CCV = 16 if CC_STUB else 1

    def ycol(slot, c):
        return slot * PERIMG + c * CHUNK

    HCOLS = PERIMG // 2          # 1568
    FINALS = [(s, h) for s in range(SLOTS) for h in (0, 1)]

    # ---- input load schedule --------------------------------------------
    # list of (sbuf_dst_ap_fn, dram_src_ap_fn); dsem marks derived from index
    loads = []
    def add_load(dst, src):
        loads.append((dst, src))
        return len(loads)  # 1-based count

    add_load(wf16[:], d_wf16[:])
    add_load(w1m[:], d_w1m[:])
    for t, dt_ in ((xhi, d_xhi), (xm8, d_xm8), (xl8, d_xl8)):
        for s in (0, 1):
            add_load(t[:, s, 0:18], dt_[:, s, 0:18])
    D_S0 = len(loads) * 16
    add_load(w8[:], d_w8[:])
    add_load(consts[:], d_consts[:])
    for (r0, r1) in ROWCH[1:]:
        for t, dt_ in ((xhi, d_xhi), (xm8, d_xm8), (xl8, d_xl8)):
            for s in (0, 1):
                add_load(t[:, s, r0:r1], dt_[:, s, r0:r1])
        if r1 == 34:
            D_S1 = len(loads) * 16
        elif r1 == 50:
            D_S2 = len(loads) * 16
        else:
            D_S3 = len(loads) * 16
    for s in (2, 3):
        for t, dt_ in ((xhi, d_xhi), (xm8, d_xm8), (xl8, d_xl8)):
            add_load(t[:, s], dt_[:, s])
    D_QG1 = len(loads) * 16
    for s in range(SLOTS):
        add_load(tq[:, s * PERIMG : (s + 1) * PERIMG],
                 d_tq[:, s * PERIMG : (s + 1) * PERIMG])
    D_TQ = len(loads) * 16
    NLOADS = len(loads)
    D_B1ST = (NLOADS + 2) * 16
    D_G1 = (NLOADS + 4) * 16
    D_B2ST = (NLOADS + 6) * 16
    D_G2 = (NLOADS + 8) * 16

    CONV1_GATES = {(0, 0): D_S0, (0, 1): D_S1, (0, 2): D_S2, (0, 3): D_S3,
                   (1, 0): D_QG1}

    with nc.Block() as block:

        @block.sync
        def _(sync):
            for dst, src in loads:
                sync.dma_start(dst, src).then_inc(dsem, 16)
            # stats chains: store half 2 / load half 2 ride on ACT and Pool
            sync.wait_ge(s_st1, 1)
            sync.dma_start(db1_in[:, 0:2], stats1[0:64, 0:2]).then_inc(dsem, 16)
            sync.wait_ge(s_cc, CCV)
            sync.dma_start(glob1[0:64, 0:4], db1_out[:]).then_inc(dsem, 16)
            sync.wait_ge(s_st2, 1)
            sync.dma_start(db2_in[:, 0:2], stats2[0:64, 0:2]).then_inc(dsem, 16)
            sync.wait_ge(s_cc, 2 * CCV)
            sync.dma_start(glob2[0:64, 0:4], db2_out[:]).then_inc(dsem, 16)
            # output stores, one per final chunk
            for k in range(NF):
                sl, h = FINALS[k]
                off = sl * PERIMG + h * HCOLS
                sync.wait_ge(s_fs, k + 1)
                sync.dma_start(
                    d_out[:, off : off + HCOLS],
                    outv[:, OUTOFF + off : OUTOFF + off + HCOLS],
                ).then_inc(dsem, 16)

        @block.tensor
        def _(tensor):
            def dummy64(n):
                # tiny fillers into never-read psum columns (960:1024)
                nc.tensor.ldweights(dscr[:, 0:64], tile_position=(0, 0))
                for i in range(n):
                    nc.tensor.matmul(pbX[0][0:64, 960:1024], dscr[:, 0:64],
                                     dscr[:, 64:128], start=True, stop=True,
                                     tile_position=(0, 0),
                                     skip_group_check=True)

            def dummy(n):
                # keep the PE p-state hot: harmless f16 matmuls into a
                # region of pbX[0] that is dead at every dummy site
                for i in range(n):
                    nc.tensor.ldweights(dscr[:, 0:64], tile_position=(0, 0))
                    nc.tensor.matmul(pbX[0][0:64, 0:448], dscr[:, 0:64],
                                     dscr[:, 64:512], start=True, stop=True,
                                     tile_position=(0, 0),
                                     skip_group_check=True)

            tensor.wait_ge(s_ms, 1)
            dummy(W0_DUMMIES)

            # conv1: f16 hi/lo, 4 quadrants (4 images concurrent on HW)
            it = 0
            for q in range(QG):
                for si, (c0, c1) in enumerate(SUPERS):
                    gate = CONV1_GATES.get((q, si))
                    if gate is not None:
                        tensor.wait_ge(dsem, gate)
                    nsub = c1 - c0
                    if it >= 2:
                        tensor.wait_ge(s_ev1, it - 1)
                    pX = pbX[it % 2]
                    pY = pbY[it % 2]
                    quads = [
                        ((0, 0), slice(0, 64), 2 * q, pX, slice(0, 64)),
                        ((64, 0), slice(64, 128), 2 * q, pY, slice(0, 64)),
                        ((0, 64), slice(0, 64), 2 * q + 1, pX, slice(64, 128)),
                        ((64, 64), slice(64, 128), 2 * q + 1, pY,
                         slice(64, 128)),
                    ]
                    for tap in range(9):
                        kh, kw = tap // 3, tap % 3
                        wcol = tap * 64
                        for tp, rows, _, _, _ in quads:
                            nc.tensor.ldweights(wf16[rows, wcol : wcol + 64],
                                                tile_position=tp)
                        for tp, rows, dslot, pdst, phalf in quads:
                            for s in range(nsub):
                                c = c0 + s
                                rap = xhi[rows, dslot,
                                          c * CHROWS + kh :
                                          c * CHROWS + kh + CHROWS,
                                          kw : kw + W]
                                nc.tensor.matmul(
                                    pdst[phalf, s * 512 : s * 512 + CHUNK],
                                    wf16[rows, wcol : wcol + 64], rap,
                                    start=(tap == 0), stop=False,
                                    tile_position=tp,
                                    skip_group_check=True)
                    # fp8 mid/lo limbs: block-diagonal DoubleRow, weights
                    # are +-2^-12 so they accumulate into the same groups
                    for li, lt in enumerate((xm8, xl8)):
                        for ip, (ta, tb) in enumerate(PAIRS):
                            kha, kwa = ta // 3, ta % 3
                            if tb == 9:
                                delta = -HP
                            else:
                                delta = (tb // 3 - kha) * HP + (tb % 3 - kwa)
                            wap = APcls(tensor=w1m[:].tensor, offset=ta * 128,
                                        ap=[[1280, 128], [128, 2], [1, 128]])
                            nc.tensor.ldweights(wap, perf_mode=DRM)
                            for sj in range(2):
                                slot = 2 * q + sj
                                pdst = pX if sj == 0 else pY
                                for s in range(nsub):
                                    c = c0 + s
                                    offa = (slot * SLOTPIX
                                            + (c * CHROWS + kha) * HP + kwa)
                                    rap = APcls(
                                        tensor=lt[:].tensor, offset=offa,
                                        ap=[[SLOTS * SLOTPIX, 128], [delta, 2],
                                            [HP, CHROWS], [1, W]])
                                    nc.tensor.matmul(
                                        pdst[:, s * 512 : s * 512 + CHUNK],
                                        wap, rap, start=False,
                                        stop=(li == 1 and ip == 4),
                                        perf_mode=DRM, skip_group_check=True)
                    tensor.drain().then_inc(s_pe1, 1)
                    it += 1

            # fill the stats1 -> bin1 barrier (evac of it=14 must be done
            # before reusing pbX[0]; evac15 targets pbX[1]/pbY[1])
            tensor.wait_ge(s_ev1, ITERS - 1)
            dummy(W1_DUMMIES)

            # conv2: fp8 block-diagonal DoubleRow, 5 tap-pairs
            it = 0
            for q in range(QG):
                tensor.wait_ge(s_sg1, 2 if q == 0 else 4)
                for si, (c0, c1) in enumerate(SUPERS):
                    nsub = c1 - c0
                    if it == 2:
                        dummy64(85)
                    if it >= 2:
                        tensor.wait_ge(s_ev2, it - 1)
                        tensor.wait_ge(s_dv2, it - 1)
                    pX = pbX[it % 2]
                    pY = pbY[it % 2]
                    for ip, (ta, tb) in enumerate(PAIRS):
                        kha, kwa = ta // 3, ta % 3
                        if tb == 9:
                            delta = -58  # zero weights; any in-bounds window
                        else:
                            delta = (tb // 3 - kha) * HP + (tb % 3 - kwa)
                        wap = APcls(tensor=w8[:].tensor, offset=ta * 128,
                                    ap=[[1280, 128], [128, 2], [1, 128]])
                        nc.tensor.ldweights(wap, perf_mode=DRM)
                        for sj in range(2):
                            slot = 2 * q + sj
                            pdst = pX if sj == 0 else pY
                            for s in range(nsub):
                                c = c0 + s
                                offa = (slot * SLOTPIX
                                        + (c * CHROWS + kha) * HP + kwa)
                                rap = APcls(
                                    tensor=bin1[:].tensor, offset=offa,
                                    ap=[[SLOTS * SLOTPIX, 128], [delta, 2],
                                        [HP, CHROWS], [1, W]])
                                nc.tensor.matmul(
                                    pdst[:, s * 512 : s * 512 + CHUNK],
                                    wap, rap, start=(ip == 0), stop=(ip == 4),
                                    perf_mode=DRM, skip_group_check=True)
                    tensor.drain().then_inc(s_pe2, 1)
                    it += 1

        @block.scalar
        def _(scalar):
            # conv1 evacs: PSUM -> y1 (f32) with sum accumulation
            it = 0
            for q in range(QG):
                for (c0, c1) in SUPERS:
                    nsub = c1 - c0
                    scalar.wait_ge(s_pe1, it + 1)
                    pX = pbX[it % 2]
                    pY = pbY[it % 2]
                    for half, slot, pt in ((0, 2 * q, pX), (1, 2 * q + 1, pY)):
                        src = pt[:, 0 : nsub * 512].rearrange(
                            "p (s k) -> p s k", s=nsub)[:, :, 0:CHUNK]
                        nc.scalar.activation(
                            y1[:, ycol(slot, c0) :
                               ycol(slot, c0) + nsub * CHUNK],
                            src, AF.Copy,
                            accum_out=ps1[:, 2 * it + half :
                                          2 * it + half + 1])
                    scalar.drain().then_inc(s_ev1, 1)
                    it += 1
            # stats1: store the bottom half's partials, then sqrt(var+eps)
            scalar.wait_ge(s_st1, 1)
            nc.scalar.dma_start(db1_in[:, 2:4],
                                stats1[64:128, 0:2]).then_inc(dsem, 16)
            scalar.wait_ge(s_cc, CCV)
            nc.scalar.dma_start(glob1[64:128, 0:4], db1_out[:]).then_inc(dsem, 16)
            scalar.wait_ge(s_st1, 2)
            nc.scalar.activation(glob1[:, 2:3], glob1[:, 3:4], AF.Sqrt,
                                 bias=consts[:, 4:5])
            scalar.drain().then_inc(s_acst, 1)
            # bin1 = Sign(y1 * a1 + b1) into padded fp8 slots
            scalar.wait_ge(s_ms, 5)
            scalar.wait_ge(s_st1, 3)
            def sign1(s):
                nc.scalar.activation(
                    bin1[:, s, 1 : 1 + H, 1 : 1 + W],
                    y1[:, s * PERIMG : (s + 1) * PERIMG],
                    AF.Sign, bias=glob1[:, 7:8], scale=glob1[:, 6:7])
                scalar.drain().then_inc(s_sg1, 1)
            sign1(0)
            sign1(1)
            sign1(2)
            sign1(3)

            # conv2 evacs of pX (slot 2q) with accum; slots 2,3 signs woven in
            def evac2(itv, c0, nsub, q):
                scalar.wait_ge(s_pe2, itv + 1)
                pX = pbX[itv % 2]
                src = pX[:, 0 : nsub * 512].rearrange(
                    "p (s k) -> p s k", s=nsub)[:, :, 0:CHUNK]
                nc.scalar.activation(
                    y2v[:, ycol(2 * q, c0) : ycol(2 * q, c0) + nsub * CHUNK],
                    src, AF.Copy,
                    accum_out=ps2[:, 2 * itv : 2 * itv + 1])
                scalar.drain().then_inc(s_ev2, 1)

            it = 0
            for q in range(QG):
                for si, (c0, c1) in enumerate(SUPERS):
                    evac2(it, c0, c1 - c0, q)
                    it += 1
            # stats2: bottom-half store, then sqrt
            scalar.wait_ge(s_st2, 1)
            nc.scalar.dma_start(db2_in[:, 2:4],
                                stats2[64:128, 0:2]).then_inc(dsem, 16)
            scalar.wait_ge(s_cc, 2 * CCV)
            nc.scalar.dma_start(glob2[64:128, 0:4], db2_out[:]).then_inc(dsem, 16)
            scalar.wait_ge(s_st2, 2)
            nc.scalar.activation(glob2[:, 2:3], glob2[:, 3:4], AF.Sqrt,
                                 bias=consts[:, 4:5])
            scalar.drain().then_inc(s_acst, 2)
            # final: out = Sign(w + b2'), w produced by DVE/Pool
            for j in range(NF):
                sl, h = FINALS[j]
                off = sl * PERIMG + h * HCOLS
                scalar.wait_ge(s_fvd, j + 1)
                nc.scalar.activation(
                    outv[:, OUTOFF + off : OUTOFF + off + HCOLS],
                    wbuf[j % 2][:, 0:HCOLS], AF.Sign,
                    bias=glob2[:, 7:8])
                scalar.drain().then_inc(s_fs, 1)

        @block.vector
        def _(vector):
            # conv1 sumsq partials
            it = 0
            for q in range(QG):
                for (c0, c1) in SUPERS:
                    nsub = c1 - c0
                    vector.wait_ge(s_ev1, it + 1)
                    for half, slot in ((0, 2 * q), (1, 2 * q + 1)):
                        yc = y1[:, ycol(slot, c0) :
                                ycol(slot, c0) + nsub * CHUNK]
                        nc.vector.scalar_tensor_tensor(
                            out=scr[:, 0 : nsub * CHUNK], in0=yc,
                            scalar=1.0, in1=yc,
                            op0=ALU.mult, op1=ALU.mult,
                            accum_out=pq1[:, 2 * it + half :
                                          2 * it + half + 1])
                    it += 1

            def stats(pstats_s, pstats_q, st, dsem_in, acst_v, statst, g,
                      which):
                nc.vector.drain()
                nc.vector.reduce_sum(statst[:, 0:1], pstats_s[:],
                                     axis=mybir.AxisListType.X)
                nc.vector.reduce_sum(statst[:, 1:2], pstats_q[:],
                                     axis=mybir.AxisListType.X)
                nc.vector.drain().then_inc(st, 1)
                vector.wait_ge(dsem, dsem_in)
                # halves side by side: fold on-partition, then bn math
                nc.vector.tensor_tensor(out=g[:, 4:6], in0=g[:, 0:2],
                                        in1=g[:, 2:4], op=ALU.add)
                nc.vector.drain()
                nc.vector.tensor_scalar_mul(g[:, 0:1], g[:, 4:5], 1.0 / N_TOT)
                nc.vector.tensor_scalar_mul(g[:, 1:2], g[:, 5:6], 1.0 / N_TOT)
                nc.vector.drain()
                nc.vector.tensor_tensor(out=g[:, 2:3], in0=g[:, 0:1],
                                        in1=g[:, 0:1], op=ALU.mult)
                nc.vector.drain()
                nc.vector.tensor_tensor(out=g[:, 3:4], in0=g[:, 1:2],
                                        in1=g[:, 2:3], op=ALU.subtract)
                nc.vector.drain().then_inc(st, 1)
                # ACT: g[:,2:3] = sqrt(g[:,3:4] + eps)
                vector.wait_ge(s_acst, acst_v)
                gcol, bcol = 2 * which, 2 * which + 1
                nc.vector.reciprocal(g[:, 3:4], g[:, 2:3])
                nc.vector.drain()
                nc.vector.tensor_tensor(out=g[:, 6:7], in0=g[:, 3:4],
                                        in1=consts[:, gcol : gcol + 1],
                                        op=ALU.mult)
                nc.vector.drain()
                nc.vector.tensor_tensor(out=g[:, 4:5], in0=g[:, 0:1],
                                        in1=g[:, 6:7], op=ALU.mult)
                nc.vector.drain()
                nc.vector.tensor_tensor(out=g[:, 7:8],
                                        in0=consts[:, bcol : bcol + 1],
                                        in1=g[:, 4:5], op=ALU.subtract)
                nc.vector.drain().then_inc(st, 1)

            stats(ps1, pq1, s_st1, D_G1, 1, stats1, glob1, 0)

            # conv2: DVE evacs pY (slot 2q+1) + both sumsq partials
            it = 0
            for q in range(QG):
                for (c0, c1) in SUPERS:
                    nsub = c1 - c0
                    vector.wait_ge(s_pe2, it + 1)
                    pY = pbY[it % 2]
                    src = pY[:, 0 : nsub * 512].rearrange(
                        "p (s k) -> p s k", s=nsub)[:, :, 0:CHUNK]
                    nc.vector.tensor_scalar(
                        y2v[:, ycol(2 * q + 1, c0) :
                            ycol(2 * q + 1, c0) + nsub * CHUNK],
                        src, 1.0, 0.0, ALU.mult, ALU.add,
                        accum_out=ps2[:, 2 * it + 1 : 2 * it + 2])
                    nc.vector.drain()
                    vector.wait_ge(s_ev2, it + 1)
                    for half, slot in ((0, 2 * q), (1, 2 * q + 1)):
                        yc = y2v[:, ycol(slot, c0) :
                                 ycol(slot, c0) + nsub * CHUNK]
                        nc.vector.scalar_tensor_tensor(
                            out=scr16[:, 0 : nsub * CHUNK], in0=yc,
                            scalar=1.0, in1=yc,
                            op0=ALU.mult, op1=ALU.mult,
                            accum_out=pq2[:, 2 * it + half :
                                          2 * it + half + 1])
                    nc.vector.drain().then_inc(s_dv2, 1)
                    it += 1

            stats(ps2, pq2, s_st2, D_G2, 2, stats2, glob2, 1)

            # final w = y2 * a2 + tq
            vector.wait_ge(dsem, D_TQ)
            for j in range(NF):
                sl, h = FINALS[j]
                off = sl * PERIMG + h * HCOLS
                if j >= 2:
                    vector.wait_ge(s_fs, j - 1)
                nc.vector.scalar_tensor_tensor(
                    out=wbuf[j % 2][:, 0:HCOLS],
                    in0=y2v[:, off : off + HCOLS],
                    scalar=glob2[:, 6:7],
                    in1=tq[:, off : off + HCOLS],
                    op0=ALU.mult, op1=ALU.add)
                nc.vector.drain().then_inc(s_fvd, 1)

        @block.gpsimd
        def _(gpsimd):
            nc.gpsimd.memset(dscr[:], 0).then_inc(s_ms, 1)
            for s in range(SLOTS):
                nc.gpsimd.memset(bin1[:, s], 0).then_inc(s_ms, 1)
            gpsimd.wait_ge(dsem, D_B1ST)
            if CC_STUB:
                nc.gpsimd.dma_start(db1_out[:], db1_in[:]).then_inc(s_cc, 16)
            else:
                nc.gpsimd.collective_compute(
                    "AllReduce", ALU.add, replica_groups=[list(range(N_CORES))],
                    ins=[db1_in[:]], outs=[db1_out[:]]).then_inc(s_cc, 1)
            gpsimd.wait_ge(dsem, D_B2ST)
            if CC_STUB:
                nc.gpsimd.dma_start(db2_out[:], db2_in[:]).then_inc(s_cc, 16)
            else:
                nc.gpsimd.collective_compute(
                    "AllReduce", ALU.add, replica_groups=[list(range(N_CORES))],
                    ins=[db2_in[:]], outs=[db2_out[:]]).then_inc(s_cc, 1)

    return nc


_CACHE = {}


def _get_nc():
    if "nc" not in _CACHE:
        _CACHE["nc"] = build_bass()
    return _CACHE["nc"]


def kernel(x, w1, gamma1, beta1, w2, gamma2, beta2):
    x = np.asarray(x, np.float32)
    w1 = np.asarray(w1, np.float32)
    w2 = np.asarray(w2, np.float32)
    gamma1 = np.asarray(gamma1, np.float32)
    beta1 = np.asarray(beta1, np.float32)
    gamma2 = np.asarray(gamma2, np.float32)
    beta2 = np.asarray(beta2, np.float32)

    f8np = mybir.dt.np(F8)

    # conv1 weights: [tap, cin, cout] -> [cin, tap*cout], rows duplicated
    wb1 = np.where(w1 >= 0, 1.0, -1.0).astype(np.float32)
    wt1a = wb1.transpose(1, 2, 3, 0).reshape(64, 9, 64)
    wt1 = wt1a.reshape(64, 576)
    wf16_np = np.concatenate([wt1, wt1], axis=0).astype(np.float16)
    # fp8 limb weights: block-diagonal +-2^-12 (exact e4m3 subnormal)
    w1m_np = np.zeros((128, 1280), np.float32)
    for t in range(9):
        w1m_np[0:64, t * 128 : t * 128 + 64] = wt1a[:, t, :] * 2.0 ** -12
        w1m_np[64:128, t * 128 + 64 : t * 128 + 128] = wt1a[:, t, :] * 2.0 ** -12

    # conv2 weights: fp8 block-diagonal, 10 taps (tap 9 zero)
    wb2 = np.where(w2 >= 0, 1.0, -1.0).astype(np.float32)
    wt2 = wb2.transpose(1, 2, 3, 0).reshape(64, 9, 64)  # [cin, tap, cout]
    w8_np = np.zeros((128, 1280), np.float32)
    for t in range(9):
        w8_np[0:64, t * 128 : t * 128 + 64] = wt2[:, t, :]
        w8_np[64:128, t * 128 + 64 : t * 128 + 128] = wt2[:, t, :]
    w8_np = w8_np.astype(f8np)
    w1m_np = w1m_np.astype(mybir.dt.np(F8E5))

    consts_np = np.zeros((128, 8), np.float32)
    for col, v in enumerate([gamma1, beta1, gamma2, beta2]):
        consts_np[0:64, col] = v
        consts_np[64:128, col] = v
    consts_np[:, 4] = EPS

    in_maps = []
    for k in range(N_CORES):
        xc = x[IMGS * k : IMGS * (k + 1)]            # [8, 64, 56, 56]
        xp = np.zeros((IMGS, C, HP, HP), np.float32)
        xp[:, :, 1 : 1 + H, 1 : 1 + W] = xc
        arr = xp.reshape(SLOTS, 2, C, HP, HP).transpose(1, 2, 0, 3, 4)
        arr = np.ascontiguousarray(arr).reshape(128, SLOTS, HP, HP)
        ahi = arr.astype(np.float16)
        r1 = arr - ahi.astype(np.float32)
        m8 = (r1 * 4096.0).astype(f8np)
        r2 = r1 - m8.astype(np.float32) / 4096.0
        l8 = (r2 * 4096.0).astype(f8np)
        # Q-pack the fp8 limbs so the block-diagonal limb matmuls land in the
        # same (permuted) arrangement the hi quadrant pass produces
        m8q = np.empty_like(m8)
        l8q = np.empty_like(l8)
        for s in range(SLOTS):
            for h in (0, 1):
                img = 4 * (s // 2) + (s % 2) + 2 * h
                sp, hp_ = img // 2, img % 2
                m8q[h * 64 : h * 64 + 64, s] = m8[hp_ * 64 : hp_ * 64 + 64, sp]
                l8q[h * 64 : h * 64 + 64, s] = l8[hp_ * 64 : hp_ * 64 + 64, sp]
        # conv1's quadrant pattern permutes (slot, half): y slot s half h holds
        # image Q(s,h) = 4*(s//2) + s%2 + 2*h. The final residual add needs x
        # in that same arrangement.
        tq_np = np.empty((128, SLOTS, PERIMG), np.float32)
        for s in range(SLOTS):
            for h in (0, 1):
                img = 4 * (s // 2) + (s % 2) + 2 * h
                tq_np[h * 64 : h * 64 + 64, s] = xc[img].reshape(C, PERIMG)
        in_maps.append({
            "xhi": ahi, "xm8": m8q, "xl8": l8q, "wf16": wf16_np,
            "w8": w8_np, "w1m": w1m_np, "consts": consts_np,
            "tq": tq_np.reshape(128, YCOLS),
        })

    nc = _get_nc()
    res = bass_utils.run_bass_kernel_spmd(nc, in_maps, core_ids=list(range(N_CORES)))

    out = np.empty((N, C, H, W), np.float32)
    for k in range(N_CORES):
        o = np.asarray(res.results[k]["outp"]).astype(np.float32)  # [128, 12544]
        o = o.reshape(2, C, SLOTS, H, W)  # [half, ch, slot, H, W]
        for s in range(SLOTS):
            for h in (0, 1):
                img = 4 * (s // 2) + (s % 2) + 2 * h
                out[IMGS * k + img] = o[h, :, s]
    return out


if __name__ == "__main__":
    rng = np.random.default_rng(0)
    xs = rng.standard_normal((N, C, H, W)).astype(np.float32)
    w1s = (rng.standard_normal((C, C, 3, 3)) * 0.1).astype(np.float32)
    w2s = (rng.standard_normal((C, C, 3, 3)) * 0.1).astype(np.float32)
    ones = np.ones(C, np.float32)
    zeros = np.zeros(C, np.float32)
    r = kernel(x=xs, w1=w1s, gamma1=ones, beta1=zeros, w2=w2s, gamma2=ones,
               beta2=zeros)
    print("ran, out uniq:", np.unique(r))


# revision 15
# speedup vs baseline: 1.2034x; 1.0065x over previous
"""BinaryBasicBlock TRN2 kernel: 8-core batch-parallel, raw Bass.

Reference computation (per core: 8 images, C=64, 56x56):
  y1   = conv3x3(x, sign(w1))            # exact: x = fp16(x) + fp16(residual)
  bin1 = sign((y1 - mu1) * rsqrt(var1+eps) * g1 + b1)   # global batch stats
  y2   = conv3x3(bin1, sign(w2))         # exact (+-1 x +-1 in fp8)
  out  = sign((y2 - mu2) * rsqrt(var2+eps) * g2 + b2 + x)

Batch stats are exact: per-core (sum, sumsq) partials are AllReduced across
the 8 cores mid-kernel (both partition halves stored side by side as [64,4]
so no on-chip cross-partition fold is needed).

Layout: channels on partitions, 2 images per 128 partitions (top/bottom
halves), 4 "slots" of [128, 58, 58] padded images per core.

conv1 runs as 9-tap f16 matmul accumulation with all four 64x64 PE quadrants
streaming four different images concurrently (hi + lo pass for fp32
exactness). conv2 runs on fp8 inputs (bin1 is +-1, exact in e4m3) as
full-128 block-diagonal DoubleRow matmuls: weights hold both partition
halves' 64x64 blocks on the diagonal and each matmul processes a pair of
taps, so 5 matmuls replace 9 per (slot, subchunk).

The PE p-state ramp is kept hot across the conv1->conv2 stats barrier by a
stream of filler matmuls into a scratch PSUM region (the tensor engine
down-clocks after idling, which would slow conv2's first ~3us).

conv2 PSUM evacuation is split: ACT evacuates slot 2q (with sum
accumulation), DVE evacuates slot 2q+1 (tensor_scalar with accum), so the
fp8 conv2 is not ACT-bound. y2 is exact in f16 (integer-valued, |y2|<=576).

Final stage: t = xhi+xlo (f32, precomputed during conv1), then per chunk
one DVE/Pool scalar_tensor_tensor (w = y2*a2 + t) and one ACT Sign with
per-channel bias, written as fp8 (+-1 exact) and stored per slot.

Toolchain constraints honored: raw Bass only, max one semaphore wait per
instruction, single PSUM reader engine per bank, drain-backed semaphore
increments on every cross-engine RAW edge, explicit DVE drains between
dependent vector ops. DoubleRow matmuls keep dst partition base 0 (ISA
constraint s3d3_mm_valid_dst_partition).
"""
import numpy as np
import ml_dtypes
import concourse.bass as bass
import concourse.mybir as mybir
from concourse import bass_utils
from concourse.ap import AP as APcls
from contextlib import ExitStack

F32 = mybir.dt.float32
BF16 = mybir.dt.bfloat16
F16 = mybir.dt.float16
F8 = mybir.dt.float8e4
F8E5 = mybir.dt.float8e5
AF = mybir.ActivationFunctionType
ALU = mybir.AluOpType
DRM = mybir.MatmulPerfMode.DoubleRow

N_CORES = 8
N, C, H, W = 64, 64, 56, 56
IMGS = N // N_CORES          # 8 images per core
SLOTS = IMGS // 2            # 4 slots (2 images per slot)
QG = SLOTS // 2              # 2 quadgroups (4 images each)
HP = H + 2                   # 58 padded
CHROWS = 8                   # output rows per 448-subchunk
CHUNK = CHROWS * W           # 448
NCH = H // CHROWS            # 7 subchunks per image
SUPERS = [(0, 2), (2, 4), (4, 6), (6, 7)]   # subchunk ranges per super-iter
NSUP = len(SUPERS)           # 4 super-iters per quadgroup
ITERS = QG * NSUP            # 8 super-iters per conv
PERIMG = H * W               # 3136
YCOLS = SLOTS * PERIMG       # 12544
SLOTPIX = HP * HP            # 3364
N_TOT = float(N * H * W)     # global batch-stat count
EPS = 1e-5
NF = SLOTS * 2               # 8 final-stage chunks (half-slots of 1568)
PAIRS = [(0, 1), (2, 3), (4, 5), (6, 7), (8, 9)]  # conv2 tap pairs (9=zero)
POOL_J = ()   # final iterations handled by GPSIMD (stt not Pool-legal)

W0_DUMMIES = 30              # PE warmup fillers (initial load latency)
W1_DUMMIES = 102             # PE fillers across the stats1 barrier

DEBUG = False
CC_STUB = False   # replace AllReduce with a local DMA (for TimelineSim)

# row chunks per slot for the staged input loads
ROWCH = [(0, 18), (18, 34), (34, 50), (50, 58)]


def build_bass():
    nc = bass.Bass(trn_type="TRN2", target_bir_lowering=False, debug=False,
                   num_devices=N_CORES)

    d_xhi = nc.dram_tensor("xhi", [128, SLOTS, HP, HP], F16, kind="ExternalInput")
    d_xm8 = nc.dram_tensor("xm8", [128, SLOTS, HP, HP], F8, kind="ExternalInput")
    d_xl8 = nc.dram_tensor("xl8", [128, SLOTS, HP, HP], F8, kind="ExternalInput")
    d_wf16 = nc.dram_tensor("wf16", [128, 576], F16, kind="ExternalInput")
    d_w8 = nc.dram_tensor("w8", [128, 1280], F8, kind="ExternalInput")
    d_w1m = nc.dram_tensor("w1m", [128, 1280], F8E5, kind="ExternalInput")
    d_consts = nc.dram_tensor("consts", [128, 8], F32, kind="ExternalInput")
    d_tq = nc.dram_tensor("tq", [128, YCOLS], F32, kind="ExternalInput")
    d_out = nc.dram_tensor("outp", [128, YCOLS], F8, kind="ExternalOutput")
    db1_in = nc.dram_tensor("db1_in", [64, 4], F32)
    db1_out = nc.dram_tensor("db1_out", [64, 4], F32, addr_space="Shared")
    db2_in = nc.dram_tensor("db2_in", [64, 4], F32)
    db2_out = nc.dram_tensor("db2_out", [64, 4], F32, addr_space="Shared")

    es = ExitStack()
    def sb(name, shape, dt):
        return es.enter_context(nc.sbuf_tensor(name, shape, dt))
    def ps(name, shape, dt):
        return es.enter_context(nc.psum_tensor(name, shape, dt))
    def sem(name):
        return es.enter_context(nc.semaphore(name))

    xhi = sb("xhi_t", [128, SLOTS, HP, HP], F16)
    xm8 = sb("xm8_t", [128, SLOTS, HP, HP], F8)
    xl8 = sb("xl8_t", [128, SLOTS, HP, HP], F8)
    wf16 = sb("wf16_t", [128, 576], F16)
    w8 = sb("w8_t", [128, 1280], F8)
    w1m = sb("w1m_t", [128, 1280], F8E5)
    consts = sb("consts_t", [128, 8], F32)
    bin1 = sb("bin1_t", [128, SLOTS, HP, HP], F8)
    tq = sb("tq_t", [128, YCOLS], F32)
    y1 = sb("y1_t", [128, YCOLS], F32)
    # y2 (f16) and the fp8 output live in y1's bytes (dead regions by then):
    #   y2v   = f16 cols 0..12543     (y1 f32 cols 0..6271   = slots 0,1)
    #   outv  = f8 cols 25088..37631  (y1 f32 cols 6272..9407 = slots 2,3lo)
    y2v = y1[:].bitcast(F16)
    outv = y1[:].bitcast(F8)
    OUTOFF = 2 * YCOLS
    NPART = 2 * ITERS            # partial columns per conv
    ps1 = sb("ps1", [128, NPART], F32)
    pq1 = sb("pq1", [128, NPART], F32)
    ps2 = sb("ps2", [128, NPART], F32)
    pq2 = sb("pq2", [128, NPART], F32)
    stats1 = sb("stats1", [128, 2], F32)
    stats2 = sb("stats2", [128, 2], F32)
    glob1 = sb("glob1", [128, 8], F32)
    glob2 = sb("glob2", [128, 8], F32)
    scr = sb("scr", [128, 2 * CHUNK], F32)
    scr16 = scr[:].bitcast(F16)
    wbuf = [sb(f"wb{i}", [128, PERIMG // 2], F32) for i in range(2)]
    dscr = sb("dscr", [128, 512], F16)
    pbX = [ps(f"pbX{i}", [128, 1024], F32) for i in range(2)]
    pbY = [ps(f"pbY{i}", [128, 1024], F32) for i in range(2)]

    dsem = sem("dsem")
    s_pe1 = sem("s_pe1"); s_ev1 = sem("s_ev1")
    s_pe2 = sem("s_pe2"); s_ev2 = sem("s_ev2"); s_dv2 = sem("s_dv2")
    s_sg1 = sem("s_sg1"); s_ms = sem("s_ms")
    s_st1 = sem("s_st1"); s_st2 = sem("s_st2"); s_acst = sem("s_acst")
    s_cc = sem("s_cc")
    s_fvd = sem("s_fvd"); s_fvp = sem("s_fvp"); s_fs = sem("s_fs")

    CCV = 16 if CC_STUB else 1

    def ycol(slot, c):
        return slot * PERIMG + c * CHUNK

    HCOLS = PERIMG // 2          # 1568
    FINALS = [(s, h) for s in range(SLOTS) for h in (0, 1)]

    # ---- input load schedule --------------------------------------------
    # list of (sbuf_dst_ap_fn, dram_src_ap_fn); dsem marks derived from index
    loads = []
    def add_load(dst, src):
        loads.append((dst, src))
        return len(loads)  # 1-based count

    add_load(wf16[:], d_wf16[:])
    add_load(w1m[:], d_w1m[:])
    for t, dt_ in ((xhi, d_xhi), (xm8, d_xm8), (xl8, d_xl8)):
        for s in (0, 1):
            add_load(t[:, s, 0:18], dt_[:, s, 0:18])
    D_S0 = len(loads) * 16
    add_load(w8[:], d_w8[:])
    add_load(consts[:], d_consts[:])
    for (r0, r1) in ROWCH[1:]:
        for t, dt_ in ((xhi, d_xhi), (xm8, d_xm8), (xl8, d_xl8)):
            for s in (0, 1):
                add_load(t[:, s, r0:r1], dt_[:, s, r0:r1])
        if r1 == 34:
            D_S1 = len(loads) * 16
        elif r1 == 50:
            D_S2 = len(loads) * 16
        else:
            D_S3 = len(loads) * 16
    for s in (2, 3):
        for t, dt_ in ((xhi, d_xhi), (xm8, d_xm8), (xl8, d_xl8)):
            add_load(t[:, s], dt_[:, s])
    D_QG1 = len(loads) * 16
    for s in range(SLOTS):
        add_load(tq[:, s * PERIMG : (s + 1) * PERIMG],
                 d_tq[:, s * PERIMG : (s + 1) * PERIMG])
    D_TQ = len(loads) * 16
    NLOADS = len(loads)
    D_B1ST = (NLOADS + 2) * 16
    D_G1 = (NLOADS + 4) * 16
    D_B2ST = (NLOADS + 6) * 16
    D_G2 = (NLOADS + 8) * 16

    CONV1_GATES = {(0, 0): D_S0, (0, 1): D_S1, (0, 2): D_S2, (0, 3): D_S3,
                   (1, 0): D_QG1}

    with nc.Block() as block:

        @block.sync
        def _(sync):
            for dst, src in loads:
                sync.dma_start(dst, src).then_inc(dsem, 16)
            # stats chains: store half 2 / load half 2 ride on ACT and Pool
            sync.wait_ge(s_st1, 1)
            sync.dma_start(db1_in[:, 0:2], stats1[0:64, 0:2]).then_inc(dsem, 16)
            sync.wait_ge(s_cc, CCV)
            sync.dma_start(glob1[0:64, 0:4], db1_out[:]).then_inc(dsem, 16)
            sync.wait_ge(s_st2, 1)
            sync.dma_start(db2_in[:, 0:2], stats2[0:64, 0:2]).then_inc(dsem, 16)
            sync.wait_ge(s_cc, 2 * CCV)
            sync.dma_start(glob2[0:64, 0:4], db2_out[:]).then_inc(dsem, 16)
            # output stores, one per final chunk
            for k in range(NF):
                sl, h = FINALS[k]
                off = sl * PERIMG + h * HCOLS
                sync.wait_ge(s_fs, k + 1)
                sync.dma_start(
                    d_out[:, off : off + HCOLS],
                    outv[:, OUTOFF + off : OUTOFF + off + HCOLS],
                ).then_inc(dsem, 16)

        @block.tensor
        def _(tensor):
            def dummy64(n):
                # tiny fillers into never-read psum columns (960:1024)
                nc.tensor.ldweights(dscr[:, 0:64], tile_position=(0, 0))
                for i in range(n):
                    nc.tensor.matmul(pbX[0][0:64, 960:1024], dscr[:, 0:64],
                                     dscr[:, 64:128], start=True, stop=True,
                                     tile_position=(0, 0),
                                     skip_group_check=True)

            def dummy(n):
                # keep the PE p-state hot: harmless f16 matmuls into a
                # region of pbX[0] that is dead at every dummy site
                for i in range(n):
                    nc.tensor.ldweights(dscr[:, 0:64], tile_position=(0, 0))
                    nc.tensor.matmul(pbX[0][0:64, 0:448], dscr[:, 0:64],
                                     dscr[:, 64:512], start=True, stop=True,
                                     tile_position=(0, 0),
                                     skip_group_check=True)

            tensor.wait_ge(s_ms, 1)
            dummy(W0_DUMMIES)

            # conv1: f16 hi/lo, 4 quadrants (4 images concurrent on HW)
            it = 0
            for q in range(QG):
                for si, (c0, c1) in enumerate(SUPERS):
                    gate = CONV1_GATES.get((q, si))
                    if gate is not None:
                        tensor.wait_ge(dsem, gate)
                    nsub = c1 - c0
                    if it >= 2:
                        tensor.wait_ge(s_ev1, it - 1)
                    pX = pbX[it % 2]
                    pY = pbY[it % 2]
                    quads = [
                        ((0, 0), slice(0, 64), 2 * q, pX, slice(0, 64)),
                        ((64, 0), slice(64, 128), 2 * q, pY, slice(0, 64)),
                        ((0, 64), slice(0, 64), 2 * q + 1, pX, slice(64, 128)),
                        ((64, 64), slice(64, 128), 2 * q + 1, pY,
                         slice(64, 128)),
                    ]
                    for tap in range(9):
                        kh, kw = tap // 3, tap % 3
                        wcol = tap * 64
                        for tp, rows, _, _, _ in quads:
                            nc.tensor.ldweights(wf16[rows, wcol : wcol + 64],
                                                tile_position=tp)
                        for tp, rows, dslot, pdst, phalf in quads:
                            for s in range(nsub):
                                c = c0 + s
                                rap = xhi[rows, dslot,
                                          c * CHROWS + kh :
                                          c * CHROWS + kh + CHROWS,
                                          kw : kw + W]
                                nc.tensor.matmul(
                                    pdst[phalf, s * 512 : s * 512 + CHUNK],
                                    wf16[rows, wcol : wcol + 64], rap,
                                    start=(tap == 0), stop=False,
                                    tile_position=tp,
                                    skip_group_check=True)
                    # fp8 mid/lo limbs: block-diagonal DoubleRow, weights
                    # are +-2^-12 so they accumulate into the same groups
                    for li, lt in enumerate((xm8, xl8)):
                        for ip, (ta, tb) in enumerate(PAIRS):
                            kha, kwa = ta // 3, ta % 3
                            if tb == 9:
                                delta = -HP
                            else:
                                delta = (tb // 3 - kha) * HP + (tb % 3 - kwa)
                            wap = APcls(tensor=w1m[:].tensor, offset=ta * 128,
                                        ap=[[1280, 128], [128, 2], [1, 128]])
                            nc.tensor.ldweights(wap, perf_mode=DRM)
                            for sj in range(2):
                                slot = 2 * q + sj
                                pdst = pX if sj == 0 else pY
                                for s in range(nsub):
                                    c = c0 + s
                                    offa = (slot * SLOTPIX
                                            + (c * CHROWS + kha) * HP + kwa)
                                    rap = APcls(
                                        tensor=lt[:].tensor, offset=offa,
                                        ap=[[SLOTS * SLOTPIX, 128], [delta, 2],
                                            [HP, CHROWS], [1, W]])
                                    nc.tensor.matmul(
                                        pdst[:, s * 512 : s * 512 + CHUNK],
                                        wap, rap, start=False,
                                        stop=(li == 1 and ip == 4),
                                        perf_mode=DRM, skip_group_check=True)
                    tensor.drain().then_inc(s_pe1, 1)
                    it += 1

            # fill the stats1 -> bin1 barrier (evac of it=14 must be done
            # before reusing pbX[0]; evac15 targets pbX[1]/pbY[1])
            tensor.wait_ge(s_ev1, ITERS - 1)
            dummy(W1_DUMMIES)

            # conv2: fp8 block-diagonal DoubleRow, 5 tap-pairs
            it = 0
            for q in range(QG):
                tensor.wait_ge(s_sg1, 2 if q == 0 else 4)
                for si, (c0, c1) in enumerate(SUPERS):
                    nsub = c1 - c0
                    nd64 = {2: 85}.get(it, 0)
                    if nd64:
                        dummy64(nd64)
                    if it >= 2:
                        tensor.wait_ge(s_ev2, it - 1)
                        tensor.wait_ge(s_dv2, it - 1)
                    pX = pbX[it % 2]
                    pY = pbY[it % 2]
                    for ip, (ta, tb) in enumerate(PAIRS):
                        kha, kwa = ta // 3, ta % 3
                        if tb == 9:
                            delta = -58  # zero weights; any in-bounds window
                        else:
                            delta = (tb // 3 - kha) * HP + (tb % 3 - kwa)
                        wap = APcls(tensor=w8[:].tensor, offset=ta * 128,
                                    ap=[[1280, 128], [128, 2], [1, 128]])
                        nc.tensor.ldweights(wap, perf_mode=DRM)
                        for sj in range(2):
                            slot = 2 * q + sj
                            pdst = pX if sj == 0 else pY
                            for s in range(nsub):
                                c = c0 + s
                                offa = (slot * SLOTPIX
                                        + (c * CHROWS + kha) * HP + kwa)
                                rap = APcls(
                                    tensor=bin1[:].tensor, offset=offa,
                                    ap=[[SLOTS * SLOTPIX, 128], [delta, 2],
                                        [HP, CHROWS], [1, W]])
                                nc.tensor.matmul(
                                    pdst[:, s * 512 : s * 512 + CHUNK],
                                    wap, rap, start=(ip == 0), stop=(ip == 4),
                                    perf_mode=DRM, skip_group_check=True)
                    tensor.drain().then_inc(s_pe2, 1)
                    it += 1

        @block.scalar
        def _(scalar):
            # conv1 evacs: PSUM -> y1 (f32) with sum accumulation
            it = 0
            for q in range(QG):
                for (c0, c1) in SUPERS:
                    nsub = c1 - c0
                    scalar.wait_ge(s_pe1, it + 1)
                    pX = pbX[it % 2]
                    pY = pbY[it % 2]
                    for half, slot, pt in ((0, 2 * q, pX), (1, 2 * q + 1, pY)):
                        src = pt[:, 0 : nsub * 512].rearrange(
                            "p (s k) -> p s k", s=nsub)[:, :, 0:CHUNK]
                        nc.scalar.activation(
                            y1[:, ycol(slot, c0) :
                               ycol(slot, c0) + nsub * CHUNK],
                            src, AF.Copy,
                            accum_out=ps1[:, 2 * it + half :
                                          2 * it + half + 1])
                    scalar.drain().then_inc(s_ev1, 1)
                    it += 1
            # stats1: store the bottom half's partials, then sqrt(var+eps)
            scalar.wait_ge(s_st1, 1)
            nc.scalar.dma_start(db1_in[:, 2:4],
                                stats1[64:128, 0:2]).then_inc(dsem, 16)
            scalar.wait_ge(s_cc, CCV)
            nc.scalar.dma_start(glob1[64:128, 0:4], db1_out[:]).then_inc(dsem, 16)
            scalar.wait_ge(s_st1, 2)
            nc.scalar.activation(glob1[:, 2:3], glob1[:, 3:4], AF.Sqrt,
                                 bias=consts[:, 4:5])
            scalar.drain().then_inc(s_acst, 1)
            # bin1 = Sign(y1 * a1 + b1) into padded fp8 slots
            scalar.wait_ge(s_ms, 5)
            scalar.wait_ge(s_st1, 3)
            def sign1(s):
                nc.scalar.activation(
                    bin1[:, s, 1 : 1 + H, 1 : 1 + W],
                    y1[:, s * PERIMG : (s + 1) * PERIMG],
                    AF.Sign, bias=glob1[:, 7:8], scale=glob1[:, 6:7])
                scalar.drain().then_inc(s_sg1, 1)
            sign1(0)
            sign1(1)
            sign1(2)
            sign1(3)

            # conv2 evacs of pX (slot 2q) with accum; slots 2,3 signs woven in
            def evac2(itv, c0, nsub, q):
                scalar.wait_ge(s_pe2, itv + 1)
                pX = pbX[itv % 2]
                src = pX[:, 0 : nsub * 512].rearrange(
                    "p (s k) -> p s k", s=nsub)[:, :, 0:CHUNK]
                nc.scalar.activation(
                    y2v[:, ycol(2 * q, c0) : ycol(2 * q, c0) + nsub * CHUNK],
                    src, AF.Copy,
                    accum_out=ps2[:, 2 * itv : 2 * itv + 1])
                scalar.drain().then_inc(s_ev2, 1)

            it = 0
            for q in range(QG):
                for si, (c0, c1) in enumerate(SUPERS):
                    evac2(it, c0, c1 - c0, q)
                    it += 1
            # stats2: bottom-half store, then sqrt
            scalar.wait_ge(s_st2, 1)
            nc.scalar.dma_start(db2_in[:, 2:4],
                                stats2[64:128, 0:2]).then_inc(dsem, 16)
            scalar.wait_ge(s_cc, 2 * CCV)
            nc.scalar.dma_start(glob2[64:128, 0:4], db2_out[:]).then_inc(dsem, 16)
            scalar.wait_ge(s_st2, 2)
            nc.scalar.activation(glob2[:, 2:3], glob2[:, 3:4], AF.Sqrt,
                                 bias=consts[:, 4:5])
            scalar.drain().then_inc(s_acst, 2)
            # final: out = Sign(w + b2'), w produced by DVE/Pool
            for j in range(NF):
                sl, h = FINALS[j]
                off = sl * PERIMG + h * HCOLS
                scalar.wait_ge(s_fvd, j + 1)
                nc.scalar.activation(
                    outv[:, OUTOFF + off : OUTOFF + off + HCOLS],
                    wbuf[j % 2][:, 0:HCOLS], AF.Sign,
                    bias=glob2[:, 7:8])
                scalar.drain().then_inc(s_fs, 1)

        @block.vector
        def _(vector):
            # conv1 sumsq partials
            it = 0
            for q in range(QG):
                for (c0, c1) in SUPERS:
                    nsub = c1 - c0
                    vector.wait_ge(s_ev1, it + 1)
                    for half, slot in ((0, 2 * q), (1, 2 * q + 1)):
                        yc = y1[:, ycol(slot, c0) :
                                ycol(slot, c0) + nsub * CHUNK]
                        nc.vector.scalar_tensor_tensor(
                            out=scr[:, 0 : nsub * CHUNK], in0=yc,
                            scalar=1.0, in1=yc,
                            op0=ALU.mult, op1=ALU.mult,
                            accum_out=pq1[:, 2 * it + half :
                                          2 * it + half + 1])
                    it += 1

            def stats(pstats_s, pstats_q, st, dsem_in, acst_v, statst, g,
                      which):
                nc.vector.drain()
                nc.vector.reduce_sum(statst[:, 0:1], pstats_s[:],
                                     axis=mybir.AxisListType.X)
                nc.vector.reduce_sum(statst[:, 1:2], pstats_q[:],
                                     axis=mybir.AxisListType.X)
                nc.vector.drain().then_inc(st, 1)
                vector.wait_ge(dsem, dsem_in)
                # halves side by side: fold on-partition, then bn math
                nc.vector.tensor_tensor(out=g[:, 4:6], in0=g[:, 0:2],
                                        in1=g[:, 2:4], op=ALU.add)
                nc.vector.drain()
                nc.vector.tensor_scalar_mul(g[:, 0:1], g[:, 4:5], 1.0 / N_TOT)
                nc.vector.tensor_scalar_mul(g[:, 1:2], g[:, 5:6], 1.0 / N_TOT)
                nc.vector.drain()
                nc.vector.tensor_tensor(out=g[:, 2:3], in0=g[:, 0:1],
                                        in1=g[:, 0:1], op=ALU.mult)
                nc.vector.drain()
                nc.vector.tensor_tensor(out=g[:, 3:4], in0=g[:, 1:2],
                                        in1=g[:, 2:3], op=ALU.subtract)
                nc.vector.drain().then_inc(st, 1)
                # ACT: g[:,2:3] = sqrt(g[:,3:4] + eps)
                vector.wait_ge(s_acst, acst_v)
                gcol, bcol = 2 * which, 2 * which + 1
                nc.vector.reciprocal(g[:, 3:4], g[:, 2:3])
                nc.vector.drain()
                nc.vector.tensor_tensor(out=g[:, 6:7], in0=g[:, 3:4],
                                        in1=consts[:, gcol : gcol + 1],
                                        op=ALU.mult)
                nc.vector.drain()
                nc.vector.tensor_tensor(out=g[:, 4:5], in0=g[:, 0:1],
                                        in1=g[:, 6:7], op=ALU.mult)
                nc.vector.drain()
                nc.vector.tensor_tensor(out=g[:, 7:8],
                                        in0=consts[:, bcol : bcol + 1],
                                        in1=g[:, 4:5], op=ALU.subtract)
                nc.vector.drain().then_inc(st, 1)

            stats(ps1, pq1, s_st1, D_G1, 1, stats1, glob1, 0)

            # conv2: DVE evacs pY (slot 2q+1) + both sumsq partials
            it = 0
            for q in range(QG):
                for (c0, c1) in SUPERS:
                    nsub = c1 - c0
                    vector.wait_ge(s_pe2, it + 1)
                    pY = pbY[it % 2]
                    src = pY[:, 0 : nsub * 512].rearrange(
                        "p (s k) -> p s k", s=nsub)[:, :, 0:CHUNK]
                    nc.vector.tensor_scalar(
                        y2v[:, ycol(2 * q + 1, c0) :
                            ycol(2 * q + 1, c0) + nsub * CHUNK],
                        src, 1.0, 0.0, ALU.mult, ALU.add,
                        accum_out=ps2[:, 2 * it + 1 : 2 * it + 2])
                    nc.vector.drain()
                    vector.wait_ge(s_ev2, it + 1)
                    for half, slot in ((0, 2 * q), (1, 2 * q + 1)):
                        yc = y2v[:, ycol(slot, c0) :
                                 ycol(slot, c0) + nsub * CHUNK]
                        nc.vector.scalar_tensor_tensor(
                            out=scr16[:, 0 : nsub * CHUNK], in0=yc,
                            scalar=1.0, in1=yc,
                            op0=ALU.mult, op1=ALU.mult,
                            accum_out=pq2[:, 2 * it + half :
                                          2 * it + half + 1])
                    nc.vector.drain().then_inc(s_dv2, 1)
                    it += 1

            stats(ps2, pq2, s_st2, D_G2, 2, stats2, glob2, 1)

            # final w = y2 * a2 + tq
            vector.wait_ge(dsem, D_TQ)
            for j in range(NF):
                sl, h = FINALS[j]
                off = sl * PERIMG + h * HCOLS
                if j >= 2:
                    vector.wait_ge(s_fs, j - 1)
                nc.vector.scalar_tensor_tensor(
                    out=wbuf[j % 2][:, 0:HCOLS],
                    in0=y2v[:, off : off + HCOLS],
                    scalar=glob2[:, 6:7],
                    in1=tq[:, off : off + HCOLS],
                    op0=ALU.mult, op1=ALU.add)
                nc.vector.drain().then_inc(s_fvd, 1)

        @block.gpsimd
        def _(gpsimd):
            nc.gpsimd.memset(dscr[:], 0).then_inc(s_ms, 1)
            for s in range(SLOTS):
                nc.gpsimd.memset(bin1[:, s], 0).then_inc(s_ms, 1)
            gpsimd.wait_ge(dsem, D_B1ST)
            if CC_STUB:
                nc.gpsimd.dma_start(db1_out[:], db1_in[:]).then_inc(s_cc, 16)
            else:
                nc.gpsimd.collective_compute(
                    "AllReduce", ALU.add, replica_groups=[list(range(N_CORES))],
                    ins=[db1_in[:]], outs=[db1_out[:]]).then_inc(s_cc, 1)
            gpsimd.wait_ge(dsem, D_B2ST)
            if CC_STUB:
                nc.gpsimd.dma_start(db2_out[:], db2_in[:]).then_inc(s_cc, 16)
            else:
                nc.gpsimd.collective_compute(
                    "AllReduce", ALU.add, replica_groups=[list(range(N_CORES))],
                    ins=[db2_in[:]], outs=[db2_out[:]]).then_inc(s_cc, 1)

    return nc


_CACHE = {}


def _get_nc():
    if "nc" not in _CACHE:
        _CACHE["nc"] = build_bass()
    return _CACHE["nc"]


def kernel(x, w1, gamma1, beta1, w2, gamma2, beta2):
    x = np.asarray(x, np.float32)
    w1 = np.asarray(w1, np.float32)
    w2 = np.asarray(w2, np.float32)
    gamma1 = np.asarray(gamma1, np.float32)
    beta1 = np.asarray(beta1, np.float32)
    gamma2 = np.asarray(gamma2, np.float32)
    beta2 = np.asarray(beta2, np.float32)

    f8np = mybir.dt.np(F8)

    # conv1 weights: [tap, cin, cout] -> [cin, tap*cout], rows duplicated
    wb1 = np.where(w1 >= 0, 1.0, -1.0).astype(np.float32)
    wt1a = wb1.transpose(1, 2, 3, 0).reshape(64, 9, 64)
    wt1 = wt1a.reshape(64, 576)
    wf16_np = np.concatenate([wt1, wt1], axis=0).astype(np.float16)
    # fp8 limb weights: block-diagonal +-2^-12 (exact e4m3 subnormal)
    w1m_np = np.zeros((128, 1280), np.float32)
    for t in range(9):
        w1m_np[0:64, t * 128 : t * 128 + 64] = wt1a[:, t, :] * 2.0 ** -12
        w1m_np[64:128, t * 128 + 64 : t * 128 + 128] = wt1a[:, t, :] * 2.0 ** -12

    # conv2 weights: fp8 block-diagonal, 10 taps (tap 9 zero)
    wb2 = np.where(w2 >= 0, 1.0, -1.0).astype(np.float32)
    wt2 = wb2.transpose(1, 2, 3, 0).reshape(64, 9, 64)  # [cin, tap, cout]
    w8_np = np.zeros((128, 1280), np.float32)
    for t in range(9):
        w8_np[0:64, t * 128 : t * 128 + 64] = wt2[:, t, :]
        w8_np[64:128, t * 128 + 64 : t * 128 + 128] = wt2[:, t, :]
    w8_np = w8_np.astype(f8np)
    w1m_np = w1m_np.astype(mybir.dt.np(F8E5))

    consts_np = np.zeros((128, 8), np.float32)
    for col, v in enumerate([gamma1, beta1, gamma2, beta2]):
        consts_np[0:64, col] = v
        consts_np[64:128, col] = v
    consts_np[:, 4] = EPS

    in_maps = []
    for k in range(N_CORES):
        xc = x[IMGS * k : IMGS * (k + 1)]            # [8, 64, 56, 56]
        xp = np.zeros((IMGS, C, HP, HP), np.float32)
        xp[:, :, 1 : 1 + H, 1 : 1 + W] = xc
        arr = xp.reshape(SLOTS, 2, C, HP, HP).transpose(1, 2, 0, 3, 4)
        arr = np.ascontiguousarray(arr).reshape(128, SLOTS, HP, HP)
        ahi = arr.astype(np.float16)
        r1 = arr - ahi.astype(np.float32)
        m8 = (r1 * 4096.0).astype(f8np)
        r2 = r1 - m8.astype(np.float32) / 4096.0
        l8 = (r2 * 4096.0).astype(f8np)
        # Q-pack the fp8 limbs so the block-diagonal limb matmuls land in the
        # same (permuted) arrangement the hi quadrant pass produces
        m8q = np.empty_like(m8)
        l8q = np.empty_like(l8)
        for s in range(SLOTS):
            for h in (0, 1):
                img = 4 * (s // 2) + (s % 2) + 2 * h
                sp, hp_ = img // 2, img % 2
                m8q[h * 64 : h * 64 + 64, s] = m8[hp_ * 64 : hp_ * 64 + 64, sp]
                l8q[h * 64 : h * 64 + 64, s] = l8[hp_ * 64 : hp_ * 64 + 64, sp]
        # conv1's quadrant pattern permutes (slot, half): y slot s half h holds
        # image Q(s,h) = 4*(s//2) + s%2 + 2*h. The final residual add needs x
        # in that same arrangement.
        tq_np = np.empty((128, SLOTS, PERIMG), np.float32)
        for s in range(SLOTS):
            for h in (0, 1):
                img = 4 * (s // 2) + (s % 2) + 2 * h
                tq_np[h * 64 : h * 64 + 64, s] = xc[img].reshape(C, PERIMG)
        in_maps.append({
            "xhi": ahi, "xm8": m8q, "xl8": l8q, "wf16": wf16_np,
            "w8": w8_np, "w1m": w1m_np, "consts": consts_np,
            "tq": tq_np.reshape(128, YCOLS),
        })

    nc = _get_nc()
    res = bass_utils.run_bass_kernel_spmd(nc, in_maps, core_ids=list(range(N_CORES)))

    out = np.empty((N, C, H, W), np.float32)
    for k in range(N_CORES):
        o = np.asarray(res.results[k]["outp"]).astype(np.float32)  # [128, 12544]
        o = o.reshape(2, C, SLOTS, H, W)  # [half, ch, slot, H, W]
        for s in range(SLOTS):
            for h in (0, 1):
                img = 4 * (s // 2) + (s % 2) + 2 * h
                out[IMGS * k + img] = o[h, :, s]
    return out


if __name__ == "__main__":
    rng = np.random.default_rng(0)
    xs = rng.standard_normal((N, C, H, W)).astype(np.float32)
    w1s = (rng.standard_normal((C, C, 3, 3)) * 0.1).astype(np.float32)
    w2s = (rng.standard_normal((C, C, 3, 3)) * 0.1).astype(np.float32)
    ones = np.ones(C, np.float32)
    zeros = np.zeros(C, np.float32)
    r = kernel(x=xs, w1=w1s, gamma1=ones, beta1=zeros, w2=w2s, gamma2=ones,
               beta2=zeros)
    print("ran, out uniq:", np.unique(r))


# revision 18
# speedup vs baseline: 1.2364x; 1.0274x over previous
"""BinaryBasicBlock TRN2 kernel: 8-core batch-parallel, raw Bass.

Reference computation (per core: 8 images, C=64, 56x56):
  y1   = conv3x3(x, sign(w1))            # exact: x = fp16(x) + fp16(residual)
  bin1 = sign((y1 - mu1) * rsqrt(var1+eps) * g1 + b1)   # global batch stats
  y2   = conv3x3(bin1, sign(w2))         # exact (+-1 x +-1 in fp8)
  out  = sign((y2 - mu2) * rsqrt(var2+eps) * g2 + b2 + x)

Batch stats are exact: per-core (sum, sumsq) partials are AllReduced across
the 8 cores mid-kernel (both partition halves stored side by side as [64,4]
so no on-chip cross-partition fold is needed).

Layout: channels on partitions, 2 images per 128 partitions (top/bottom
halves), 4 "slots" of [128, 58, 58] padded images per core.

conv1 runs as 9-tap f16 matmul accumulation with all four 64x64 PE quadrants
streaming four different images concurrently (hi + lo pass for fp32
exactness). conv2 runs on fp8 inputs (bin1 is +-1, exact in e4m3) as
full-128 block-diagonal DoubleRow matmuls: weights hold both partition
halves' 64x64 blocks on the diagonal and each matmul processes a pair of
taps, so 5 matmuls replace 9 per (slot, subchunk).

The PE p-state ramp is kept hot across the conv1->conv2 stats barrier by a
stream of filler matmuls into a scratch PSUM region (the tensor engine
down-clocks after idling, which would slow conv2's first ~3us).

conv2 PSUM evacuation is split: ACT evacuates slot 2q (with sum
accumulation), DVE evacuates slot 2q+1 (tensor_scalar with accum), so the
fp8 conv2 is not ACT-bound. y2 is exact in f16 (integer-valued, |y2|<=576).

Final stage: t = xhi+xlo (f32, precomputed during conv1), then per chunk
one DVE/Pool scalar_tensor_tensor (w = y2*a2 + t) and one ACT Sign with
per-channel bias, written as fp8 (+-1 exact) and stored per slot.

Toolchain constraints honored: raw Bass only, max one semaphore wait per
instruction, single PSUM reader engine per bank, drain-backed semaphore
increments on every cross-engine RAW edge, explicit DVE drains between
dependent vector ops. DoubleRow matmuls keep dst partition base 0 (ISA
constraint s3d3_mm_valid_dst_partition).
"""
import numpy as np
import ml_dtypes
import concourse.bass as bass
import concourse.mybir as mybir
from concourse import bass_utils
from concourse.ap import AP as APcls
from contextlib import ExitStack

F32 = mybir.dt.float32
BF16 = mybir.dt.bfloat16
F16 = mybir.dt.float16
F8 = mybir.dt.float8e4
F8E5 = mybir.dt.float8e5
AF = mybir.ActivationFunctionType
ALU = mybir.AluOpType
DRM = mybir.MatmulPerfMode.DoubleRow

N_CORES = 8
N, C, H, W = 64, 64, 56, 56
IMGS = N // N_CORES          # 8 images per core
SLOTS = IMGS // 2            # 4 slots (2 images per slot)
QG = SLOTS // 2              # 2 quadgroups (4 images each)
HP = H + 2                   # 58 padded
CHROWS = 8                   # output rows per 448-subchunk
CHUNK = CHROWS * W           # 448
NCH = H // CHROWS            # 7 subchunks per image
SUPERS = [(0, 2), (2, 4), (4, 6), (6, 7)]   # subchunk ranges per super-iter
NSUP = len(SUPERS)           # 4 super-iters per quadgroup
ITERS = QG * NSUP            # 8 super-iters per conv
PERIMG = H * W               # 3136
YCOLS = SLOTS * PERIMG       # 12544
SLOTPIX = HP * HP            # 3364
N_TOT = float(N * H * W)     # global batch-stat count
EPS = 1e-5
NF = SLOTS * 2               # 8 final-stage chunks (half-slots of 1568)
PAIRS = [(0, 1), (2, 3), (4, 5), (6, 7), (8, 9)]  # conv2 tap pairs (9=zero)
POOL_J = ()   # final iterations handled by GPSIMD (stt not Pool-legal)

W0_DUMMIES = 30              # PE warmup fillers (initial load latency)
W1_DUMMIES = 102             # PE fillers across the stats1 barrier

DEBUG = False
CC_STUB = False   # replace AllReduce with a local DMA (for TimelineSim)

# row chunks per slot for the staged input loads
ROWCH = [(0, 18), (18, 34), (34, 50), (50, 58)]


def build_bass():
    nc = bass.Bass(trn_type="TRN2", target_bir_lowering=False, debug=False,
                   num_devices=N_CORES)

    d_xhi = nc.dram_tensor("xhi", [128, SLOTS, HP, HP], F16, kind="ExternalInput")
    d_xm8 = nc.dram_tensor("xm8", [128, SLOTS, HP, HP], F8, kind="ExternalInput")
    d_xl8 = nc.dram_tensor("xl8", [128, SLOTS, HP, HP], F8, kind="ExternalInput")
    d_wf16 = nc.dram_tensor("wf16", [128, 576], F16, kind="ExternalInput")
    d_w8 = nc.dram_tensor("w8", [128, 1280], F8, kind="ExternalInput")
    d_w1m = nc.dram_tensor("w1m", [128, 1280], F8E5, kind="ExternalInput")
    d_consts = nc.dram_tensor("consts", [128, 8], F32, kind="ExternalInput")
    d_tq = nc.dram_tensor("tq", [128, YCOLS], F32, kind="ExternalInput")
    d_out = nc.dram_tensor("outp", [128, YCOLS], F8, kind="ExternalOutput")
    db1_in = nc.dram_tensor("db1_in", [64, 4], F32)
    db1_out = nc.dram_tensor("db1_out", [64, 4], F32, addr_space="Shared")
    db2_in = nc.dram_tensor("db2_in", [64, 4], F32)
    db2_out = nc.dram_tensor("db2_out", [64, 4], F32, addr_space="Shared")

    es = ExitStack()
    def sb(name, shape, dt):
        return es.enter_context(nc.sbuf_tensor(name, shape, dt))
    def ps(name, shape, dt):
        return es.enter_context(nc.psum_tensor(name, shape, dt))
    def sem(name):
        return es.enter_context(nc.semaphore(name))

    xhi = sb("xhi_t", [128, SLOTS, HP, HP], F16)
    xm8 = sb("xm8_t", [128, SLOTS, HP, HP], F8)
    xl8 = sb("xl8_t", [128, SLOTS, HP, HP], F8)
    wf16 = sb("wf16_t", [128, 576], F16)
    w8 = sb("w8_t", [128, 1280], F8)
    w1m = sb("w1m_t", [128, 1280], F8E5)
    consts = sb("consts_t", [128, 8], F32)
    bin1 = sb("bin1_t", [128, SLOTS, HP, HP], F8)
    tq = sb("tq_t", [128, YCOLS], F32)
    y1 = sb("y1_t", [128, YCOLS], F32)
    # y2 (f16) and the fp8 output live in y1's bytes (dead regions by then):
    #   y2v   = f16 cols 0..12543     (y1 f32 cols 0..6271   = slots 0,1)
    #   outv  = f8 cols 25088..37631  (y1 f32 cols 6272..9407 = slots 2,3lo)
    y2v = y1[:].bitcast(F16)
    outv = y1[:].bitcast(F8)
    OUTOFF = 2 * YCOLS
    NPART = 2 * ITERS            # partial columns per conv
    ps1 = sb("ps1", [128, NPART], F32)
    pq1 = sb("pq1", [128, NPART], F32)
    ps2 = sb("ps2", [128, NPART], F32)
    pq2 = sb("pq2", [128, NPART], F32)
    stats1 = sb("stats1", [128, 2], F32)
    stats2 = sb("stats2", [128, 2], F32)
    glob1 = sb("glob1", [128, 8], F32)
    glob2 = sb("glob2", [128, 8], F32)
    scr = sb("scr", [128, 2 * CHUNK], F32)
    scr16 = scr[:].bitcast(F16)
    wbuf = [sb(f"wb{i}", [128, PERIMG // 2], F32) for i in range(3)]
    pbuf = sb("pbuf", [128, PERIMG // 2], F32)
    hbuf = sb("hbuf", [128, PERIMG // 2], F32)
    dscr = sb("dscr", [128, 512], F16)
    pbX = [ps(f"pbX{i}", [128, 1024], F32) for i in range(2)]
    pbY = [ps(f"pbY{i}", [128, 1024], F32) for i in range(2)]

    dsem = sem("dsem")
    s_pe1 = sem("s_pe1"); s_ev1 = sem("s_ev1")
    s_pe2 = sem("s_pe2"); s_ev2 = sem("s_ev2"); s_dv2 = sem("s_dv2")
    s_sg1 = sem("s_sg1"); s_ms = sem("s_ms")
    s_st1 = sem("s_st1"); s_st2 = sem("s_st2"); s_acst = sem("s_acst")
    s_cc = sem("s_cc")
    s_fvd = sem("s_fvd"); s_fvp = sem("s_fvp"); s_fs = sem("s_fs")

    CCV = 16 if CC_STUB else 1

    def ycol(slot, c):
        return slot * PERIMG + c * CHUNK

    HCOLS = PERIMG // 2          # 1568
    FINALS = [(s, h) for s in range(SLOTS) for h in (0, 1)]

    # ---- input load schedule --------------------------------------------
    # list of (sbuf_dst_ap_fn, dram_src_ap_fn); dsem marks derived from index
    loads = []
    def add_load(dst, src):
        loads.append((dst, src))
        return len(loads)  # 1-based count

    add_load(wf16[:], d_wf16[:])
    add_load(w1m[:], d_w1m[:])
    for t, dt_ in ((xhi, d_xhi), (xm8, d_xm8), (xl8, d_xl8)):
        for s in (0, 1):
            add_load(t[:, s, 0:18], dt_[:, s, 0:18])
    D_S0 = len(loads) * 16
    add_load(w8[:], d_w8[:])
    add_load(consts[:], d_consts[:])
    for (r0, r1) in ROWCH[1:]:
        for t, dt_ in ((xhi, d_xhi), (xm8, d_xm8), (xl8, d_xl8)):
            for s in (0, 1):
                add_load(t[:, s, r0:r1], dt_[:, s, r0:r1])
        if r1 == 34:
            D_S1 = len(loads) * 16
        elif r1 == 50:
            D_S2 = len(loads) * 16
        else:
            D_S3 = len(loads) * 16
    for s in (2, 3):
        for t, dt_ in ((xhi, d_xhi), (xm8, d_xm8), (xl8, d_xl8)):
            add_load(t[:, s], dt_[:, s])
    D_QG1 = len(loads) * 16
    for s in range(SLOTS):
        add_load(tq[:, s * PERIMG : (s + 1) * PERIMG],
                 d_tq[:, s * PERIMG : (s + 1) * PERIMG])
    D_TQ = len(loads) * 16
    NLOADS = len(loads)
    D_B1ST = (NLOADS + 2) * 16
    D_G1 = (NLOADS + 4) * 16
    D_B2ST = (NLOADS + 6) * 16
    D_G2 = (NLOADS + 8) * 16

    CONV1_GATES = {(0, 0): D_S0, (0, 1): D_S1, (0, 2): D_S2, (0, 3): D_S3,
                   (1, 0): D_QG1}

    with nc.Block() as block:

        @block.sync
        def _(sync):
            for dst, src in loads:
                sync.dma_start(dst, src).then_inc(dsem, 16)
            # stats chains: store half 2 / load half 2 ride on ACT and Pool
            sync.wait_ge(s_st1, 1)
            sync.dma_start(db1_in[:, 0:2], stats1[0:64, 0:2]).then_inc(dsem, 16)
            sync.wait_ge(s_cc, CCV)
            sync.dma_start(glob1[0:64, 0:4], db1_out[:]).then_inc(dsem, 16)
            sync.wait_ge(s_st2, 1)
            sync.dma_start(db2_in[:, 0:2], stats2[0:64, 0:2]).then_inc(dsem, 16)
            sync.wait_ge(s_cc, 2 * CCV)
            sync.dma_start(glob2[0:64, 0:4], db2_out[:]).then_inc(dsem, 16)
            # output stores, one per final chunk
            for k in range(NF):
                sl, h = FINALS[k]
                off = sl * PERIMG + h * HCOLS
                sync.wait_ge(s_fs, k + 1)
                sync.dma_start(
                    d_out[:, off : off + HCOLS],
                    outv[:, OUTOFF + off : OUTOFF + off + HCOLS],
                ).then_inc(dsem, 16)

        @block.tensor
        def _(tensor):
            def dummy64(n):
                # tiny fillers into never-read psum columns (960:1024)
                nc.tensor.ldweights(dscr[:, 0:64], tile_position=(0, 0))
                for i in range(n):
                    nc.tensor.matmul(pbX[0][0:64, 960:1024], dscr[:, 0:64],
                                     dscr[:, 64:128], start=True, stop=True,
                                     tile_position=(0, 0),
                                     skip_group_check=True)

            def dummy(n):
                # keep the PE p-state hot: harmless f16 matmuls into a
                # region of pbX[0] that is dead at every dummy site
                for i in range(n):
                    nc.tensor.ldweights(dscr[:, 0:64], tile_position=(0, 0))
                    nc.tensor.matmul(pbX[0][0:64, 0:448], dscr[:, 0:64],
                                     dscr[:, 64:512], start=True, stop=True,
                                     tile_position=(0, 0),
                                     skip_group_check=True)

            tensor.wait_ge(s_ms, 1)
            dummy(W0_DUMMIES)

            # conv1: f16 hi/lo, 4 quadrants (4 images concurrent on HW)
            it = 0
            for q in range(QG):
                for si, (c0, c1) in enumerate(SUPERS):
                    gate = CONV1_GATES.get((q, si))
                    if gate is not None:
                        tensor.wait_ge(dsem, gate)
                    nsub = c1 - c0
                    if it >= 2:
                        tensor.wait_ge(s_ev1, it - 1)
                    pX = pbX[it % 2]
                    pY = pbY[it % 2]
                    quads = [
                        ((0, 0), slice(0, 64), 2 * q, pX, slice(0, 64)),
                        ((64, 0), slice(64, 128), 2 * q, pY, slice(0, 64)),
                        ((0, 64), slice(0, 64), 2 * q + 1, pX, slice(64, 128)),
                        ((64, 64), slice(64, 128), 2 * q + 1, pY,
                         slice(64, 128)),
                    ]
                    for tap in range(9):
                        kh, kw = tap // 3, tap % 3
                        wcol = tap * 64
                        for tp, rows, _, _, _ in quads:
                            nc.tensor.ldweights(wf16[rows, wcol : wcol + 64],
                                                tile_position=tp)
                        for tp, rows, dslot, pdst, phalf in quads:
                            for s in range(nsub):
                                c = c0 + s
                                rap = xhi[rows, dslot,
                                          c * CHROWS + kh :
                                          c * CHROWS + kh + CHROWS,
                                          kw : kw + W]
                                nc.tensor.matmul(
                                    pdst[phalf, s * 512 : s * 512 + CHUNK],
                                    wf16[rows, wcol : wcol + 64], rap,
                                    start=(tap == 0), stop=False,
                                    tile_position=tp,
                                    skip_group_check=True)
                    # fp8 mid/lo limbs: block-diagonal DoubleRow, weights
                    # are +-2^-12 so they accumulate into the same groups
                    for li, lt in enumerate((xm8, xl8)):
                        for ip, (ta, tb) in enumerate(PAIRS):
                            kha, kwa = ta // 3, ta % 3
                            if tb == 9:
                                delta = -HP
                            else:
                                delta = (tb // 3 - kha) * HP + (tb % 3 - kwa)
                            wap = APcls(tensor=w1m[:].tensor, offset=ta * 128,
                                        ap=[[1280, 128], [128, 2], [1, 128]])
                            nc.tensor.ldweights(wap, perf_mode=DRM)
                            for sj in range(2):
                                slot = 2 * q + sj
                                pdst = pX if sj == 0 else pY
                                for s in range(nsub):
                                    c = c0 + s
                                    offa = (slot * SLOTPIX
                                            + (c * CHROWS + kha) * HP + kwa)
                                    rap = APcls(
                                        tensor=lt[:].tensor, offset=offa,
                                        ap=[[SLOTS * SLOTPIX, 128], [delta, 2],
                                            [HP, CHROWS], [1, W]])
                                    nc.tensor.matmul(
                                        pdst[:, s * 512 : s * 512 + CHUNK],
                                        wap, rap, start=False,
                                        stop=(li == 1 and ip == 4),
                                        perf_mode=DRM, skip_group_check=True)
                    tensor.drain().then_inc(s_pe1, 1)
                    it += 1

            # fill the stats1 -> bin1 barrier (evac of it=14 must be done
            # before reusing pbX[0]; evac15 targets pbX[1]/pbY[1])
            tensor.wait_ge(s_ev1, ITERS - 1)
            dummy(W1_DUMMIES)

            # conv2: fp8 block-diagonal DoubleRow, 5 tap-pairs
            it = 0
            for q in range(QG):
                tensor.wait_ge(s_sg1, 2 if q == 0 else 4)
                for si, (c0, c1) in enumerate(SUPERS):
                    nsub = c1 - c0
                    nd64 = {2: 185}.get(it, 0)
                    if nd64:
                        dummy64(nd64)
                    if it >= 2:
                        tensor.wait_ge(s_ev2, it - 1)
                        tensor.wait_ge(s_dv2, it - 1)
                    pX = pbX[it % 2]
                    pY = pbY[it % 2]
                    for ip, (ta, tb) in enumerate(PAIRS):
                        kha, kwa = ta // 3, ta % 3
                        if tb == 9:
                            delta = -58  # zero weights; any in-bounds window
                        else:
                            delta = (tb // 3 - kha) * HP + (tb % 3 - kwa)
                        wap = APcls(tensor=w8[:].tensor, offset=ta * 128,
                                    ap=[[1280, 128], [128, 2], [1, 128]])
                        nc.tensor.ldweights(wap, perf_mode=DRM)
                        for sj in range(2):
                            slot = 2 * q + sj
                            pdst = pX if sj == 0 else pY
                            for s in range(nsub):
                                c = c0 + s
                                offa = (slot * SLOTPIX
                                        + (c * CHROWS + kha) * HP + kwa)
                                rap = APcls(
                                    tensor=bin1[:].tensor, offset=offa,
                                    ap=[[SLOTS * SLOTPIX, 128], [delta, 2],
                                        [HP, CHROWS], [1, W]])
                                nc.tensor.matmul(
                                    pdst[:, s * 512 : s * 512 + CHUNK],
                                    wap, rap, start=(ip == 0), stop=(ip == 4),
                                    perf_mode=DRM, skip_group_check=True)
                    tensor.drain().then_inc(s_pe2, 1)
                    it += 1

        @block.scalar
        def _(scalar):
            # conv1 evacs: PSUM -> y1 (f32) with sum accumulation
            it = 0
            for q in range(QG):
                for (c0, c1) in SUPERS:
                    nsub = c1 - c0
                    scalar.wait_ge(s_pe1, it + 1)
                    pX = pbX[it % 2]
                    pY = pbY[it % 2]
                    for half, slot, pt in ((0, 2 * q, pX), (1, 2 * q + 1, pY)):
                        src = pt[:, 0 : nsub * 512].rearrange(
                            "p (s k) -> p s k", s=nsub)[:, :, 0:CHUNK]
                        nc.scalar.activation(
                            y1[:, ycol(slot, c0) :
                               ycol(slot, c0) + nsub * CHUNK],
                            src, AF.Copy,
                            accum_out=ps1[:, 2 * it + half :
                                          2 * it + half + 1])
                    scalar.drain().then_inc(s_ev1, 1)
                    it += 1
            # stats1: store the bottom half's partials, then sqrt(var+eps)
            scalar.wait_ge(s_st1, 1)
            nc.scalar.dma_start(db1_in[:, 2:4],
                                stats1[64:128, 0:2]).then_inc(dsem, 16)
            scalar.wait_ge(s_cc, CCV)
            nc.scalar.dma_start(glob1[64:128, 0:4], db1_out[:]).then_inc(dsem, 16)
            scalar.wait_ge(s_st1, 2)
            nc.scalar.activation(glob1[:, 2:3], glob1[:, 3:4], AF.Sqrt,
                                 bias=consts[:, 4:5])
            scalar.drain().then_inc(s_acst, 1)
            # bin1 = Sign(y1 * a1 + b1) into padded fp8 slots
            scalar.wait_ge(s_ms, 5)
            scalar.wait_ge(s_st1, 3)
            def sign1(s):
                nc.scalar.activation(
                    bin1[:, s, 1 : 1 + H, 1 : 1 + W],
                    y1[:, s * PERIMG : (s + 1) * PERIMG],
                    AF.Sign, bias=glob1[:, 7:8], scale=glob1[:, 6:7])
                scalar.drain().then_inc(s_sg1, 1)
            sign1(0)
            sign1(1)
            sign1(2)
            sign1(3)

            # conv2 evacs of pX (slot 2q) with accum; slots 2,3 signs woven in
            def evac2(itv, c0, nsub, q):
                scalar.wait_ge(s_pe2, itv + 1)
                pX = pbX[itv % 2]
                src = pX[:, 0 : nsub * 512].rearrange(
                    "p (s k) -> p s k", s=nsub)[:, :, 0:CHUNK]
                nc.scalar.activation(
                    y2v[:, ycol(2 * q, c0) : ycol(2 * q, c0) + nsub * CHUNK],
                    src, AF.Copy,
                    accum_out=ps2[:, 2 * itv : 2 * itv + 1])
                scalar.drain().then_inc(s_ev2, 1)

            it = 0
            for q in range(QG):
                for si, (c0, c1) in enumerate(SUPERS):
                    evac2(it, c0, c1 - c0, q)
                    it += 1
            # stats2: bottom-half store, then sqrt
            scalar.wait_ge(s_st2, 1)
            nc.scalar.dma_start(db2_in[:, 2:4],
                                stats2[64:128, 0:2]).then_inc(dsem, 16)
            scalar.wait_ge(s_cc, 2 * CCV)
            nc.scalar.dma_start(glob2[64:128, 0:4], db2_out[:]).then_inc(dsem, 16)
            scalar.wait_ge(s_st2, 2)
            nc.scalar.activation(glob2[:, 2:3], glob2[:, 3:4], AF.Sqrt,
                                 bias=consts[:, 4:5])
            scalar.drain().then_inc(s_acst, 2)
            # final: out = Sign(w + b2'), w produced by DVE/Pool
            for j in range(NF):
                sl, h = FINALS[j]
                off = sl * PERIMG + h * HCOLS
                if j == NF - 1:
                    scalar.wait_ge(s_fvp, 1)
                    srcb = pbuf[:, 0:HCOLS]
                else:
                    scalar.wait_ge(s_fvd, j + 1)
                    srcb = wbuf[j % 3][:, 0:HCOLS]
                nc.scalar.activation(
                    outv[:, OUTOFF + off : OUTOFF + off + HCOLS],
                    srcb, AF.Sign, bias=glob2[:, 7:8])
                scalar.drain().then_inc(s_fs, 1)

        @block.vector
        def _(vector):
            # conv1 sumsq partials
            it = 0
            for q in range(QG):
                for (c0, c1) in SUPERS:
                    nsub = c1 - c0
                    vector.wait_ge(s_ev1, it + 1)
                    for half, slot in ((0, 2 * q), (1, 2 * q + 1)):
                        yc = y1[:, ycol(slot, c0) :
                                ycol(slot, c0) + nsub * CHUNK]
                        nc.vector.scalar_tensor_tensor(
                            out=scr[:, 0 : nsub * CHUNK], in0=yc,
                            scalar=1.0, in1=yc,
                            op0=ALU.mult, op1=ALU.mult,
                            accum_out=pq1[:, 2 * it + half :
                                          2 * it + half + 1])
                    it += 1

            def stats(pstats_s, pstats_q, st, dsem_in, acst_v, statst, g,
                      which):
                nc.vector.drain()
                nc.vector.reduce_sum(statst[:, 0:1], pstats_s[:],
                                     axis=mybir.AxisListType.X)
                nc.vector.reduce_sum(statst[:, 1:2], pstats_q[:],
                                     axis=mybir.AxisListType.X)
                nc.vector.drain().then_inc(st, 1)
                vector.wait_ge(dsem, dsem_in)
                # halves side by side: fold on-partition, then bn math
                nc.vector.tensor_tensor(out=g[:, 4:6], in0=g[:, 0:2],
                                        in1=g[:, 2:4], op=ALU.add)
                nc.vector.drain()
                nc.vector.tensor_scalar_mul(g[:, 0:1], g[:, 4:5], 1.0 / N_TOT)
                nc.vector.tensor_scalar_mul(g[:, 1:2], g[:, 5:6], 1.0 / N_TOT)
                nc.vector.drain()
                nc.vector.tensor_tensor(out=g[:, 2:3], in0=g[:, 0:1],
                                        in1=g[:, 0:1], op=ALU.mult)
                nc.vector.drain()
                nc.vector.tensor_tensor(out=g[:, 3:4], in0=g[:, 1:2],
                                        in1=g[:, 2:3], op=ALU.subtract)
                nc.vector.drain().then_inc(st, 1)
                # ACT: g[:,2:3] = sqrt(g[:,3:4] + eps)
                vector.wait_ge(s_acst, acst_v)
                gcol, bcol = 2 * which, 2 * which + 1
                nc.vector.reciprocal(g[:, 3:4], g[:, 2:3])
                nc.vector.drain()
                nc.vector.tensor_tensor(out=g[:, 6:7], in0=g[:, 3:4],
                                        in1=consts[:, gcol : gcol + 1],
                                        op=ALU.mult)
                nc.vector.drain()
                nc.vector.tensor_tensor(out=g[:, 4:5], in0=g[:, 0:1],
                                        in1=g[:, 6:7], op=ALU.mult)
                nc.vector.drain()
                nc.vector.tensor_tensor(out=g[:, 7:8],
                                        in0=consts[:, bcol : bcol + 1],
                                        in1=g[:, 4:5], op=ALU.subtract)
                nc.vector.drain().then_inc(st, 1)

            stats(ps1, pq1, s_st1, D_G1, 1, stats1, glob1, 0)

            # conv2: DVE evacs pY (slot 2q+1) + both sumsq partials
            it = 0
            for q in range(QG):
                for (c0, c1) in SUPERS:
                    nsub = c1 - c0
                    vector.wait_ge(s_pe2, it + 1)
                    pY = pbY[it % 2]
                    src = pY[:, 0 : nsub * 512].rearrange(
                        "p (s k) -> p s k", s=nsub)[:, :, 0:CHUNK]
                    nc.vector.tensor_scalar(
                        y2v[:, ycol(2 * q + 1, c0) :
                            ycol(2 * q + 1, c0) + nsub * CHUNK],
                        src, 1.0, 0.0, ALU.mult, ALU.add,
                        accum_out=ps2[:, 2 * it + 1 : 2 * it + 2])
                    nc.vector.drain()
                    vector.wait_ge(s_ev2, it + 1)
                    for half, slot in ((0, 2 * q), (1, 2 * q + 1)):
                        yc = y2v[:, ycol(slot, c0) :
                                 ycol(slot, c0) + nsub * CHUNK]
                        nc.vector.scalar_tensor_tensor(
                            out=scr16[:, 0 : nsub * CHUNK], in0=yc,
                            scalar=1.0, in1=yc,
                            op0=ALU.mult, op1=ALU.mult,
                            accum_out=pq2[:, 2 * it + half :
                                          2 * it + half + 1])
                    nc.vector.drain().then_inc(s_dv2, 1)
                    it += 1

            stats(ps2, pq2, s_st2, D_G2, 2, stats2, glob2, 1)

            # final w = y2 * a2 + tq
            vector.wait_ge(dsem, D_TQ)
            for j in range(NF - 1):
                sl, h = FINALS[j]
                off = sl * PERIMG + h * HCOLS
                if j >= 3:
                    vector.wait_ge(s_fs, j - 2)
                nc.vector.scalar_tensor_tensor(
                    out=wbuf[j % 3][:, 0:HCOLS],
                    in0=y2v[:, off : off + HCOLS],
                    scalar=glob2[:, 6:7],
                    in1=tq[:, off : off + HCOLS],
                    op0=ALU.mult, op1=ALU.add)
                nc.vector.drain().then_inc(s_fvd, 1)

        @block.gpsimd
        def _(gpsimd):
            nc.gpsimd.memset(dscr[:], 0).then_inc(s_ms, 1)
            for s in range(SLOTS):
                nc.gpsimd.memset(bin1[:, s], 0).then_inc(s_ms, 1)
            gpsimd.wait_ge(dsem, D_B1ST)
            if CC_STUB:
                nc.gpsimd.dma_start(db1_out[:], db1_in[:]).then_inc(s_cc, 16)
            else:
                nc.gpsimd.collective_compute(
                    "AllReduce", ALU.add, replica_groups=[list(range(N_CORES))],
                    ins=[db1_in[:]], outs=[db1_out[:]]).then_inc(s_cc, 1)
            gpsimd.wait_ge(dsem, D_B2ST)
            if CC_STUB:
                nc.gpsimd.dma_start(db2_out[:], db2_in[:]).then_inc(s_cc, 16)
            else:
                nc.gpsimd.collective_compute(
                    "AllReduce", ALU.add, replica_groups=[list(range(N_CORES))],
                    ins=[db2_in[:]], outs=[db2_out[:]]).then_inc(s_cc, 1)
            # final w for the last chunk: h = y2*a2, then + tq
            gpsimd.wait_ge(s_st2, 3)
            _sl, _h = FINALS[NF - 1]
            _off = _sl * PERIMG + _h * HCOLS
            nc.gpsimd.tensor_tensor(
                out=hbuf[:, 0:HCOLS], in0=y2v[:, _off : _off + HCOLS],
                in1=glob2[:, 6:7].broadcast_to((128, HCOLS)), op=ALU.mult)
            gpsimd.drain()
            nc.gpsimd.tensor_tensor(
                out=pbuf[:, 0:HCOLS], in0=hbuf[:, 0:HCOLS],
                in1=tq[:, _off : _off + HCOLS], op=ALU.add)
            gpsimd.drain().then_inc(s_fvp, 1)

    return nc


_CACHE = {}


def _get_nc():
    if "nc" not in _CACHE:
        _CACHE["nc"] = build_bass()
    return _CACHE["nc"]


def kernel(x, w1, gamma1, beta1, w2, gamma2, beta2):
    x = np.asarray(x, np.float32)
    w1 = np.asarray(w1, np.float32)
    w2 = np.asarray(w2, np.float32)
    gamma1 = np.asarray(gamma1, np.float32)
    beta1 = np.asarray(beta1, np.float32)
    gamma2 = np.asarray(gamma2, np.float32)
    beta2 = np.asarray(beta2, np.float32)

    f8np = mybir.dt.np(F8)

    # conv1 weights: [tap, cin, cout] -> [cin, tap*cout], rows duplicated
    wb1 = np.where(w1 >= 0, 1.0, -1.0).astype(np.float32)
    wt1a = wb1.transpose(1, 2, 3, 0).reshape(64, 9, 64)
    wt1 = wt1a.reshape(64, 576)
    wf16_np = np.concatenate([wt1, wt1], axis=0).astype(np.float16)
    # fp8 limb weights: block-diagonal +-2^-12 (exact e4m3 subnormal)
    w1m_np = np.zeros((128, 1280), np.float32)
    for t in range(9):
        w1m_np[0:64, t * 128 : t * 128 + 64] = wt1a[:, t, :] * 2.0 ** -12
        w1m_np[64:128, t * 128 + 64 : t * 128 + 128] = wt1a[:, t, :] * 2.0 ** -12

    # conv2 weights: fp8 block-diagonal, 10 taps (tap 9 zero)
    wb2 = np.where(w2 >= 0, 1.0, -1.0).astype(np.float32)
    wt2 = wb2.transpose(1, 2, 3, 0).reshape(64, 9, 64)  # [cin, tap, cout]
    w8_np = np.zeros((128, 1280), np.float32)
    for t in range(9):
        w8_np[0:64, t * 128 : t * 128 + 64] = wt2[:, t, :]
        w8_np[64:128, t * 128 + 64 : t * 128 + 128] = wt2[:, t, :]
    w8_np = w8_np.astype(f8np)
    w1m_np = w1m_np.astype(mybir.dt.np(F8E5))

    consts_np = np.zeros((128, 8), np.float32)
    for col, v in enumerate([gamma1, beta1, gamma2, beta2]):
        consts_np[0:64, col] = v
        consts_np[64:128, col] = v
    consts_np[:, 4] = EPS

    in_maps = []
    for k in range(N_CORES):
        xc = x[IMGS * k : IMGS * (k + 1)]            # [8, 64, 56, 56]
        xp = np.zeros((IMGS, C, HP, HP), np.float32)
        xp[:, :, 1 : 1 + H, 1 : 1 + W] = xc
        arr = xp.reshape(SLOTS, 2, C, HP, HP).transpose(1, 2, 0, 3, 4)
        arr = np.ascontiguousarray(arr).reshape(128, SLOTS, HP, HP)
        ahi = arr.astype(np.float16)
        r1 = arr - ahi.astype(np.float32)
        m8 = (r1 * 4096.0).astype(f8np)
        r2 = r1 - m8.astype(np.float32) / 4096.0
        l8 = (r2 * 4096.0).astype(f8np)
        # Q-pack the fp8 limbs so the block-diagonal limb matmuls land in the
        # same (permuted) arrangement the hi quadrant pass produces
        m8q = np.empty_like(m8)
        l8q = np.empty_like(l8)
        for s in range(SLOTS):
            for h in (0, 1):
                img = 4 * (s // 2) + (s % 2) + 2 * h
                sp, hp_ = img // 2, img % 2
                m8q[h * 64 : h * 64 + 64, s] = m8[hp_ * 64 : hp_ * 64 + 64, sp]
                l8q[h * 64 : h * 64 + 64, s] = l8[hp_ * 64 : hp_ * 64 + 64, sp]
        # conv1's quadrant pattern permutes (slot, half): y slot s half h holds
        # image Q(s,h) = 4*(s//2) + s%2 + 2*h. The final residual add needs x
        # in that same arrangement.
        tq_np = np.empty((128, SLOTS, PERIMG), np.float32)
        for s in range(SLOTS):
            for h in (0, 1):
                img = 4 * (s // 2) + (s % 2) + 2 * h
                tq_np[h * 64 : h * 64 + 64, s] = xc[img].reshape(C, PERIMG)
        in_maps.append({
            "xhi": ahi, "xm8": m8q, "xl8": l8q, "wf16": wf16_np,
            "w8": w8_np, "w1m": w1m_np, "consts": consts_np,
            "tq": tq_np.reshape(128, YCOLS),
        })

    nc = _get_nc()
    res = bass_utils.run_bass_kernel_spmd(nc, in_maps, core_ids=list(range(N_CORES)))

    out = np.empty((N, C, H, W), np.float32)
    for k in range(N_CORES):
        o = np.asarray(res.results[k]["outp"]).astype(np.float32)  # [128, 12544]
        o = o.reshape(2, C, SLOTS, H, W)  # [half, ch, slot, H, W]
        for s in range(SLOTS):
            for h in (0, 1):
                img = 4 * (s // 2) + (s % 2) + 2 * h
                out[IMGS * k + img] = o[h, :, s]
    return out


if __name__ == "__main__":
    rng = np.random.default_rng(0)
    xs = rng.standard_normal((N, C, H, W)).astype(np.float32)
    w1s = (rng.standard_normal((C, C, 3, 3)) * 0.1).astype(np.float32)
    w2s = (rng.standard_normal((C, C, 3, 3)) * 0.1).astype(np.float32)
    ones = np.ones(C, np.float32)
    zeros = np.zeros(C, np.float32)
    r = kernel(x=xs, w1=w1s, gamma1=ones, beta1=zeros, w2=w2s, gamma2=ones,
               beta2=zeros)
    print("ran, out uniq:", np.unique(r))


# revision 19
# speedup vs baseline: 1.2431x; 1.0054x over previous
"""BinaryBasicBlock TRN2 kernel: 8-core batch-parallel, raw Bass.

Reference computation (per core: 8 images, C=64, 56x56):
  y1   = conv3x3(x, sign(w1))            # exact: x = fp16(x) + fp16(residual)
  bin1 = sign((y1 - mu1) * rsqrt(var1+eps) * g1 + b1)   # global batch stats
  y2   = conv3x3(bin1, sign(w2))         # exact (+-1 x +-1 in fp8)
  out  = sign((y2 - mu2) * rsqrt(var2+eps) * g2 + b2 + x)

Batch stats are exact: per-core (sum, sumsq) partials are AllReduced across
the 8 cores mid-kernel (both partition halves stored side by side as [64,4]
so no on-chip cross-partition fold is needed).

Layout: channels on partitions, 2 images per 128 partitions (top/bottom
halves), 4 "slots" of [128, 58, 58] padded images per core.

conv1 runs as 9-tap f16 matmul accumulation with all four 64x64 PE quadrants
streaming four different images concurrently (hi + lo pass for fp32
exactness). conv2 runs on fp8 inputs (bin1 is +-1, exact in e4m3) as
full-128 block-diagonal DoubleRow matmuls: weights hold both partition
halves' 64x64 blocks on the diagonal and each matmul processes a pair of
taps, so 5 matmuls replace 9 per (slot, subchunk).

The PE p-state ramp is kept hot across the conv1->conv2 stats barrier by a
stream of filler matmuls into a scratch PSUM region (the tensor engine
down-clocks after idling, which would slow conv2's first ~3us).

conv2 PSUM evacuation is split: ACT evacuates slot 2q (with sum
accumulation), DVE evacuates slot 2q+1 (tensor_scalar with accum), so the
fp8 conv2 is not ACT-bound. y2 is exact in f16 (integer-valued, |y2|<=576).

Final stage: t = xhi+xlo (f32, precomputed during conv1), then per chunk
one DVE/Pool scalar_tensor_tensor (w = y2*a2 + t) and one ACT Sign with
per-channel bias, written as fp8 (+-1 exact) and stored per slot.

Toolchain constraints honored: raw Bass only, max one semaphore wait per
instruction, single PSUM reader engine per bank, drain-backed semaphore
increments on every cross-engine RAW edge, explicit DVE drains between
dependent vector ops. DoubleRow matmuls keep dst partition base 0 (ISA
constraint s3d3_mm_valid_dst_partition).
"""
import numpy as np
import ml_dtypes
import concourse.bass as bass
import concourse.mybir as mybir
from concourse import bass_utils
from concourse.ap import AP as APcls
from contextlib import ExitStack

F32 = mybir.dt.float32
BF16 = mybir.dt.bfloat16
F16 = mybir.dt.float16
F8 = mybir.dt.float8e4
F8E5 = mybir.dt.float8e5
AF = mybir.ActivationFunctionType
ALU = mybir.AluOpType
DRM = mybir.MatmulPerfMode.DoubleRow

N_CORES = 8
N, C, H, W = 64, 64, 56, 56
IMGS = N // N_CORES          # 8 images per core
SLOTS = IMGS // 2            # 4 slots (2 images per slot)
QG = SLOTS // 2              # 2 quadgroups (4 images each)
HP = H + 2                   # 58 padded
CHROWS = 8                   # output rows per 448-subchunk
CHUNK = CHROWS * W           # 448
NCH = H // CHROWS            # 7 subchunks per image
SUPERS = [(0, 2), (2, 4), (4, 6), (6, 7)]   # subchunk ranges per super-iter
NSUP = len(SUPERS)           # 4 super-iters per quadgroup
ITERS = QG * NSUP            # 8 super-iters per conv
PERIMG = H * W               # 3136
YCOLS = SLOTS * PERIMG       # 12544
SLOTPIX = HP * HP            # 3364
N_TOT = float(N * H * W)     # global batch-stat count
EPS = 1e-5
NF = SLOTS * 2               # 8 final-stage chunks (half-slots of 1568)
PAIRS = [(0, 1), (2, 3), (4, 5), (6, 7), (8, 9)]  # conv2 tap pairs (9=zero)
POOL_J = ()   # final iterations handled by GPSIMD (stt not Pool-legal)

W0_DUMMIES = 30              # PE warmup fillers (initial load latency)
W1_DUMMIES = 102             # PE fillers across the stats1 barrier

DEBUG = False
CC_STUB = False   # replace AllReduce with a local DMA (for TimelineSim)

# row chunks per slot for the staged input loads
ROWCH = [(0, 18), (18, 34), (34, 50), (50, 58)]


def build_bass():
    nc = bass.Bass(trn_type="TRN2", target_bir_lowering=False, debug=False,
                   num_devices=N_CORES)

    d_xhi = nc.dram_tensor("xhi", [128, SLOTS, HP, HP], F16, kind="ExternalInput")
    d_xm8 = nc.dram_tensor("xm8", [128, SLOTS, HP, HP], F8, kind="ExternalInput")
    d_xl8 = nc.dram_tensor("xl8", [128, SLOTS, HP, HP], F8, kind="ExternalInput")
    d_wf16 = nc.dram_tensor("wf16", [128, 576], F16, kind="ExternalInput")
    d_w8 = nc.dram_tensor("w8", [128, 1280], F8, kind="ExternalInput")
    d_w1m = nc.dram_tensor("w1m", [128, 1280], F8E5, kind="ExternalInput")
    d_consts = nc.dram_tensor("consts", [128, 8], F32, kind="ExternalInput")
    d_tq = nc.dram_tensor("tq", [128, YCOLS], F32, kind="ExternalInput")
    d_out = nc.dram_tensor("outp", [128, YCOLS], F8, kind="ExternalOutput")
    db1_in = nc.dram_tensor("db1_in", [64, 4], F32)
    db1_out = nc.dram_tensor("db1_out", [64, 4], F32, addr_space="Shared")
    db2_in = nc.dram_tensor("db2_in", [64, 4], F32)
    db2_out = nc.dram_tensor("db2_out", [64, 4], F32, addr_space="Shared")

    es = ExitStack()
    def sb(name, shape, dt):
        return es.enter_context(nc.sbuf_tensor(name, shape, dt))
    def ps(name, shape, dt):
        return es.enter_context(nc.psum_tensor(name, shape, dt))
    def sem(name):
        return es.enter_context(nc.semaphore(name))

    xhi = sb("xhi_t", [128, SLOTS, HP, HP], F16)
    xm8 = sb("xm8_t", [128, SLOTS, HP, HP], F8)
    xl8 = sb("xl8_t", [128, SLOTS, HP, HP], F8)
    wf16 = sb("wf16_t", [128, 576], F16)
    w8 = sb("w8_t", [128, 1280], F8)
    w1m = sb("w1m_t", [128, 1280], F8E5)
    consts = sb("consts_t", [128, 8], F32)
    bin1 = sb("bin1_t", [128, SLOTS, HP, HP], F8)
    tq = sb("tq_t", [128, YCOLS], F32)
    y1 = sb("y1_t", [128, YCOLS], F32)
    # y2 (f16) and the fp8 output live in y1's bytes (dead regions by then):
    #   y2v   = f16 cols 0..12543     (y1 f32 cols 0..6271   = slots 0,1)
    #   outv  = f8 cols 25088..37631  (y1 f32 cols 6272..9407 = slots 2,3lo)
    y2v = y1[:].bitcast(F16)
    outv = y1[:].bitcast(F8)
    OUTOFF = 2 * YCOLS
    NPART = 2 * ITERS            # partial columns per conv
    ps1 = sb("ps1", [128, NPART], F32)
    pq1 = sb("pq1", [128, NPART], F32)
    ps2 = sb("ps2", [128, NPART], F32)
    pq2 = sb("pq2", [128, NPART], F32)
    stats1 = sb("stats1", [128, 2], F32)
    stats2 = sb("stats2", [128, 2], F32)
    glob1 = sb("glob1", [128, 8], F32)
    glob2 = sb("glob2", [128, 8], F32)
    scr = sb("scr", [128, 2 * CHUNK], F32)
    scr16 = scr[:].bitcast(F16)
    wbuf = [sb(f"wb{i}", [128, PERIMG // 2], F32) for i in range(3)]
    pbuf = sb("pbuf", [128, PERIMG // 2], F32)
    hbuf = sb("hbuf", [128, PERIMG // 2], F32)
    dscr = sb("dscr", [128, 512], F16)
    pbX = [ps(f"pbX{i}", [128, 1024], F32) for i in range(2)]
    pbY = [ps(f"pbY{i}", [128, 1024], F32) for i in range(2)]

    dsem = sem("dsem")
    s_pe1 = sem("s_pe1"); s_ev1 = sem("s_ev1")
    s_pe2 = sem("s_pe2"); s_ev2 = sem("s_ev2"); s_dv2 = sem("s_dv2")
    s_sg1 = sem("s_sg1"); s_ms = sem("s_ms")
    s_st1 = sem("s_st1"); s_st2 = sem("s_st2"); s_acst = sem("s_acst")
    s_cc = sem("s_cc")
    s_fvd = sem("s_fvd"); s_fvp = sem("s_fvp"); s_fs = sem("s_fs")

    CCV = 16 if CC_STUB else 1

    def ycol(slot, c):
        return slot * PERIMG + c * CHUNK

    HCOLS = PERIMG // 2          # 1568
    FINALS = [(s, h) for s in range(SLOTS) for h in (0, 1)]

    # ---- input load schedule --------------------------------------------
    # list of (sbuf_dst_ap_fn, dram_src_ap_fn); dsem marks derived from index
    loads = []
    def add_load(dst, src):
        loads.append((dst, src))
        return len(loads)  # 1-based count

    add_load(wf16[:], d_wf16[:])
    add_load(w1m[:], d_w1m[:])
    for t, dt_ in ((xhi, d_xhi), (xm8, d_xm8), (xl8, d_xl8)):
        for s in (0, 1):
            add_load(t[:, s, 0:18], dt_[:, s, 0:18])
    D_S0 = len(loads) * 16
    add_load(w8[:], d_w8[:])
    add_load(consts[:], d_consts[:])
    for (r0, r1) in ROWCH[1:]:
        for t, dt_ in ((xhi, d_xhi), (xm8, d_xm8), (xl8, d_xl8)):
            for s in (0, 1):
                add_load(t[:, s, r0:r1], dt_[:, s, r0:r1])
        if r1 == 34:
            D_S1 = len(loads) * 16
        elif r1 == 50:
            D_S2 = len(loads) * 16
        else:
            D_S3 = len(loads) * 16
    for s in (2, 3):
        for t, dt_ in ((xhi, d_xhi), (xm8, d_xm8), (xl8, d_xl8)):
            add_load(t[:, s], dt_[:, s])
    D_QG1 = len(loads) * 16
    for s in range(SLOTS):
        add_load(tq[:, s * PERIMG : (s + 1) * PERIMG],
                 d_tq[:, s * PERIMG : (s + 1) * PERIMG])
    D_TQ = len(loads) * 16
    NLOADS = len(loads)
    D_B1ST = (NLOADS + 2) * 16
    D_G1 = (NLOADS + 4) * 16
    D_B2ST = (NLOADS + 6) * 16
    D_G2 = (NLOADS + 8) * 16

    CONV1_GATES = {(0, 0): D_S0, (0, 1): D_S1, (0, 2): D_S2, (0, 3): D_S3,
                   (1, 0): D_QG1}

    with nc.Block() as block:

        @block.sync
        def _(sync):
            for dst, src in loads:
                sync.dma_start(dst, src).then_inc(dsem, 16)
            # stats chains: store half 2 / load half 2 ride on ACT and Pool
            sync.wait_ge(s_st1, 1)
            sync.dma_start(db1_in[:, 0:2], stats1[0:64, 0:2]).then_inc(dsem, 16)
            sync.wait_ge(s_cc, CCV)
            sync.dma_start(glob1[0:64, 0:4], db1_out[:]).then_inc(dsem, 16)
            sync.wait_ge(s_st2, 1)
            sync.dma_start(db2_in[:, 0:2], stats2[0:64, 0:2]).then_inc(dsem, 16)
            sync.wait_ge(s_cc, 2 * CCV)
            sync.dma_start(glob2[0:64, 0:4], db2_out[:]).then_inc(dsem, 16)
            # output stores, one per final chunk
            for k in range(NF):
                sl, h = FINALS[k]
                off = sl * PERIMG + h * HCOLS
                sync.wait_ge(s_fs, k + 1)
                sync.dma_start(
                    d_out[:, off : off + HCOLS],
                    outv[:, OUTOFF + off : OUTOFF + off + HCOLS],
                ).then_inc(dsem, 16)

        @block.tensor
        def _(tensor):
            def dummy64(n):
                # tiny fillers into never-read psum columns (960:1024)
                nc.tensor.ldweights(dscr[:, 0:64], tile_position=(0, 0))
                for i in range(n):
                    nc.tensor.matmul(pbX[0][0:64, 960:1024], dscr[:, 0:64],
                                     dscr[:, 64:128], start=True, stop=True,
                                     tile_position=(0, 0),
                                     skip_group_check=True)

            def dummy(n):
                # keep the PE p-state hot: harmless f16 matmuls into a
                # region of pbX[0] that is dead at every dummy site
                for i in range(n):
                    nc.tensor.ldweights(dscr[:, 0:64], tile_position=(0, 0))
                    nc.tensor.matmul(pbX[0][0:64, 0:448], dscr[:, 0:64],
                                     dscr[:, 64:512], start=True, stop=True,
                                     tile_position=(0, 0),
                                     skip_group_check=True)

            tensor.wait_ge(s_ms, 1)
            dummy(W0_DUMMIES)

            # conv1: f16 hi/lo, 4 quadrants (4 images concurrent on HW)
            it = 0
            for q in range(QG):
                for si, (c0, c1) in enumerate(SUPERS):
                    gate = CONV1_GATES.get((q, si))
                    if gate is not None:
                        tensor.wait_ge(dsem, gate)
                    nsub = c1 - c0
                    if it >= 2:
                        tensor.wait_ge(s_ev1, it - 1)
                    pX = pbX[it % 2]
                    pY = pbY[it % 2]
                    quads = [
                        ((0, 0), slice(0, 64), 2 * q, pX, slice(0, 64)),
                        ((64, 0), slice(64, 128), 2 * q, pY, slice(0, 64)),
                        ((0, 64), slice(0, 64), 2 * q + 1, pX, slice(64, 128)),
                        ((64, 64), slice(64, 128), 2 * q + 1, pY,
                         slice(64, 128)),
                    ]
                    for tap in range(9):
                        kh, kw = tap // 3, tap % 3
                        wcol = tap * 64
                        for tp, rows, _, _, _ in quads:
                            nc.tensor.ldweights(wf16[rows, wcol : wcol + 64],
                                                tile_position=tp)
                        for tp, rows, dslot, pdst, phalf in quads:
                            for s in range(nsub):
                                c = c0 + s
                                rap = xhi[rows, dslot,
                                          c * CHROWS + kh :
                                          c * CHROWS + kh + CHROWS,
                                          kw : kw + W]
                                nc.tensor.matmul(
                                    pdst[phalf, s * 512 : s * 512 + CHUNK],
                                    wf16[rows, wcol : wcol + 64], rap,
                                    start=(tap == 0), stop=False,
                                    tile_position=tp,
                                    skip_group_check=True)
                    # fp8 mid/lo limbs: block-diagonal DoubleRow, weights
                    # are +-2^-12 so they accumulate into the same groups
                    for li, lt in enumerate((xm8, xl8)):
                        for ip, (ta, tb) in enumerate(PAIRS):
                            kha, kwa = ta // 3, ta % 3
                            if tb == 9:
                                delta = -HP
                            else:
                                delta = (tb // 3 - kha) * HP + (tb % 3 - kwa)
                            wap = APcls(tensor=w1m[:].tensor, offset=ta * 128,
                                        ap=[[1280, 128], [128, 2], [1, 128]])
                            nc.tensor.ldweights(wap, perf_mode=DRM)
                            for sj in range(2):
                                slot = 2 * q + sj
                                pdst = pX if sj == 0 else pY
                                for s in range(nsub):
                                    c = c0 + s
                                    offa = (slot * SLOTPIX
                                            + (c * CHROWS + kha) * HP + kwa)
                                    rap = APcls(
                                        tensor=lt[:].tensor, offset=offa,
                                        ap=[[SLOTS * SLOTPIX, 128], [delta, 2],
                                            [HP, CHROWS], [1, W]])
                                    nc.tensor.matmul(
                                        pdst[:, s * 512 : s * 512 + CHUNK],
                                        wap, rap, start=False,
                                        stop=(li == 1 and ip == 4),
                                        perf_mode=DRM, skip_group_check=True)
                    tensor.drain().then_inc(s_pe1, 1)
                    it += 1

            # fill the stats1 -> bin1 barrier (evac of it=14 must be done
            # before reusing pbX[0]; evac15 targets pbX[1]/pbY[1])
            tensor.wait_ge(s_ev1, ITERS - 1)
            dummy(W1_DUMMIES)

            # conv2: fp8 block-diagonal DoubleRow, 5 tap-pairs
            it = 0
            for q in range(QG):
                tensor.wait_ge(s_sg1, 2 if q == 0 else 4)
                for si, (c0, c1) in enumerate(SUPERS):
                    nsub = c1 - c0
                    nd64 = {2: 185}.get(it, 0)
                    if nd64:
                        dummy64(nd64)
                    if it >= 2:
                        tensor.wait_ge(s_ev2, it - 1)
                        tensor.wait_ge(s_dv2, it - 1)
                    pX = pbX[it % 2]
                    pY = pbY[it % 2]
                    for ip, (ta, tb) in enumerate(PAIRS):
                        kha, kwa = ta // 3, ta % 3
                        if tb == 9:
                            delta = -58  # zero weights; any in-bounds window
                        else:
                            delta = (tb // 3 - kha) * HP + (tb % 3 - kwa)
                        wap = APcls(tensor=w8[:].tensor, offset=ta * 128,
                                    ap=[[1280, 128], [128, 2], [1, 128]])
                        nc.tensor.ldweights(wap, perf_mode=DRM)
                        for sj in range(2):
                            slot = 2 * q + sj
                            pdst = pX if sj == 0 else pY
                            for s in range(nsub):
                                c = c0 + s
                                offa = (slot * SLOTPIX
                                        + (c * CHROWS + kha) * HP + kwa)
                                rap = APcls(
                                    tensor=bin1[:].tensor, offset=offa,
                                    ap=[[SLOTS * SLOTPIX, 128], [delta, 2],
                                        [HP, CHROWS], [1, W]])
                                nc.tensor.matmul(
                                    pdst[:, s * 512 : s * 512 + CHUNK],
                                    wap, rap, start=(ip == 0), stop=(ip == 4),
                                    perf_mode=DRM, skip_group_check=True)
                    tensor.drain().then_inc(s_pe2, 1)
                    it += 1

        @block.scalar
        def _(scalar):
            # conv1 evacs: PSUM -> y1 (f32) with sum accumulation
            it = 0
            for q in range(QG):
                for (c0, c1) in SUPERS:
                    nsub = c1 - c0
                    scalar.wait_ge(s_pe1, it + 1)
                    pX = pbX[it % 2]
                    pY = pbY[it % 2]
                    for half, slot, pt in ((0, 2 * q, pX), (1, 2 * q + 1, pY)):
                        src = pt[:, 0 : nsub * 512].rearrange(
                            "p (s k) -> p s k", s=nsub)[:, :, 0:CHUNK]
                        nc.scalar.activation(
                            y1[:, ycol(slot, c0) :
                               ycol(slot, c0) + nsub * CHUNK],
                            src, AF.Copy,
                            accum_out=ps1[:, 2 * it + half :
                                          2 * it + half + 1])
                    scalar.drain().then_inc(s_ev1, 1)
                    it += 1
            # stats1: store the bottom half's partials, then sqrt(var+eps)
            scalar.wait_ge(s_st1, 1)
            nc.scalar.dma_start(db1_in[:, 2:4],
                                stats1[64:128, 0:2]).then_inc(dsem, 16)
            scalar.wait_ge(s_cc, CCV)
            nc.scalar.dma_start(glob1[64:128, 0:4], db1_out[:]).then_inc(dsem, 16)
            scalar.wait_ge(s_st1, 2)
            nc.scalar.activation(glob1[:, 2:3], glob1[:, 3:4], AF.Sqrt,
                                 bias=consts[:, 4:5])
            scalar.drain().then_inc(s_acst, 1)
            # bin1 = Sign(y1 * a1 + b1) into padded fp8 slots
            scalar.wait_ge(s_ms, 5)
            scalar.wait_ge(s_st1, 3)
            def sign1(s):
                nc.scalar.activation(
                    bin1[:, s, 1 : 1 + H, 1 : 1 + W],
                    y1[:, s * PERIMG : (s + 1) * PERIMG],
                    AF.Sign, bias=glob1[:, 7:8], scale=glob1[:, 6:7])
                scalar.drain().then_inc(s_sg1, 1)
            sign1(0)
            sign1(1)
            sign1(2)
            sign1(3)

            # conv2 evacs of pX (slot 2q) with accum; slots 2,3 signs woven in
            def evac2(itv, c0, nsub, q):
                scalar.wait_ge(s_pe2, itv + 1)
                pX = pbX[itv % 2]
                src = pX[:, 0 : nsub * 512].rearrange(
                    "p (s k) -> p s k", s=nsub)[:, :, 0:CHUNK]
                nc.scalar.activation(
                    y2v[:, ycol(2 * q, c0) : ycol(2 * q, c0) + nsub * CHUNK],
                    src, AF.Copy,
                    accum_out=ps2[:, 2 * itv : 2 * itv + 1])
                scalar.drain().then_inc(s_ev2, 1)

            it = 0
            for q in range(QG):
                for si, (c0, c1) in enumerate(SUPERS):
                    evac2(it, c0, c1 - c0, q)
                    it += 1
            # stats2: bottom-half store, then sqrt
            scalar.wait_ge(s_st2, 1)
            nc.scalar.dma_start(db2_in[:, 2:4],
                                stats2[64:128, 0:2]).then_inc(dsem, 16)
            scalar.wait_ge(s_cc, 2 * CCV)
            nc.scalar.dma_start(glob2[64:128, 0:4], db2_out[:]).then_inc(dsem, 16)
            scalar.wait_ge(s_st2, 2)
            nc.scalar.activation(glob2[:, 2:3], glob2[:, 3:4], AF.Sqrt,
                                 bias=consts[:, 4:5])
            scalar.drain().then_inc(s_acst, 2)
            # final: out = Sign(w + b2'), w produced by DVE/Pool
            for j in range(NF):
                sl, h = FINALS[j]
                off = sl * PERIMG + h * HCOLS
                if j == 5:
                    scalar.wait_ge(s_fvp, 1)
                    srcb = pbuf[:, 0:HCOLS]
                else:
                    scalar.wait_ge(s_fvd, j + 1 if j < 5 else j)
                    srcb = wbuf[j % 3][:, 0:HCOLS]
                nc.scalar.activation(
                    outv[:, OUTOFF + off : OUTOFF + off + HCOLS],
                    srcb, AF.Sign, bias=glob2[:, 7:8])
                scalar.drain().then_inc(s_fs, 1)

        @block.vector
        def _(vector):
            # conv1 sumsq partials
            it = 0
            for q in range(QG):
                for (c0, c1) in SUPERS:
                    nsub = c1 - c0
                    vector.wait_ge(s_ev1, it + 1)
                    for half, slot in ((0, 2 * q), (1, 2 * q + 1)):
                        yc = y1[:, ycol(slot, c0) :
                                ycol(slot, c0) + nsub * CHUNK]
                        nc.vector.scalar_tensor_tensor(
                            out=scr[:, 0 : nsub * CHUNK], in0=yc,
                            scalar=1.0, in1=yc,
                            op0=ALU.mult, op1=ALU.mult,
                            accum_out=pq1[:, 2 * it + half :
                                          2 * it + half + 1])
                    it += 1

            def stats(pstats_s, pstats_q, st, dsem_in, acst_v, statst, g,
                      which):
                nc.vector.drain()
                nc.vector.reduce_sum(statst[:, 0:1], pstats_s[:],
                                     axis=mybir.AxisListType.X)
                nc.vector.reduce_sum(statst[:, 1:2], pstats_q[:],
                                     axis=mybir.AxisListType.X)
                nc.vector.drain().then_inc(st, 1)
                vector.wait_ge(dsem, dsem_in)
                # halves side by side: fold on-partition, then bn math
                nc.vector.tensor_tensor(out=g[:, 4:6], in0=g[:, 0:2],
                                        in1=g[:, 2:4], op=ALU.add)
                nc.vector.drain()
                nc.vector.tensor_scalar_mul(g[:, 0:1], g[:, 4:5], 1.0 / N_TOT)
                nc.vector.tensor_scalar_mul(g[:, 1:2], g[:, 5:6], 1.0 / N_TOT)
                nc.vector.drain()
                nc.vector.tensor_tensor(out=g[:, 2:3], in0=g[:, 0:1],
                                        in1=g[:, 0:1], op=ALU.mult)
                nc.vector.drain()
                nc.vector.tensor_tensor(out=g[:, 3:4], in0=g[:, 1:2],
                                        in1=g[:, 2:3], op=ALU.subtract)
                nc.vector.drain().then_inc(st, 1)
                # ACT: g[:,2:3] = sqrt(g[:,3:4] + eps)
                vector.wait_ge(s_acst, acst_v)
                gcol, bcol = 2 * which, 2 * which + 1
                nc.vector.reciprocal(g[:, 3:4], g[:, 2:3])
                nc.vector.drain()
                nc.vector.tensor_tensor(out=g[:, 6:7], in0=g[:, 3:4],
                                        in1=consts[:, gcol : gcol + 1],
                                        op=ALU.mult)
                nc.vector.drain()
                nc.vector.tensor_tensor(out=g[:, 4:5], in0=g[:, 0:1],
                                        in1=g[:, 6:7], op=ALU.mult)
                nc.vector.drain()
                nc.vector.tensor_tensor(out=g[:, 7:8],
                                        in0=consts[:, bcol : bcol + 1],
                                        in1=g[:, 4:5], op=ALU.subtract)
                nc.vector.drain().then_inc(st, 1)

            stats(ps1, pq1, s_st1, D_G1, 1, stats1, glob1, 0)

            # conv2: DVE evacs pY (slot 2q+1) + both sumsq partials
            it = 0
            for q in range(QG):
                for (c0, c1) in SUPERS:
                    nsub = c1 - c0
                    vector.wait_ge(s_pe2, it + 1)
                    pY = pbY[it % 2]
                    src = pY[:, 0 : nsub * 512].rearrange(
                        "p (s k) -> p s k", s=nsub)[:, :, 0:CHUNK]
                    nc.vector.tensor_scalar(
                        y2v[:, ycol(2 * q + 1, c0) :
                            ycol(2 * q + 1, c0) + nsub * CHUNK],
                        src, 1.0, 0.0, ALU.mult, ALU.add,
                        accum_out=ps2[:, 2 * it + 1 : 2 * it + 2])
                    nc.vector.drain()
                    vector.wait_ge(s_ev2, it + 1)
                    for half, slot in ((0, 2 * q), (1, 2 * q + 1)):
                        yc = y2v[:, ycol(slot, c0) :
                                 ycol(slot, c0) + nsub * CHUNK]
                        nc.vector.scalar_tensor_tensor(
                            out=scr16[:, 0 : nsub * CHUNK], in0=yc,
                            scalar=1.0, in1=yc,
                            op0=ALU.mult, op1=ALU.mult,
                            accum_out=pq2[:, 2 * it + half :
                                          2 * it + half + 1])
                    nc.vector.drain().then_inc(s_dv2, 1)
                    it += 1

            stats(ps2, pq2, s_st2, D_G2, 2, stats2, glob2, 1)

            # final w = y2 * a2 + tq
            vector.wait_ge(dsem, D_TQ)
            for j in range(NF):
                if j == 5:
                    continue
                sl, h = FINALS[j]
                off = sl * PERIMG + h * HCOLS
                if j >= 3:
                    vector.wait_ge(s_fs, j - 2)
                nc.vector.scalar_tensor_tensor(
                    out=wbuf[j % 3][:, 0:HCOLS],
                    in0=y2v[:, off : off + HCOLS],
                    scalar=glob2[:, 6:7],
                    in1=tq[:, off : off + HCOLS],
                    op0=ALU.mult, op1=ALU.add)
                nc.vector.drain().then_inc(s_fvd, 1)

        @block.gpsimd
        def _(gpsimd):
            nc.gpsimd.memset(dscr[:], 0).then_inc(s_ms, 1)
            for s in range(SLOTS):
                nc.gpsimd.memset(bin1[:, s], 0).then_inc(s_ms, 1)
            gpsimd.wait_ge(dsem, D_B1ST)
            if CC_STUB:
                nc.gpsimd.dma_start(db1_out[:], db1_in[:]).then_inc(s_cc, 16)
            else:
                nc.gpsimd.collective_compute(
                    "AllReduce", ALU.add, replica_groups=[list(range(N_CORES))],
                    ins=[db1_in[:]], outs=[db1_out[:]]).then_inc(s_cc, 1)
            gpsimd.wait_ge(dsem, D_B2ST)
            if CC_STUB:
                nc.gpsimd.dma_start(db2_out[:], db2_in[:]).then_inc(s_cc, 16)
            else:
                nc.gpsimd.collective_compute(
                    "AllReduce", ALU.add, replica_groups=[list(range(N_CORES))],
                    ins=[db2_in[:]], outs=[db2_out[:]]).then_inc(s_cc, 1)
            # final w for the last chunk: h = y2*a2, then + tq
            gpsimd.wait_ge(s_st2, 3)
            _sl, _h = FINALS[5]
            _off = _sl * PERIMG + _h * HCOLS
            nc.gpsimd.tensor_tensor(
                out=hbuf[:, 0:HCOLS], in0=y2v[:, _off : _off + HCOLS],
                in1=glob2[:, 6:7].broadcast_to((128, HCOLS)), op=ALU.mult)
            gpsimd.drain()
            nc.gpsimd.tensor_tensor(
                out=pbuf[:, 0:HCOLS], in0=hbuf[:, 0:HCOLS],
                in1=tq[:, _off : _off + HCOLS], op=ALU.add)
            gpsimd.drain().then_inc(s_fvp, 1)

    return nc


_CACHE = {}


def _get_nc():
    if "nc" not in _CACHE:
        _CACHE["nc"] = build_bass()
    return _CACHE["nc"]


def kernel(x, w1, gamma1, beta1, w2, gamma2, beta2):
    x = np.asarray(x, np.float32)
    w1 = np.asarray(w1, np.float32)
    w2 = np.asarray(w2, np.float32)
    gamma1 = np.asarray(gamma1, np.float32)
    beta1 = np.asarray(beta1, np.float32)
    gamma2 = np.asarray(gamma2, np.float32)
    beta2 = np.asarray(beta2, np.float32)

    f8np = mybir.dt.np(F8)

    # conv1 weights: [tap, cin, cout] -> [cin, tap*cout], rows duplicated
    wb1 = np.where(w1 >= 0, 1.0, -1.0).astype(np.float32)
    wt1a = wb1.transpose(1, 2, 3, 0).reshape(64, 9, 64)
    wt1 = wt1a.reshape(64, 576)
    wf16_np = np.concatenate([wt1, wt1], axis=0).astype(np.float16)
    # fp8 limb weights: block-diagonal +-2^-12 (exact e4m3 subnormal)
    w1m_np = np.zeros((128, 1280), np.float32)
    for t in range(9):
        w1m_np[0:64, t * 128 : t * 128 + 64] = wt1a[:, t, :] * 2.0 ** -12
        w1m_np[64:128, t * 128 + 64 : t * 128 + 128] = wt1a[:, t, :] * 2.0 ** -12

    # conv2 weights: fp8 block-diagonal, 10 taps (tap 9 zero)
    wb2 = np.where(w2 >= 0, 1.0, -1.0).astype(np.float32)
    wt2 = wb2.transpose(1, 2, 3, 0).reshape(64, 9, 64)  # [cin, tap, cout]
    w8_np = np.zeros((128, 1280), np.float32)
    for t in range(9):
        w8_np[0:64, t * 128 : t * 128 + 64] = wt2[:, t, :]
        w8_np[64:128, t * 128 + 64 : t * 128 + 128] = wt2[:, t, :]
    w8_np = w8_np.astype(f8np)
    w1m_np = w1m_np.astype(mybir.dt.np(F8E5))

    consts_np = np.zeros((128, 8), np.float32)
    for col, v in enumerate([gamma1, beta1, gamma2, beta2]):
        consts_np[0:64, col] = v
        consts_np[64:128, col] = v
    consts_np[:, 4] = EPS

    in_maps = []
    for k in range(N_CORES):
        xc = x[IMGS * k : IMGS * (k + 1)]            # [8, 64, 56, 56]
        xp = np.zeros((IMGS, C, HP, HP), np.float32)
        xp[:, :, 1 : 1 + H, 1 : 1 + W] = xc
        arr = xp.reshape(SLOTS, 2, C, HP, HP).transpose(1, 2, 0, 3, 4)
        arr = np.ascontiguousarray(arr).reshape(128, SLOTS, HP, HP)
        ahi = arr.astype(np.float16)
        r1 = arr - ahi.astype(np.float32)
        m8 = (r1 * 4096.0).astype(f8np)
        r2 = r1 - m8.astype(np.float32) / 4096.0
        l8 = (r2 * 4096.0).astype(f8np)
        # Q-pack the fp8 limbs so the block-diagonal limb matmuls land in the
        # same (permuted) arrangement the hi quadrant pass produces
        m8q = np.empty_like(m8)
        l8q = np.empty_like(l8)
        for s in range(SLOTS):
            for h in (0, 1):
                img = 4 * (s // 2) + (s % 2) + 2 * h
                sp, hp_ = img // 2, img % 2
                m8q[h * 64 : h * 64 + 64, s] = m8[hp_ * 64 : hp_ * 64 + 64, sp]
                l8q[h * 64 : h * 64 + 64, s] = l8[hp_ * 64 : hp_ * 64 + 64, sp]
        # conv1's quadrant pattern permutes (slot, half): y slot s half h holds
        # image Q(s,h) = 4*(s//2) + s%2 + 2*h. The final residual add needs x
        # in that same arrangement.
        tq_np = np.empty((128, SLOTS, PERIMG), np.float32)
        for s in range(SLOTS):
            for h in (0, 1):
                img = 4 * (s // 2) + (s % 2) + 2 * h
                tq_np[h * 64 : h * 64 + 64, s] = xc[img].reshape(C, PERIMG)
        in_maps.append({
            "xhi": ahi, "xm8": m8q, "xl8": l8q, "wf16": wf16_np,
            "w8": w8_np, "w1m": w1m_np, "consts": consts_np,
            "tq": tq_np.reshape(128, YCOLS),
        })

    nc = _get_nc()
    res = bass_utils.run_bass_kernel_spmd(nc, in_maps, core_ids=list(range(N_CORES)))

    out = np.empty((N, C, H, W), np.float32)
    for k in range(N_CORES):
        o = np.asarray(res.results[k]["outp"]).astype(np.float32)  # [128, 12544]
        o = o.reshape(2, C, SLOTS, H, W)  # [half, ch, slot, H, W]
        for s in range(SLOTS):
            for h in (0, 1):
                img = 4 * (s // 2) + (s % 2) + 2 * h
                out[IMGS * k + img] = o[h, :, s]
    return out


if __name__ == "__main__":
    rng = np.random.default_rng(0)
    xs = rng.standard_normal((N, C, H, W)).astype(np.float32)
    w1s = (rng.standard_normal((C, C, 3, 3)) * 0.1).astype(np.float32)
    w2s = (rng.standard_normal((C, C, 3, 3)) * 0.1).astype(np.float32)
    ones = np.ones(C, np.float32)
    zeros = np.zeros(C, np.float32)
    r = kernel(x=xs, w1=w1s, gamma1=ones, beta1=zeros, w2=w2s, gamma2=ones,
               beta2=zeros)
    print("ran, out uniq:", np.unique(r))
